# revision 39
# baseline (speedup 1.0000x reference)
"""Trainium2 Bass kernel for a GQA attention layer (dense transformer).

Reference computation (B=1, S=2048, DIM=2048, 32 q-heads, 8 kv-heads, hd=64):
    xq = x @ wq; xk = x @ wk; xv = x @ wv
    rope(xq, xk); GQA causal attention; out = attn @ wo

Sharding: tensor-parallel over heads across 8 cores. Core c owns q-heads
4c..4c+3 (wq cols), kv-head c (wk/wv cols), and wo rows 256c..256c+255.
Each core computes a full [S, DIM] partial of the output projection; the
host sums the 8 partials (the TP all-reduce, done at gather time).

Kernel layout strategy (everything "transposed", head_dim on partitions):
  - QT/KT/VT = W.T @ x computed with lhsT = weight shard (natural [DIM, m]
    layout), rhs = x.T tiles streamed from DRAM.
  - RoPE pairs are de-interleaved by permuting wq/wk columns on the host so
    the rotation partner sits 16 partitions away (within a 32-partition
    quadrant, so DVE stream_shuffle can swap them).
  - Scores are computed transposed: S^T[k, q] = K^T.T @ Q^T per 128-row
    k-tile; exp on ACT (scale fused); causal mask = upper-tri multiply on
    the single diagonal 128x128 block of each k-tile (on GPSIMD).
  - P@V runs q-major: out[q,hd] = sum_k P^T[k,q].T @ Vp[k,hd|1] per
    (k-tile, 128-q subtile) with N=65 moving columns - 65 PE cycles per
    k-tile instead of 512, full 128x128 array use. The ones column makes
    psum col 64 the softmax denominator.
  - Normalize: one reciprocal per head ([128,4]), then tensor_scalar_mul
    psum->SBUF (per-partition scalar = per-q denom) into O_sb, then PE
    transposes [128q,64] -> [64,128q] (bf16 psum) and 2x-speed DVE copies
    into OT for the output projection.
  - Output projection from O^T with wo shard as rhs; partials DMA'd fp16.
"""

import numpy as np
import ml_dtypes

import concourse.bass as bass
import concourse.mybir as mybir
from concourse import bacc
from concourse.tile import TileContext
from concourse.masks import make_identity
from concourse.bass_utils import run_bass_kernel_spmd

# ---------------------------------------------------------------- constants
S = 2048          # sequence length
DIM = 2048        # model dim
NH = 32           # query heads
NKV = 8           # kv heads
HD = 64           # head dim
NCORES = 8
HQ = NH // NCORES          # query heads per core = 4
QW = HQ * HD               # q width per core = 256
KT_S = S // 128            # 16 seq k-tiles
KT_D = DIM // 128          # 16 dim k-tiles
NSC = S // 512             # 4 s-chunks
SCALE = 1.0 / 8.0          # 1/sqrt(64)
HD1 = HD + 1               # V tile width incl ones column

# matmul dtype knob: 'bf16' | 'f32' | 'f32r'
MM = 'bf16'
MASK_POOL = True           # causal masks on GPSIMD (else DVE)
NWARM = 12                 # p-state warmup transposes
ACT_COPY_EVERY = 4         # every Nth WO psum->sbuf copy goes to ACT

_SHUF_SWAP16 = list(range(16, 32)) + list(range(16))


def _dtypes():
    if MM == 'bf16':
        return mybir.dt.bfloat16, mybir.dt.bfloat16, ml_dtypes.bfloat16
    if MM == 'f32':
        return mybir.dt.float32, mybir.dt.float32, np.float32
    if MM == 'f32r':
        return mybir.dt.float32, mybir.dt.float32r, np.float32
    raise ValueError(MM)


def _mm_ap(ap, mmdt):
    """View an AP in the matmul dtype (bitcast f32 -> f32r when needed)."""
    if ap.dtype != mmdt:
        return ap.bitcast(mmdt)
    return ap


def build_program():
    """Build the per-core Bass program (same program on all 8 cores).

    Emission is a fine-grained software pipeline: attention beats for chunk
    sc (S^T mega-matmul for head h + PV beats of head h-1) are merged with
    the projection matmuls of chunk sc+1 and the WO units of chunk sc-1.

    PSUM (8 banks): pjo 3 (projection passes + PV accumulators, shared tag)
    + ps 4 (two [128,1024] score megas) + pw 1 (WO + transposes).
    """
    sdt, mmdt, _ = _dtypes()
    f32 = mybir.dt.float32
    f16 = mybir.dt.float16

    nc = bacc.Bacc("TRN2", target_bir_lowering=False, debug=False,
                   num_devices=NCORES)

    xT = nc.dram_tensor("xT", [DIM, S], sdt, kind="ExternalInput")
    wqkv = nc.dram_tensor("wqkv", [DIM, QW + 2 * HD], sdt,
                          kind="ExternalInput")
    wo_s = nc.dram_tensor("wo_s", [QW, DIM], sdt, kind="ExternalInput")
    cosE = nc.dram_tensor("cosE", [64, S], f32, kind="ExternalInput")
    sinE = nc.dram_tensor("sinE", [64, S], f32, kind="ExternalInput")
    utri = nc.dram_tensor("utri", [128, 128], sdt, kind="ExternalInput")
    out = nc.dram_tensor("out", [S, DIM], f16, kind="ExternalOutput")

    WQKV = QW + 2 * HD  # 384

    import contextlib
    with TileContext(nc) as tc, contextlib.ExitStack() as ctx:
        const = ctx.enter_context(tc.tile_pool(name="const", bufs=1))
        work = ctx.enter_context(tc.tile_pool(name="work", bufs=2))
        xtp = ctx.enter_context(tc.tile_pool(name="xtp", bufs=10))
        ptp = ctx.enter_context(tc.tile_pool(name="ptp", bufs=20))
        small = ctx.enter_context(tc.tile_pool(name="small", bufs=5))
        osb = ctx.enter_context(tc.tile_pool(name="osb", bufs=4))
        opool = ctx.enter_context(tc.tile_pool(name="opool", bufs=2))

        pjo = ctx.enter_context(tc.tile_pool(name="pjo", bufs=3,
                                             space="PSUM"))
        ps = ctx.enter_context(tc.tile_pool(name="ps", bufs=2, space="PSUM"))
        pw = ctx.enter_context(tc.tile_pool(name="pw", bufs=1, space="PSUM"))

        # ----------------------------------------------- persistent SBUF
        w_sb = const.tile([128, KT_D * WQKV], sdt, tag="w_sb")
        wo_sb = const.tile([128, 2 * DIM], sdt, tag="wo_sb")
        cos_sb = const.tile([128, S], f32, tag="cos_sb")
        sin_sb = const.tile([128, S], f32, tag="sin_sb")
        utri_sb = const.tile([128, 128], sdt, tag="utri_sb")
        ident = const.tile([128, 128], sdt, tag="ident")
        QT = const.tile([64, HQ * S], sdt, tag="QT")
        KVt = const.tile([128, S], sdt, tag="KVt")
        Vp = const.tile([128, KT_S * HD1], sdt, tag="Vp")
        OT = const.tile([128, 2 * S], sdt, tag="OT")

        # p-state warmup: keep PE streaming during the initial DMA wait so
        # the frequency ramp (3us) is over before real matmuls start; the
        # source tile only needs to hold valid bits, so a fast DVE memset
        # unblocks the first transpose ~200ns in
        wsrc = const.tile([128, 128], sdt, tag="wsrc")
        nc.vector.memset(wsrc[:], 0.0)
        warm = pw.tile([128, 128], sdt, tag="pw", name="warm")
        for _ in range(NWARM):
            nc.tensor.transpose(warm[:], wsrc[:], wsrc[:])

        make_identity(nc, ident[:])
        nc.gpsimd.memset(Vp[:], 1.0)  # ones columns for denominator

        wo_copy_cnt = [0]

        # ---------------------------------------------- thunk generators
        def proj_thunks(sc, fused=False):
            """Projection of chunk sc: KV pass, K-rope, V transposes, then
            Q passes (one PSUM slot each, sequential). With fused=True
            (prologue) all three matmuls run per k-tile, using 3 slots."""
            s0 = sc * 512
            xts = []
            st = {}

            # small first batches so the first matmul starts early; bigger
            # later ones amortize HWDGE overhead
            batches = [2, 2, 4, 4, 4] if sc == 0 else [4, 4, 4, 4]
            starts = [sum(batches[:i]) for i in range(len(batches))]
            kt_slot = {}
            for bi, (b0, bn) in enumerate(zip(starts, batches)):
                for j in range(bn):
                    kt_slot[b0 + j] = (bi, j, bn, b0)

            def dma_kv(kt):
                bi, j, bn, b0 = kt_slot[kt]
                if j == 0:
                    if sc == 0:
                        nc.sync.dma_start(
                            w_sb[:, b0 * WQKV:(b0 + bn) * WQKV].rearrange(
                                "r (k w) -> r k w", k=bn),
                            wqkv[b0 * 128:(b0 + bn) * 128, :].rearrange(
                                "(k r) w -> r k w", k=bn))
                    xt4 = xtp.tile([128, 4 * 512], sdt, tag="xt", name="xt4")
                    nc.sync.dma_start(
                        xt4[:, 0:bn * 512].rearrange("r (k c) -> r k c", k=bn),
                        xT[b0 * 128:(b0 + bn) * 128,
                           s0:s0 + 512].rearrange("(k r) c -> r k c", k=bn))
                    xts.append(xt4)
                    if sc == 0 and kt == 8:
                        # cos rides late in chunk 0 (needed by k_rope only
                        # after the full KV pass); rows [64:128] are a copy
                        # of [0:64] (64-row periodicity) done on idle ACT
                        nc.sync.dma_start(cos_sb[0:64, :], cosE[:])
                    if sc == 0 and kt == 12:
                        nc.sync.dma_start(sin_sb[0:64, :], sinE[:])
                        nc.sync.dma_start(utri_sb[:], utri[:])
                bi, j, bn, b0 = kt_slot[kt]
                xt = xts[bi][:, j * 512:j * 512 + 512]
                if kt == 0:
                    st["pkv"] = pjo.tile([128, 512], f32, tag="pjo",
                                         name="pkv")
                    if fused:
                        st["fq0"] = pjo.tile([128, 512], f32, tag="pjo",
                                             name="fq0")
                        st["fq1"] = pjo.tile([128, 512], f32, tag="pjo",
                                             name="fq1")
                nc.tensor.matmul(
                    st["pkv"][:],
                    _mm_ap(w_sb[:, kt * WQKV + 256:kt * WQKV + 384], mmdt),
                    _mm_ap(xt, mmdt),
                    start=(kt == 0), stop=(kt == KT_D - 1))
                if fused:
                    for mt in range(2):
                        nc.tensor.matmul(
                            st[f"fq{mt}"][:],
                            _mm_ap(w_sb[:, kt * WQKV + mt * 128:
                                        kt * WQKV + mt * 128 + 128], mmdt),
                            _mm_ap(xt, mmdt),
                            start=(kt == 0), stop=(kt == KT_D - 1))

            def k_rope():
                pkv = st["pkv"]
                shufk = work.tile([64, 512], f32, tag="shufk", name="shufk")
                m1k = work.tile([64, 512], f32, tag="m1k", name="m1k")
                t2k = work.tile([64, 512], f32, tag="t2k", name="t2k")
                nc.vector.stream_shuffle(shufk[:], pkv[0:64, :],
                                         _SHUF_SWAP16)
                nc.vector.tensor_mul(m1k[:], pkv[0:64, :],
                                     cos_sb[0:64, s0:s0 + 512])
                nc.vector.tensor_mul(t2k[:], shufk[:],
                                     sin_sb[0:64, s0:s0 + 512])
                nc.vector.tensor_add(KVt[0:64, s0:s0 + 512], m1k[:], t2k[:])
                nc.vector.tensor_copy(KVt[64:128, s0:s0 + 512],
                                      pkv[64:128, :])

            def v_trans(kt):
                pv = pw.tile([128, HD], sdt, tag="pw", name="pv")
                nc.tensor.transpose(
                    pv[:], KVt[64:128, kt * 128:(kt + 1) * 128],
                    ident[64:128, 64:128])
                nc.vector.tensor_copy(
                    Vp[:, kt * HD1:kt * HD1 + HD], pv[:])

            def q_mm(mt, kt):
                if kt == 0:
                    st["pq"] = pjo.tile([128, 512], f32, tag="pjo",
                                        name="pq")
                w0 = kt * WQKV + mt * 128
                bi, j, bn, b0 = kt_slot[kt]
                xt = xts[bi][:, j * 512:j * 512 + 512]
                nc.tensor.matmul(
                    st["pq"][:], _mm_ap(w_sb[:, w0:w0 + 128], mmdt),
                    _mm_ap(xt, mmdt),
                    start=(kt == 0), stop=(kt == KT_D - 1))

            def q_rope(mt):
                pq = st[f"fq{mt}"] if fused else st["pq"]
                shuf = work.tile([128, 512], f32, tag="shuf", name="shuf")
                m1 = work.tile([128, 512], f32, tag="m1", name="m1")
                t2 = work.tile([128, 512], f32, tag="t2", name="t2")
                nc.vector.stream_shuffle(shuf[:], pq[:], _SHUF_SWAP16)
                nc.vector.tensor_mul(m1[:], pq[:], cos_sb[:, s0:s0 + 512])
                nc.vector.tensor_mul(t2[:], shuf[:], sin_sb[:, s0:s0 + 512])
                he = (2 * mt) * S
                ho = (2 * mt + 1) * S
                nc.vector.tensor_add(
                    QT[:, he + s0:he + s0 + 512], m1[0:64, :], t2[0:64, :])
                nc.vector.tensor_add(
                    QT[:, ho + s0:ho + s0 + 512], m1[64:128, :],
                    t2[64:128, :])

            def cs_dup():
                # duplicate cos/sin rows [0:64] into [64:128] for q_rope
                # (cos on idle ACT, sin on idle GPSIMD, in parallel)
                nc.scalar.copy(cos_sb[64:128, :], cos_sb[0:64, :])
                nc.gpsimd.tensor_copy(sin_sb[64:128, :], sin_sb[0:64, :])

            th = [lambda kt=kt: dma_kv(kt) for kt in range(KT_D)]
            th.append(k_rope)
            if sc == 0:
                th.append(cs_dup)
            th += [lambda kt=kt: v_trans(kt)
                   for kt in range(4 * sc, 4 * sc + 4)]
            if fused:
                th += [lambda mt=mt: q_rope(mt) for mt in range(2)]
            else:
                for mt in range(2):
                    th += [lambda mt=mt, kt=kt: q_mm(mt, kt)
                           for kt in range(KT_D)]
                    th.append(lambda mt=mt: q_rope(mt))
            return th

        def s_thunks(qc, h, tiles):
            """S^T mega matmuls + exp + mask for one head; fills `tiles`."""
            q0 = qc * 512
            hf = h * S
            nkt = 4 * qc + 4
            thunks = []
            for pi in range(nkt // 2):
                def th(pi=pi):
                    kts = (2 * pi, 2 * pi + 1)
                    ps_t = ps.tile([128, 1024], f32, tag="ps", name="ps_t")
                    pt_t = ptp.tile([128, 1024], sdt, tag="pt", name="pt_t")
                    for li, kt in enumerate(kts):
                        dj = kt - 4 * qc
                        qo = 128 * dj if dj >= 0 else 0
                        lo = li * 512
                        nc.tensor.matmul(
                            ps_t[:, lo + qo:lo + 512],
                            _mm_ap(KVt[0:64, kt * 128:(kt + 1) * 128], mmdt),
                            _mm_ap(QT[:, hf + q0 + qo:hf + q0 + 512], mmdt),
                            start=True, stop=True)
                    if 2 * pi + 1 < 4 * qc:
                        nc.scalar.activation(
                            pt_t[:], ps_t[:],
                            mybir.ActivationFunctionType.Exp, scale=SCALE)
                    else:
                        for li, kt in enumerate(kts):
                            dj = kt - 4 * qc
                            qo = 128 * dj if dj >= 0 else 0
                            lo = li * 512
                            nc.scalar.activation(
                                pt_t[:, lo + qo:lo + 512],
                                ps_t[:, lo + qo:lo + 512],
                                mybir.ActivationFunctionType.Exp,
                                scale=SCALE)
                    for li, kt in enumerate(kts):
                        dj = kt - 4 * qc
                        qo = 128 * dj if dj >= 0 else 0
                        lo = li * 512
                        if dj >= 0:
                            eng = nc.gpsimd if MASK_POOL else nc.vector
                            eng.tensor_mul(
                                pt_t[:, lo + qo:lo + qo + 128],
                                pt_t[:, lo + qo:lo + qo + 128], utri_sb[:])
                        tiles.append((kt, qo, lo, pt_t))
                thunks.append(th)
            return thunks

        def pv_thunks(qc, h, tiles):
            """q-major PV accumulation + normalization + O transposes.

            For each incoming P^T tile (k-tile kt), run the N=65 matmuls for
            each live 128-q subtile j: out[q,hd|den] += P^T[:,j*128:].T@Vp.
            After the last k-tile: reciprocal of the denominators, normalize
            psum->O_sb (bf16), PE-transpose to O^T, 2x DVE copy into OT.
            """
            q0 = qc * 512
            hp = (h % 2) * 64
            of = (h // 2) * S
            nkt0 = 4 * qc + 4
            state = {}

            def pv_series(j):
                """All matmuls of q-subtile j back-to-back: start=True
                clears the whole bank's has_written bits, so accumulation
                series sharing a psum bank must not interleave."""
                if "pv" not in state:
                    state["pv"] = pjo.tile([128, 512], f32, tag="pjo",
                                           name="pvt")
                pv_t = state["pv"]
                for kt in range(0, 4 * qc + j + 1):
                    _, qo, lo, pt_t = tiles[kt]
                    nc.tensor.matmul(
                        pv_t[:, j * 128:j * 128 + HD1],
                        _mm_ap(pt_t[:, lo + j * 128:
                                    lo + (j + 1) * 128], mmdt),
                        _mm_ap(Vp[:, kt * HD1:(kt + 1) * HD1], mmdt),
                        start=(kt == 0), stop=(kt == 4 * qc + j))

            def norm():
                pv_t = state["pv"]
                rc = small.tile([128, 4], f32, tag="rc", name="rc")
                dsb = small.tile([128, 4], f32, tag="dsb", name="dsb")
                o_sb = state["o_sb"] = osb_head()
                pvr = pv_t.rearrange("p (b c) -> p b c", c=128)
                nc.vector.tensor_copy(
                    dsb[:].rearrange("p (b c) -> p b c", c=1),
                    pvr[:, :, HD:HD + 1])
                nc.vector.reciprocal(rc[:], dsb[:])
                for j in range(4):
                    nc.vector.tensor_scalar_mul(
                        o_sb[:, j * HD:(j + 1) * HD],
                        pv_t[:, j * 128:j * 128 + HD],
                        rc[:, j:j + 1])

            def trans(half):
                # transpose via a regular matmul against the identity:
                # out[hd, q] = o_sb[q, hd]^T @ I -- f32 psum output, so
                # nonzero free offsets behave like the score megas
                o_sb = state["o_sb"]
                tp = pw.tile([64, 256], f32, tag="pw", name="tp")
                for st2 in range(2):
                    stg = 2 * half + st2
                    nc.tensor.matmul(
                        tp[:, st2 * 128:(st2 + 1) * 128],
                        _mm_ap(o_sb[:, stg * HD:(stg + 1) * HD], mmdt),
                        _mm_ap(ident[:], mmdt),
                        start=True, stop=True)
                nc.vector.tensor_copy(
                    OT[hp:hp + 64, of + q0 + half * 256:
                       of + q0 + half * 256 + 256], tp[:])

            def osb_head():
                return opool.tile([128, 4 * HD], sdt, tag=f"osb{h % 2}",
                                  name="o_sb")

            th = [lambda j=j: pv_series(j) for j in range(4)]
            th.append(norm)
            th += [lambda half=half: trans(half) for half in range(2)]
            return th

        def wo_half(qt, np2, half, obs, pool=None, ptag="pw",
                    copy_eng="mix", split_dma=False):
            """One 512-wide n-chunk of out row-block qt; the final chunk
            fires one [128,2048] fp16 DMA for the whole row-block (fewer
            HWDGE round-trips than per-chunk DMAs). The epilogue splits
            per-np2 ([128,1024]) so the last transfer is shorter."""
            pool = pool or pw
            if (np2, half) == (0, 0):
                obs[qt] = osb.tile([128, 2048], f16, tag="ob", name="ob")
            ob = obs[qt]
            ncn = 2 * np2 + half
            pw_t = pool.tile([128, 512], f32, tag=ptag, name="pw_t")
            for mt in range(2):
                nc.tensor.matmul(
                    pw_t[:],
                    _mm_ap(OT[:, mt * S + qt * 128:
                              mt * S + (qt + 1) * 128], mmdt),
                    _mm_ap(wo_sb[:, mt * DIM + ncn * 512:
                                 mt * DIM + ncn * 512 + 512], mmdt),
                    start=(mt == 0), stop=(mt == 1))
            wo_copy_cnt[0] += 1
            use_act = {"mix": wo_copy_cnt[0] % 3 == 0,
                       "dve": False,
                       "alt": wo_copy_cnt[0] % 2 == 1}[copy_eng]
            if use_act:
                nc.scalar.copy(ob[:, ncn * 512:ncn * 512 + 512], pw_t[:])
            else:
                nc.vector.tensor_copy(
                    ob[:, ncn * 512:ncn * 512 + 512], pw_t[:])
            if split_dma and half == 1:
                nc.sync.dma_start(
                    out[qt * 128:(qt + 1) * 128,
                        np2 * 1024:np2 * 1024 + 1024],
                    ob[:, np2 * 1024:np2 * 1024 + 1024])
                if np2 == 1:
                    del obs[qt]
            elif not split_dma and (np2, half) == (1, 1):
                del obs[qt]
                nc.sync.dma_start(
                    out[qt * 128:(qt + 1) * 128, :], ob[:])
        wo_obs = {}

        # ------------------------------------- merged emission schedule
        def merge(primary, *others):
            """Emit primary thunks; proportionally interleave the others."""
            counters = [0.0] * len(others)
            n = max(1, len(primary))
            for beat in primary:
                for j, lst in enumerate(others):
                    counters[j] += len(lst) / n
                    while counters[j] >= 1.0 and lst:
                        lst.pop(0)()
                        counters[j] -= 1.0
                for th in beat:
                    th()
            for lst in others:
                while lst:
                    lst.pop(0)()

        for th in proj_thunks(0, fused=True):       # prologue
            th()

        prev = None                      # (qc, h, tiles) awaiting PV
        for sc in range(NSC):
            if sc == 1:
                nc.sync.dma_start(wo_sb[:, 0:DIM], wo_s[0:128, :])
                nc.sync.dma_start(wo_sb[:, DIM:2 * DIM], wo_s[128:256, :])
            pstream = proj_thunks(sc + 1) if sc + 1 < NSC else []
            # in the last window there is no projection work: alternate WO
            # psum between the mostly-idle pjo pool and pw so WO units are
            # not gated on a single bank's copy drain, and keep its copies
            # off the exp-saturated ACT engine
            if sc == NSC - 1:
                wpp = [(pjo, "pjo", "dve")]
            else:
                wpp = [(None, "pw", "mix")]
            wostream = ([lambda qt=qt, np2=np2, half=half,
                         w=wpp[(2 * np2 + half) % len(wpp)]:
                         wo_half(qt, np2, half, wo_obs, pool=w[0],
                                 ptag=w[1], copy_eng=w[2])
                         for qt in range(4 * (sc - 1), 4 * (sc - 1) + 4)
                         for np2 in range(2)
                         for half in range(2)] if sc >= 1 else [])
            for h in range(HQ):
                tiles = []
                sth = s_thunks(sc, h, tiles)
                pth = pv_thunks(*prev) if prev is not None else []
                beats = []
                for i in range(max(len(sth), len(pth))):
                    beat = []
                    if i < len(pth):
                        beat.append(pth[i])
                    if i < len(sth):
                        beat.append(sth[i])
                    beats.append(beat)
                # WO of sc-1 needs PV(sc-1, 3) done: that PV is head 0 here
                if h == 0:
                    ptake = max(1, len(pstream) // HQ) if pstream else 0
                    merge(beats, pstream[:ptake])
                    pstream = pstream[ptake:]
                else:
                    ptake = (len(pstream) // (HQ - h)) if pstream else 0
                    wtake = (len(wostream) // (HQ - h)) if wostream else 0
                    merge(beats, pstream[:ptake], wostream[:wtake])
                    pstream = pstream[ptake:]
                    wostream = wostream[wtake:]
                prev = (sc, h, tiles)
            merge([], pstream, wostream)

        # epilogue: PV of the last head, then WO of chunk 3; the score
        # pool's banks are free now, so WO rotates through those too.
        # trans(0) covers q-tiles 12-13, trans(1) covers 14-15: start the
        # WO units for each pair as soon as its OT columns are complete.
        pth = pv_thunks(*prev)
        for th in pth[:-1]:          # pairs + norm + trans(0)
            th()
        epi = [0]
        pools = [(pw, "pw"), (ps, "ps"), (pjo, "pjo")]

        def epi_wo(qt, np2):
            pool, ptag = pools[epi[0] % 3]
            for half in range(2):
                wo_half(qt, np2, half, wo_obs, pool=pool, ptag=ptag,
                        copy_eng="mix", split_dma=True)
            epi[0] += 1

        epi_wo(12, 0)
        pth[-1]()                    # trans(1) overlaps qt-12 copies
        epi_wo(12, 1)
        for qt in range(13, 16):
            for np2 in range(2):
                epi_wo(qt, np2)

    nc.compile()
    return nc


# ------------------------------------------------------------- host side
def _pair_perm64():
    """Column permutation putting the RoPE partner 16 partitions away."""
    return np.array([2 * (16 * (j // 32) + (j % 16)) + ((j % 32) // 16)
                     for j in range(64)])


def _host_prep(x, freqs_cos, freqs_sin, wq, wk, wv, wo):
    _, _, npdt = _dtypes()
    x = np.asarray(x, np.float32)
    fc = np.asarray(freqs_cos, np.float32)
    fs = np.asarray(freqs_sin, np.float32)
    wq = np.asarray(wq, np.float32)
    wk = np.asarray(wk, np.float32)
    wv = np.asarray(wv, np.float32)
    wo = np.asarray(wo, np.float32)

    perm = _pair_perm64()
    xT = np.ascontiguousarray(x[0].T).astype(npdt)

    p = np.arange(64)
    pair = 16 * ((p % 64) // 32) + (p % 16)
    sign = np.where((p % 32) < 16, -1.0, 1.0).astype(np.float32)
    cosE = np.ascontiguousarray(fc[:, pair].T)                  # [64, S]
    sinE = np.ascontiguousarray(fs[:, pair].T) * sign[:, None]  # [64, S]
    utri = np.triu(np.ones((128, 128), np.float32)).astype(npdt)

    in_maps = []
    for c in range(NCORES):
        qcols = np.concatenate(
            [wq[:, (4 * c + i) * 64 + perm] for i in range(HQ)], axis=1)
        kcols = wk[:, c * 64 + perm]
        vcols = wv[:, c * 64:(c + 1) * 64]
        wqkv_c = np.concatenate([qcols, kcols, vcols], axis=1).astype(npdt)
        wo_c = wo[QW * c:QW * (c + 1), :].astype(npdt)
        in_maps.append({
            "xT": xT,
            "wqkv": np.ascontiguousarray(wqkv_c),
            "wo_s": np.ascontiguousarray(wo_c),
            "cosE": cosE.astype(np.float32),
            "sinE": np.ascontiguousarray(sinE).astype(np.float32),
            "utri": np.ascontiguousarray(utri),
        })
    return in_maps


_NC_CACHE = {}


def get_program():
    if MM not in _NC_CACHE:
        _NC_CACHE[MM] = build_program()
    return _NC_CACHE[MM]


def kernel(x, freqs_cos, freqs_sin, wq, wk, wv, wo):
    nc = get_program()
    in_maps = _host_prep(x, freqs_cos, freqs_sin, wq, wk, wv, wo)
    res = run_bass_kernel_spmd(nc, in_maps, core_ids=list(range(NCORES)))
    acc = np.zeros((S, DIM), np.float64)
    for r in res.results:
        acc += r["out"].astype(np.float64)
    return acc.astype(np.float32).reshape(1, S, DIM)


# revision 44
# speedup vs baseline: 1.0118x; 1.0118x over previous
"""Trainium2 Bass kernel for a GQA attention layer (dense transformer).

Reference computation (B=1, S=2048, DIM=2048, 32 q-heads, 8 kv-heads, hd=64):
    xq = x @ wq; xk = x @ wk; xv = x @ wv
    rope(xq, xk); GQA causal attention; out = attn @ wo

Sharding: tensor-parallel over heads across 8 cores. Core c owns q-heads
4c..4c+3 (wq cols), kv-head c (wk/wv cols), and wo rows 256c..256c+255.
Each core computes a full [S, DIM] partial of the output projection; the
host sums the 8 partials (the TP all-reduce, done at gather time).

Kernel layout strategy (everything "transposed", head_dim on partitions):
  - QT/KT/VT = W.T @ x computed with lhsT = weight shard (natural [DIM, m]
    layout), rhs = x.T tiles streamed from DRAM.
  - RoPE pairs are de-interleaved by permuting wq/wk columns on the host so
    the rotation partner sits 16 partitions away (within a 32-partition
    quadrant, so DVE stream_shuffle can swap them).
  - Scores are computed transposed: S^T[k, q] = K^T.T @ Q^T per 128-row
    k-tile; exp on ACT (scale fused); causal mask = upper-tri multiply on
    the single diagonal 128x128 block of each k-tile (on GPSIMD).
  - P@V runs q-major: out[q,hd] = sum_k P^T[k,q].T @ Vp[k,hd|1] per
    (k-tile, 128-q subtile) with N=65 moving columns - 65 PE cycles per
    k-tile instead of 512, full 128x128 array use. The ones column makes
    psum col 64 the softmax denominator.
  - Normalize: one reciprocal per head ([128,4]), then tensor_scalar_mul
    psum->SBUF (per-partition scalar = per-q denom) into O_sb, then PE
    transposes [128q,64] -> [64,128q] (bf16 psum) and 2x-speed DVE copies
    into OT for the output projection.
  - Output projection from O^T with wo shard as rhs; partials DMA'd fp16.
"""

import numpy as np
import ml_dtypes

import concourse.bass as bass
import concourse.mybir as mybir
from concourse import bacc
from concourse.tile import TileContext
from concourse.masks import make_identity
from concourse.bass_utils import run_bass_kernel_spmd

# ---------------------------------------------------------------- constants
S = 2048          # sequence length
DIM = 2048        # model dim
NH = 32           # query heads
NKV = 8           # kv heads
HD = 64           # head dim
NCORES = 8
HQ = NH // NCORES          # query heads per core = 4
QW = HQ * HD               # q width per core = 256
KT_S = S // 128            # 16 seq k-tiles
KT_D = DIM // 128          # 16 dim k-tiles
NSC = S // 512             # 4 s-chunks
SCALE = 1.0 / 8.0          # 1/sqrt(64)
HD1 = HD + 1               # V tile width incl ones column

# matmul dtype knob: 'bf16' | 'f32' | 'f32r'
MM = 'bf16'
MASK_POOL = True           # causal masks on GPSIMD (else DVE)
NWARM = 12                 # p-state warmup transposes
ACT_COPY_EVERY = 4         # every Nth WO psum->sbuf copy goes to ACT

_SHUF_SWAP16 = list(range(16, 32)) + list(range(16))


def _dtypes():
    if MM == 'bf16':
        return mybir.dt.bfloat16, mybir.dt.bfloat16, ml_dtypes.bfloat16
    if MM == 'f32':
        return mybir.dt.float32, mybir.dt.float32, np.float32
    if MM == 'f32r':
        return mybir.dt.float32, mybir.dt.float32r, np.float32
    raise ValueError(MM)


def _mm_ap(ap, mmdt):
    """View an AP in the matmul dtype (bitcast f32 -> f32r when needed)."""
    if ap.dtype != mmdt:
        return ap.bitcast(mmdt)
    return ap


def build_program():
    """Build the per-core Bass program (same program on all 8 cores).

    Emission is a fine-grained software pipeline: attention beats for chunk
    sc (S^T mega-matmul for head h + PV beats of head h-1) are merged with
    the projection matmuls of chunk sc+1 and the WO units of chunk sc-1.

    PSUM (8 banks): pjo 3 (projection passes + PV accumulators, shared tag)
    + ps 4 (two [128,1024] score megas) + pw 1 (WO + transposes).
    """
    sdt, mmdt, _ = _dtypes()
    f32 = mybir.dt.float32
    f16 = mybir.dt.float16

    nc = bacc.Bacc("TRN2", target_bir_lowering=False, debug=False,
                   num_devices=NCORES)

    xT = nc.dram_tensor("xT", [DIM, S], sdt, kind="ExternalInput")
    wqkv = nc.dram_tensor("wqkv", [DIM, QW + 2 * HD], sdt,
                          kind="ExternalInput")
    wo_s = nc.dram_tensor("wo_s", [QW, DIM], sdt, kind="ExternalInput")
    cosE = nc.dram_tensor("cosE", [64, S], f32, kind="ExternalInput")
    sinE = nc.dram_tensor("sinE", [64, S], f32, kind="ExternalInput")
    utri = nc.dram_tensor("utri", [128, 128], sdt, kind="ExternalInput")
    out = nc.dram_tensor("out", [S, DIM], f16, kind="ExternalOutput")

    WQKV = QW + 2 * HD  # 384

    import contextlib
    with TileContext(nc) as tc, contextlib.ExitStack() as ctx:
        const = ctx.enter_context(tc.tile_pool(name="const", bufs=1))
        work = ctx.enter_context(tc.tile_pool(name="work", bufs=2))
        xtp = ctx.enter_context(tc.tile_pool(name="xtp", bufs=10))
        ptp = ctx.enter_context(tc.tile_pool(name="ptp", bufs=20))
        small = ctx.enter_context(tc.tile_pool(name="small", bufs=5))
        osb = ctx.enter_context(tc.tile_pool(name="osb", bufs=4))
        opool = ctx.enter_context(tc.tile_pool(name="opool", bufs=2))

        pjo = ctx.enter_context(tc.tile_pool(name="pjo", bufs=3,
                                             space="PSUM"))
        ps = ctx.enter_context(tc.tile_pool(name="ps", bufs=2, space="PSUM"))
        pw = ctx.enter_context(tc.tile_pool(name="pw", bufs=1, space="PSUM"))

        # ----------------------------------------------- persistent SBUF
        w_sb = const.tile([128, KT_D * WQKV], sdt, tag="w_sb")
        wo_sb = const.tile([128, 2 * DIM], sdt, tag="wo_sb")
        cos_sb = const.tile([128, S], f32, tag="cos_sb")
        sin_sb = const.tile([128, S], f32, tag="sin_sb")
        utri_sb = const.tile([128, 128], sdt, tag="utri_sb")
        ident = const.tile([128, 128], sdt, tag="ident")
        QT = const.tile([64, HQ * S], sdt, tag="QT")
        KVt = const.tile([128, S], sdt, tag="KVt")
        Vp = const.tile([128, KT_S * HD1], sdt, tag="Vp")
        OT = const.tile([128, 2 * S], sdt, tag="OT")

        # p-state warmup: keep PE streaming during the initial DMA wait so
        # the frequency ramp (3us) is over before real matmuls start; the
        # source tile only needs to hold valid bits, so a fast DVE memset
        # unblocks the first transpose ~200ns in
        wsrc = const.tile([128, 128], sdt, tag="wsrc")
        nc.vector.memset(wsrc[:], 0.0)
        warm = pw.tile([128, 128], sdt, tag="pw", name="warm")
        for _ in range(NWARM):
            nc.tensor.transpose(warm[:], wsrc[:], wsrc[:])

        make_identity(nc, ident[:])
        nc.gpsimd.memset(Vp[:], 1.0)  # ones columns for denominator

        wo_copy_cnt = [0]

        # ---------------------------------------------- thunk generators
        def proj_thunks(sc, fused=False):
            """Projection of chunk sc: KV pass, K-rope, V transposes, then
            Q passes (one PSUM slot each, sequential). With fused=True
            (prologue) all three matmuls run per k-tile, using 3 slots."""
            s0 = sc * 512
            xts = []
            st = {}

            # small first batches so the first matmul starts early; bigger
            # later ones amortize HWDGE overhead
            batches = [2, 2, 4, 4, 4] if sc == 0 else [4, 4, 4, 4]
            starts = [sum(batches[:i]) for i in range(len(batches))]
            kt_slot = {}
            for bi, (b0, bn) in enumerate(zip(starts, batches)):
                for j in range(bn):
                    kt_slot[b0 + j] = (bi, j, bn, b0)

            def dma_kv(kt):
                bi, j, bn, b0 = kt_slot[kt]
                if j == 0:
                    if sc == 0:
                        nc.sync.dma_start(
                            w_sb[:, b0 * WQKV:(b0 + bn) * WQKV].rearrange(
                                "r (k w) -> r k w", k=bn),
                            wqkv[b0 * 128:(b0 + bn) * 128, :].rearrange(
                                "(k r) w -> r k w", k=bn))
                    xt4 = xtp.tile([128, 4 * 512], sdt, tag="xt", name="xt4")
                    nc.sync.dma_start(
                        xt4[:, 0:bn * 512].rearrange("r (k c) -> r k c", k=bn),
                        xT[b0 * 128:(b0 + bn) * 128,
                           s0:s0 + 512].rearrange("(k r) c -> r k c", k=bn))
                    xts.append(xt4)
                    if sc == 0 and kt == 8:
                        # cos rides late in chunk 0 (needed by k_rope only
                        # after the full KV pass); rows [64:128] are a copy
                        # of [0:64] (64-row periodicity) done on idle ACT
                        nc.sync.dma_start(cos_sb[0:64, :], cosE[:])
                    if sc == 0 and kt == 12:
                        nc.sync.dma_start(sin_sb[0:64, :], sinE[:])
                        nc.sync.dma_start(utri_sb[:], utri[:])
                bi, j, bn, b0 = kt_slot[kt]
                xt = xts[bi][:, j * 512:j * 512 + 512]
                if kt == 0:
                    st["pkv"] = pjo.tile([128, 512], f32, tag="pjo",
                                         name="pkv")
                    if fused:
                        st["fq0"] = pjo.tile([128, 512], f32, tag="pjo",
                                             name="fq0")
                        st["fq1"] = pjo.tile([128, 512], f32, tag="pjo",
                                             name="fq1")
                nc.tensor.matmul(
                    st["pkv"][:],
                    _mm_ap(w_sb[:, kt * WQKV + 256:kt * WQKV + 384], mmdt),
                    _mm_ap(xt, mmdt),
                    start=(kt == 0), stop=(kt == KT_D - 1))
                if fused:
                    for mt in range(2):
                        nc.tensor.matmul(
                            st[f"fq{mt}"][:],
                            _mm_ap(w_sb[:, kt * WQKV + mt * 128:
                                        kt * WQKV + mt * 128 + 128], mmdt),
                            _mm_ap(xt, mmdt),
                            start=(kt == 0), stop=(kt == KT_D - 1))

            def k_rope():
                pkv = st["pkv"]
                shufk = work.tile([64, 512], f32, tag="shufk", name="shufk")
                m1k = work.tile([64, 512], f32, tag="m1k", name="m1k")
                t2k = work.tile([64, 512], f32, tag="t2k", name="t2k")
                nc.vector.stream_shuffle(shufk[:], pkv[0:64, :],
                                         _SHUF_SWAP16)
                nc.vector.tensor_mul(m1k[:], pkv[0:64, :],
                                     cos_sb[0:64, s0:s0 + 512])
                nc.vector.tensor_mul(t2k[:], shufk[:],
                                     sin_sb[0:64, s0:s0 + 512])
                nc.vector.tensor_add(KVt[0:64, s0:s0 + 512], m1k[:], t2k[:])
                nc.scalar.copy(KVt[64:128, s0:s0 + 512],
                               pkv[64:128, :])

            def v_trans(kt):
                pv = pw.tile([128, HD], sdt, tag="pw", name="pv")
                nc.tensor.transpose(
                    pv[:], KVt[64:128, kt * 128:(kt + 1) * 128],
                    ident[64:128, 64:128])
                nc.vector.tensor_copy(
                    Vp[:, kt * HD1:kt * HD1 + HD], pv[:])

            def q_mm(mt, kt):
                if kt == 0:
                    st["pq"] = pjo.tile([128, 512], f32, tag="pjo",
                                        name="pq")
                w0 = kt * WQKV + mt * 128
                bi, j, bn, b0 = kt_slot[kt]
                xt = xts[bi][:, j * 512:j * 512 + 512]
                nc.tensor.matmul(
                    st["pq"][:], _mm_ap(w_sb[:, w0:w0 + 128], mmdt),
                    _mm_ap(xt, mmdt),
                    start=(kt == 0), stop=(kt == KT_D - 1))

            def q_rope(mt):
                pq = st[f"fq{mt}"] if fused else st["pq"]
                shuf = work.tile([128, 512], f32, tag="shuf", name="shuf")
                m1 = work.tile([128, 512], f32, tag="m1", name="m1")
                t2 = work.tile([128, 512], f32, tag="t2", name="t2")
                nc.vector.stream_shuffle(shuf[:], pq[:], _SHUF_SWAP16)
                nc.vector.tensor_mul(m1[:], pq[:], cos_sb[:, s0:s0 + 512])
                nc.vector.tensor_mul(t2[:], shuf[:], sin_sb[:, s0:s0 + 512])
                he = (2 * mt) * S
                ho = (2 * mt + 1) * S
                nc.vector.tensor_add(
                    QT[:, he + s0:he + s0 + 512], m1[0:64, :], t2[0:64, :])
                nc.vector.tensor_add(
                    QT[:, ho + s0:ho + s0 + 512], m1[64:128, :],
                    t2[64:128, :])

            def cs_dup():
                # duplicate cos/sin rows [0:64] into [64:128] for q_rope
                # (cos on idle ACT, sin on idle GPSIMD, in parallel)
                nc.scalar.copy(cos_sb[64:128, :], cos_sb[0:64, :])
                nc.gpsimd.tensor_copy(sin_sb[64:128, :], sin_sb[0:64, :])

            th = [lambda kt=kt: dma_kv(kt) for kt in range(KT_D)]
            th.append(k_rope)
            if sc == 0:
                th.append(cs_dup)
            th += [lambda kt=kt: v_trans(kt)
                   for kt in range(4 * sc, 4 * sc + 4)]
            if fused:
                th += [lambda mt=mt: q_rope(mt) for mt in range(2)]
            else:
                for mt in range(2):
                    th += [lambda mt=mt, kt=kt: q_mm(mt, kt)
                           for kt in range(KT_D)]
                    th.append(lambda mt=mt: q_rope(mt))
            return th

        def s_thunks(qc, h, tiles):
            """S^T mega matmuls + exp + mask for one head; fills `tiles`."""
            q0 = qc * 512
            hf = h * S
            nkt = 4 * qc + 4
            thunks = []
            for pi in range(nkt // 2):
                def th(pi=pi):
                    kts = (2 * pi, 2 * pi + 1)
                    ps_t = ps.tile([128, 1024], f32, tag="ps", name="ps_t")
                    pt_t = ptp.tile([128, 1024], sdt, tag="pt", name="pt_t")
                    for li, kt in enumerate(kts):
                        dj = kt - 4 * qc
                        qo = 128 * dj if dj >= 0 else 0
                        lo = li * 512
                        nc.tensor.matmul(
                            ps_t[:, lo + qo:lo + 512],
                            _mm_ap(KVt[0:64, kt * 128:(kt + 1) * 128], mmdt),
                            _mm_ap(QT[:, hf + q0 + qo:hf + q0 + 512], mmdt),
                            start=True, stop=True)
                    if 2 * pi + 1 < 4 * qc:
                        nc.scalar.activation(
                            pt_t[:], ps_t[:],
                            mybir.ActivationFunctionType.Exp, scale=SCALE)
                    else:
                        for li, kt in enumerate(kts):
                            dj = kt - 4 * qc
                            qo = 128 * dj if dj >= 0 else 0
                            lo = li * 512
                            nc.scalar.activation(
                                pt_t[:, lo + qo:lo + 512],
                                ps_t[:, lo + qo:lo + 512],
                                mybir.ActivationFunctionType.Exp,
                                scale=SCALE)
                    for li, kt in enumerate(kts):
                        dj = kt - 4 * qc
                        qo = 128 * dj if dj >= 0 else 0
                        lo = li * 512
                        if dj >= 0:
                            eng = nc.gpsimd if MASK_POOL else nc.vector
                            eng.tensor_mul(
                                pt_t[:, lo + qo:lo + qo + 128],
                                pt_t[:, lo + qo:lo + qo + 128], utri_sb[:])
                        tiles.append((kt, qo, lo, pt_t))
                thunks.append(th)
            return thunks

        def pv_thunks(qc, h, tiles):
            """q-major PV accumulation + normalization + O transposes.

            For each incoming P^T tile (k-tile kt), run the N=65 matmuls for
            each live 128-q subtile j: out[q,hd|den] += P^T[:,j*128:].T@Vp.
            After the last k-tile: reciprocal of the denominators, normalize
            psum->O_sb (bf16), PE-transpose to O^T, 2x DVE copy into OT.
            """
            q0 = qc * 512
            hp = (h % 2) * 64
            of = (h // 2) * S
            nkt0 = 4 * qc + 4
            state = {}

            def pv_series(j):
                """All matmuls of q-subtile j back-to-back: start=True
                clears the whole bank's has_written bits, so accumulation
                series sharing a psum bank must not interleave."""
                if "pv" not in state:
                    state["pv"] = pjo.tile([128, 512], f32, tag="pjo",
                                           name="pvt")
                pv_t = state["pv"]
                for kt in range(0, 4 * qc + j + 1):
                    _, qo, lo, pt_t = tiles[kt]
                    nc.tensor.matmul(
                        pv_t[:, j * 128:j * 128 + HD1],
                        _mm_ap(pt_t[:, lo + j * 128:
                                    lo + (j + 1) * 128], mmdt),
                        _mm_ap(Vp[:, kt * HD1:(kt + 1) * HD1], mmdt),
                        start=(kt == 0), stop=(kt == 4 * qc + j))

            def norm():
                pv_t = state["pv"]
                rc = small.tile([128, 4], f32, tag="rc", name="rc")
                dsb = small.tile([128, 4], f32, tag="dsb", name="dsb")
                o_sb = state["o_sb"] = osb_head()
                pvr = pv_t.rearrange("p (b c) -> p b c", c=128)
                nc.vector.tensor_copy(
                    dsb[:].rearrange("p (b c) -> p b c", c=1),
                    pvr[:, :, HD:HD + 1])
                nc.vector.reciprocal(rc[:], dsb[:])
                for j in range(4):
                    nc.vector.tensor_scalar_mul(
                        o_sb[:, j * HD:(j + 1) * HD],
                        pv_t[:, j * 128:j * 128 + HD],
                        rc[:, j:j + 1])

            def trans(half):
                # transpose via a regular matmul against the identity:
                # out[hd, q] = o_sb[q, hd]^T @ I -- f32 psum output, so
                # nonzero free offsets behave like the score megas
                o_sb = state["o_sb"]
                tp = pw.tile([64, 256], f32, tag="pw", name="tp")
                for st2 in range(2):
                    stg = 2 * half + st2
                    nc.tensor.matmul(
                        tp[:, st2 * 128:(st2 + 1) * 128],
                        _mm_ap(o_sb[:, stg * HD:(stg + 1) * HD], mmdt),
                        _mm_ap(ident[:], mmdt),
                        start=True, stop=True)
                oc_eng = nc.scalar if qc <= 1 else nc.vector
                (oc_eng.copy if qc <= 1 else nc.vector.tensor_copy)(
                    OT[hp:hp + 64, of + q0 + half * 256:
                       of + q0 + half * 256 + 256], tp[:])

            def osb_head():
                return opool.tile([128, 4 * HD], sdt, tag=f"osb{h % 2}",
                                  name="o_sb")

            th = [lambda j=j: pv_series(j) for j in range(4)]
            th.append(norm)
            th += [lambda half=half: trans(half) for half in range(2)]
            return th

        def wo_half(qt, np2, half, obs, pool=None, ptag="pw",
                    copy_eng="mix", split_dma=False):
            """One 512-wide n-chunk of out row-block qt; the final chunk
            fires one [128,2048] fp16 DMA for the whole row-block (fewer
            HWDGE round-trips than per-chunk DMAs). The epilogue splits
            per-np2 ([128,1024]) so the last transfer is shorter."""
            pool = pool or pw
            if (np2, half) == (0, 0):
                obs[qt] = osb.tile([128, 2048], f16, tag="ob", name="ob")
            ob = obs[qt]
            ncn = 2 * np2 + half
            pw_t = pool.tile([128, 512], f32, tag=ptag, name="pw_t")
            for mt in range(2):
                nc.tensor.matmul(
                    pw_t[:],
                    _mm_ap(OT[:, mt * S + qt * 128:
                              mt * S + (qt + 1) * 128], mmdt),
                    _mm_ap(wo_sb[:, mt * DIM + ncn * 512:
                                 mt * DIM + ncn * 512 + 512], mmdt),
                    start=(mt == 0), stop=(mt == 1))
            wo_copy_cnt[0] += 1
            use_act = {"mix": wo_copy_cnt[0] % 3 == 0,
                       "dve": False,
                       "alt": wo_copy_cnt[0] % 2 == 1}[copy_eng]
            if use_act:
                nc.scalar.copy(ob[:, ncn * 512:ncn * 512 + 512], pw_t[:])
            else:
                nc.vector.tensor_copy(
                    ob[:, ncn * 512:ncn * 512 + 512], pw_t[:])
            if split_dma and half == 1:
                nc.sync.dma_start(
                    out[qt * 128:(qt + 1) * 128,
                        np2 * 1024:np2 * 1024 + 1024],
                    ob[:, np2 * 1024:np2 * 1024 + 1024])
                if np2 == 1:
                    del obs[qt]
            elif not split_dma and (np2, half) == (1, 1):
                del obs[qt]
                nc.sync.dma_start(
                    out[qt * 128:(qt + 1) * 128, :], ob[:])
        wo_obs = {}

        # ------------------------------------- merged emission schedule
        def merge(primary, *others):
            """Emit primary thunks; proportionally interleave the others."""
            counters = [0.0] * len(others)
            n = max(1, len(primary))
            for beat in primary:
                for j, lst in enumerate(others):
                    counters[j] += len(lst) / n
                    while counters[j] >= 1.0 and lst:
                        lst.pop(0)()
                        counters[j] -= 1.0
                for th in beat:
                    th()
            for lst in others:
                while lst:
                    lst.pop(0)()

        for th in proj_thunks(0, fused=True):       # prologue
            th()

        prev = None                      # (qc, h, tiles) awaiting PV
        for sc in range(NSC):
            if sc == 1:
                nc.sync.dma_start(wo_sb[:, 0:DIM], wo_s[0:128, :])
                nc.sync.dma_start(wo_sb[:, DIM:2 * DIM], wo_s[128:256, :])
            pstream = proj_thunks(sc + 1) if sc + 1 < NSC else []
            # in the last window there is no projection work: alternate WO
            # psum between the mostly-idle pjo pool and pw so WO units are
            # not gated on a single bank's copy drain, and keep its copies
            # off the exp-saturated ACT engine
            if sc == NSC - 1:
                wpp = [(pjo, "pjo", "dve")]
            else:
                wpp = [(None, "pw", "mix")]
            wostream = ([lambda qt=qt, np2=np2, half=half,
                         w=wpp[(2 * np2 + half) % len(wpp)]:
                         wo_half(qt, np2, half, wo_obs, pool=w[0],
                                 ptag=w[1], copy_eng=w[2])
                         for qt in range(4 * (sc - 1), 4 * (sc - 1) + 4)
                         for np2 in range(2)
                         for half in range(2)] if sc >= 1 else [])
            for h in range(HQ):
                tiles = []
                sth = s_thunks(sc, h, tiles)
                pth = pv_thunks(*prev) if prev is not None else []
                beats = []
                for i in range(max(len(sth), len(pth))):
                    beat = []
                    if i < len(pth):
                        beat.append(pth[i])
                    if i < len(sth):
                        beat.append(sth[i])
                    beats.append(beat)
                # WO of sc-1 needs PV(sc-1, 3) done: that PV is head 0 here
                if h == 0:
                    ptake = max(1, len(pstream) // HQ) if pstream else 0
                    merge(beats, pstream[:ptake])
                    pstream = pstream[ptake:]
                else:
                    ptake = (len(pstream) // (HQ - h)) if pstream else 0
                    wtake = (len(wostream) // (HQ - h)) if wostream else 0
                    merge(beats, pstream[:ptake], wostream[:wtake])
                    pstream = pstream[ptake:]
                    wostream = wostream[wtake:]
                prev = (sc, h, tiles)
            merge([], pstream, wostream)

        # epilogue: PV of the last head, then WO of chunk 3; the score
        # pool's banks are free now, so WO rotates through those too.
        # trans(0) covers q-tiles 12-13, trans(1) covers 14-15: start the
        # WO units for each pair as soon as its OT columns are complete.
        pth = pv_thunks(*prev)
        for th in pth[:-1]:          # pairs + norm + trans(0)
            th()
        epi = [0]
        pools = [(pw, "pw"), (ps, "ps"), (pjo, "pjo")]

        def epi_wo(qt, np2):
            pool, ptag = pools[epi[0] % 3]
            for half in range(2):
                wo_half(qt, np2, half, wo_obs, pool=pool, ptag=ptag,
                        copy_eng="mix", split_dma=True)
            epi[0] += 1

        epi_wo(12, 0)
        pth[-1]()                    # trans(1) overlaps qt-12 copies
        epi_wo(12, 1)
        for qt in range(13, 16):
            for np2 in range(2):
                epi_wo(qt, np2)

    nc.compile()
    return nc


# ------------------------------------------------------------- host side
def _pair_perm64():
    """Column permutation putting the RoPE partner 16 partitions away."""
    return np.array([2 * (16 * (j // 32) + (j % 16)) + ((j % 32) // 16)
                     for j in range(64)])


def _host_prep(x, freqs_cos, freqs_sin, wq, wk, wv, wo):
    _, _, npdt = _dtypes()
    x = np.asarray(x, np.float32)
    fc = np.asarray(freqs_cos, np.float32)
    fs = np.asarray(freqs_sin, np.float32)
    wq = np.asarray(wq, np.float32)
    wk = np.asarray(wk, np.float32)
    wv = np.asarray(wv, np.float32)
    wo = np.asarray(wo, np.float32)

    perm = _pair_perm64()
    xT = np.ascontiguousarray(x[0].T).astype(npdt)

    p = np.arange(64)
    pair = 16 * ((p % 64) // 32) + (p % 16)
    sign = np.where((p % 32) < 16, -1.0, 1.0).astype(np.float32)
    cosE = np.ascontiguousarray(fc[:, pair].T)                  # [64, S]
    sinE = np.ascontiguousarray(fs[:, pair].T) * sign[:, None]  # [64, S]
    utri = np.triu(np.ones((128, 128), np.float32)).astype(npdt)

    in_maps = []
    for c in range(NCORES):
        qcols = np.concatenate(
            [wq[:, (4 * c + i) * 64 + perm] for i in range(HQ)], axis=1)
        kcols = wk[:, c * 64 + perm]
        vcols = wv[:, c * 64:(c + 1) * 64]
        wqkv_c = np.concatenate([qcols, kcols, vcols], axis=1).astype(npdt)
        wo_c = wo[QW * c:QW * (c + 1), :].astype(npdt)
        in_maps.append({
            "xT": xT,
            "wqkv": np.ascontiguousarray(wqkv_c),
            "wo_s": np.ascontiguousarray(wo_c),
            "cosE": cosE.astype(np.float32),
            "sinE": np.ascontiguousarray(sinE).astype(np.float32),
            "utri": np.ascontiguousarray(utri),
        })
    return in_maps


_NC_CACHE = {}


def get_program():
    if MM not in _NC_CACHE:
        _NC_CACHE[MM] = build_program()
    return _NC_CACHE[MM]


def kernel(x, freqs_cos, freqs_sin, wq, wk, wv, wo):
    nc = get_program()
    in_maps = _host_prep(x, freqs_cos, freqs_sin, wq, wk, wv, wo)
    res = run_bass_kernel_spmd(nc, in_maps, core_ids=list(range(NCORES)))
    acc = np.zeros((S, DIM), np.float64)
    for r in res.results:
        acc += r["out"].astype(np.float64)
    return acc.astype(np.float32).reshape(1, S, DIM)


# revision 57
# speedup vs baseline: 1.0333x; 1.0213x over previous
"""Trainium2 Bass kernel for a GQA attention layer (dense transformer).

Reference computation (B=1, S=2048, DIM=2048, 32 q-heads, 8 kv-heads, hd=64):
    xq = x @ wq; xk = x @ wk; xv = x @ wv
    rope(xq, xk); GQA causal attention; out = attn @ wo

Sharding: tensor-parallel over heads across 8 cores. Core c owns q-heads
4c..4c+3 (wq cols), kv-head c (wk/wv cols), and wo rows 256c..256c+255.
Each core computes a full [S, DIM] partial of the output projection; the
host sums the 8 partials (the TP all-reduce, done at gather time).

Kernel layout strategy (everything "transposed", head_dim on partitions):
  - QT/KT/VT = W.T @ x computed with lhsT = weight shard (natural [DIM, m]
    layout), rhs = x.T tiles streamed from DRAM.
  - RoPE pairs are de-interleaved by permuting wq/wk columns on the host so
    the rotation partner sits 16 partitions away (within a 32-partition
    quadrant, so DVE stream_shuffle can swap them).
  - Scores are computed transposed: S^T[k, q] = K^T.T @ Q^T per 128-row
    k-tile; exp on ACT (scale fused); causal mask = upper-tri multiply on
    the single diagonal 128x128 block of each k-tile (on GPSIMD).
  - P@V runs q-major: out[q,hd] = sum_k P^T[k,q].T @ Vp[k,hd|1] per
    (k-tile, 128-q subtile) with N=65 moving columns - 65 PE cycles per
    k-tile instead of 512, full 128x128 array use. The ones column makes
    psum col 64 the softmax denominator.
  - Normalize: one reciprocal per head ([128,4]), then tensor_scalar_mul
    psum->SBUF (per-partition scalar = per-q denom) into O_sb, then PE
    transposes [128q,64] -> [64,128q] (bf16 psum) and 2x-speed DVE copies
    into OT for the output projection.
  - Output projection from O^T with wo shard as rhs; partials DMA'd fp16.
"""

import numpy as np
import ml_dtypes

import concourse.bass as bass
import concourse.mybir as mybir
from concourse import bacc
from concourse.tile import TileContext
from concourse.masks import make_identity
from concourse.bass_utils import run_bass_kernel_spmd

# ---------------------------------------------------------------- constants
S = 2048          # sequence length
DIM = 2048        # model dim
NH = 32           # query heads
NKV = 8           # kv heads
HD = 64           # head dim
NCORES = 8
HQ = NH // NCORES          # query heads per core = 4
QW = HQ * HD               # q width per core = 256
KT_S = S // 128            # 16 seq k-tiles
KT_D = DIM // 128          # 16 dim k-tiles
NSC = S // 512             # 4 s-chunks
SCALE = 1.0 / 8.0          # 1/sqrt(64)
HD1 = HD + 1               # V tile width incl ones column

# matmul dtype knob: 'bf16' | 'f32' | 'f32r'
MM = 'bf16'
MASK_POOL = True           # causal masks on GPSIMD (else DVE)
NWARM = 12                 # p-state warmup transposes
ACT_COPY_EVERY = 4         # every Nth WO psum->sbuf copy goes to ACT

_SHUF_SWAP16 = list(range(16, 32)) + list(range(16))


def _dtypes():
    if MM == 'bf16':
        return mybir.dt.bfloat16, mybir.dt.bfloat16, ml_dtypes.bfloat16
    if MM == 'f32':
        return mybir.dt.float32, mybir.dt.float32, np.float32
    if MM == 'f32r':
        return mybir.dt.float32, mybir.dt.float32r, np.float32
    raise ValueError(MM)


def _mm_ap(ap, mmdt):
    """View an AP in the matmul dtype (bitcast f32 -> f32r when needed)."""
    if ap.dtype != mmdt:
        return ap.bitcast(mmdt)
    return ap


def build_program():
    """Build the per-core Bass program (same program on all 8 cores).

    Emission is a fine-grained software pipeline: attention beats for chunk
    sc (S^T mega-matmul for head h + PV beats of head h-1) are merged with
    the projection matmuls of chunk sc+1 and the WO units of chunk sc-1.

    PSUM (8 banks): pjo 3 (projection passes + PV accumulators, shared tag)
    + ps 4 (two [128,1024] score megas) + pw 1 (WO + transposes).
    """
    sdt, mmdt, _ = _dtypes()
    f32 = mybir.dt.float32
    f16 = mybir.dt.float16

    nc = bacc.Bacc("TRN2", target_bir_lowering=False, debug=False,
                   num_devices=NCORES)

    xT = nc.dram_tensor("xT", [DIM, S], sdt, kind="ExternalInput")
    wqkv = nc.dram_tensor("wqkv", [DIM, QW + 2 * HD], sdt,
                          kind="ExternalInput")
    wo_s = nc.dram_tensor("wo_s", [QW, DIM], sdt, kind="ExternalInput")
    cosE = nc.dram_tensor("cosE", [64, S], f32, kind="ExternalInput")
    sinE = nc.dram_tensor("sinE", [64, S], f32, kind="ExternalInput")
    utri = nc.dram_tensor("utri", [128, 128], sdt, kind="ExternalInput")
    out = nc.dram_tensor("out", [S, DIM], f16, kind="ExternalOutput")

    WQKV = QW + 2 * HD  # 384

    import contextlib
    with TileContext(nc) as tc, contextlib.ExitStack() as ctx:
        const = ctx.enter_context(tc.tile_pool(name="const", bufs=1))
        work = ctx.enter_context(tc.tile_pool(name="work", bufs=2))
        xtp = ctx.enter_context(tc.tile_pool(name="xtp", bufs=10))
        ptp = ctx.enter_context(tc.tile_pool(name="ptp", bufs=20))
        small = ctx.enter_context(tc.tile_pool(name="small", bufs=5))
        osb = ctx.enter_context(tc.tile_pool(name="osb", bufs=4))
        opool = ctx.enter_context(tc.tile_pool(name="opool", bufs=2))

        pjo = ctx.enter_context(tc.tile_pool(name="pjo", bufs=3,
                                             space="PSUM"))
        ps = ctx.enter_context(tc.tile_pool(name="ps", bufs=2, space="PSUM"))
        pw = ctx.enter_context(tc.tile_pool(name="pw", bufs=1, space="PSUM"))

        # ----------------------------------------------- persistent SBUF
        w_sb = const.tile([128, KT_D * WQKV], sdt, tag="w_sb")
        wo_sb = const.tile([128, 2 * DIM], sdt, tag="wo_sb")
        cos_sb = const.tile([128, S], f32, tag="cos_sb")
        sin_sb = const.tile([128, S], f32, tag="sin_sb")
        utri_sb = const.tile([128, 128], sdt, tag="utri_sb")
        ident = const.tile([128, 128], sdt, tag="ident")
        QT = const.tile([64, HQ * S], sdt, tag="QT")
        KVt = const.tile([128, S], sdt, tag="KVt")
        Vp = const.tile([128, KT_S * HD1], sdt, tag="Vp")
        OT = const.tile([128, 2 * S], sdt, tag="OT")

        # p-state warmup: keep PE streaming during the initial DMA wait so
        # the frequency ramp (3us) is over before real matmuls start; the
        # source tile only needs to hold valid bits, so a fast DVE memset
        # unblocks the first transpose ~200ns in
        wsrc = const.tile([128, 128], sdt, tag="wsrc")
        nc.vector.memset(wsrc[:], 0.0)
        warm = pw.tile([128, 128], sdt, tag="pw", name="warm")
        for _ in range(NWARM):
            nc.tensor.transpose(warm[:], wsrc[:], wsrc[:])

        make_identity(nc, ident[:])
        nc.gpsimd.memset(Vp[:], 1.0)  # ones columns for denominator

        wo_copy_cnt = [0]

        # ---------------------------------------------- thunk generators
        def proj_thunks(sc, fused=False):
            """Projection of chunk sc: KV pass, K-rope, V transposes, then
            Q passes (one PSUM slot each, sequential). With fused=True
            (prologue) all three matmuls run per k-tile, using 3 slots."""
            s0 = sc * 512
            xts = []
            st = {}

            # small first batches so the first matmul starts early; bigger
            # later ones amortize HWDGE overhead
            batches = [2, 2, 4, 4, 4] if sc == 0 else [4, 4, 4, 4]
            starts = [sum(batches[:i]) for i in range(len(batches))]
            kt_slot = {}
            for bi, (b0, bn) in enumerate(zip(starts, batches)):
                for j in range(bn):
                    kt_slot[b0 + j] = (bi, j, bn, b0)

            def dma_kv(kt):
                bi, j, bn, b0 = kt_slot[kt]
                if j == 0:
                    if sc == 0:
                        nc.sync.dma_start(
                            w_sb[:, b0 * WQKV:(b0 + bn) * WQKV].rearrange(
                                "r (k w) -> r k w", k=bn),
                            wqkv[b0 * 128:(b0 + bn) * 128, :].rearrange(
                                "(k r) w -> r k w", k=bn))
                    xt4 = xtp.tile([128, 4 * 512], sdt, tag="xt", name="xt4")
                    nc.sync.dma_start(
                        xt4[:, 0:bn * 512].rearrange("r (k c) -> r k c", k=bn),
                        xT[b0 * 128:(b0 + bn) * 128,
                           s0:s0 + 512].rearrange("(k r) c -> r k c", k=bn))
                    xts.append(xt4)
                    if sc == 0 and kt == 8:
                        # cos rides late in chunk 0 (needed by k_rope only
                        # after the full KV pass); rows [64:128] are a copy
                        # of [0:64] (64-row periodicity) done on idle ACT
                        nc.sync.dma_start(cos_sb[0:64, :], cosE[:])
                    if sc == 0 and kt == 12:
                        nc.sync.dma_start(sin_sb[0:64, :], sinE[:])
                        nc.sync.dma_start(utri_sb[:], utri[:])
                bi, j, bn, b0 = kt_slot[kt]
                xt = xts[bi][:, j * 512:j * 512 + 512]
                if kt == 0:
                    st["pkv"] = pjo.tile([128, 512], f32, tag="pjo",
                                         name="pkv")
                    if fused:
                        st["fq0"] = pjo.tile([128, 512], f32, tag="pjo",
                                             name="fq0")
                        st["fq1"] = pjo.tile([128, 512], f32, tag="pjo",
                                             name="fq1")
                nc.tensor.matmul(
                    st["pkv"][:],
                    _mm_ap(w_sb[:, kt * WQKV + 256:kt * WQKV + 384], mmdt),
                    _mm_ap(xt, mmdt),
                    start=(kt == 0), stop=(kt == KT_D - 1))
                if fused:
                    for mt in range(2):
                        nc.tensor.matmul(
                            st[f"fq{mt}"][:],
                            _mm_ap(w_sb[:, kt * WQKV + mt * 128:
                                        kt * WQKV + mt * 128 + 128], mmdt),
                            _mm_ap(xt, mmdt),
                            start=(kt == 0), stop=(kt == KT_D - 1))

            def k_rope():
                pkv = st["pkv"]
                shufk = work.tile([64, 512], f32, tag="shufk", name="shufk")
                m1k = work.tile([64, 512], f32, tag="m1k", name="m1k")
                t2k = work.tile([64, 512], f32, tag="t2k", name="t2k")
                nc.vector.stream_shuffle(shufk[:], pkv[0:64, :],
                                         _SHUF_SWAP16)
                nc.vector.tensor_mul(m1k[:], pkv[0:64, :],
                                     cos_sb[0:64, s0:s0 + 512])
                nc.vector.tensor_mul(t2k[:], shufk[:],
                                     sin_sb[0:64, s0:s0 + 512])
                nc.vector.tensor_add(KVt[0:64, s0:s0 + 512], m1k[:], t2k[:])
                nc.scalar.copy(KVt[64:128, s0:s0 + 512],
                               pkv[64:128, :])

            def v_trans(kt):
                pv = pw.tile([128, HD], sdt, tag="pw", name="pv")
                nc.tensor.transpose(
                    pv[:], KVt[64:128, kt * 128:(kt + 1) * 128],
                    ident[64:128, 64:128])
                nc.vector.tensor_copy(
                    Vp[:, kt * HD1:kt * HD1 + HD], pv[:])

            def q_mm(mt, kt):
                if kt == 0:
                    st["pq"] = pjo.tile([128, 512], f32, tag="pjo",
                                        name="pq")
                w0 = kt * WQKV + mt * 128
                bi, j, bn, b0 = kt_slot[kt]
                xt = xts[bi][:, j * 512:j * 512 + 512]
                nc.tensor.matmul(
                    st["pq"][:], _mm_ap(w_sb[:, w0:w0 + 128], mmdt),
                    _mm_ap(xt, mmdt),
                    start=(kt == 0), stop=(kt == KT_D - 1))

            def q_rope(mt):
                pq = st[f"fq{mt}"] if fused else st["pq"]
                shuf = work.tile([128, 512], f32, tag="shuf", name="shuf")
                m1 = work.tile([128, 512], f32, tag="m1", name="m1")
                t2 = work.tile([128, 512], f32, tag="t2", name="t2")
                nc.vector.stream_shuffle(shuf[:], pq[:], _SHUF_SWAP16)
                nc.vector.tensor_mul(m1[:], pq[:], cos_sb[:, s0:s0 + 512])
                nc.vector.tensor_mul(t2[:], shuf[:], sin_sb[:, s0:s0 + 512])
                he = (2 * mt) * S
                ho = (2 * mt + 1) * S
                nc.vector.tensor_add(
                    QT[:, he + s0:he + s0 + 512], m1[0:64, :], t2[0:64, :])
                nc.vector.tensor_add(
                    QT[:, ho + s0:ho + s0 + 512], m1[64:128, :],
                    t2[64:128, :])

            def cs_dup():
                # duplicate cos/sin rows [0:64] into [64:128] for q_rope
                # (cos on idle ACT, sin on idle GPSIMD, in parallel)
                nc.scalar.copy(cos_sb[64:128, :], cos_sb[0:64, :])
                nc.gpsimd.tensor_copy(sin_sb[64:128, :], sin_sb[0:64, :])

            th = [lambda kt=kt: dma_kv(kt) for kt in range(KT_D)]
            th.append(k_rope)
            if sc == 0:
                th.append(cs_dup)
            th += [lambda kt=kt: v_trans(kt)
                   for kt in range(4 * sc, 4 * sc + 4)]
            if fused:
                th += [lambda mt=mt: q_rope(mt) for mt in range(2)]
            else:
                for mt in range(2):
                    th += [lambda mt=mt, kt=kt: q_mm(mt, kt)
                           for kt in range(KT_D)]
                    th.append(lambda mt=mt: q_rope(mt))
            return th

        def s_thunks(qc, h, tiles):
            """S^T mega matmuls + exp + mask for one head; fills `tiles`."""
            q0 = qc * 512
            hf = h * S
            nkt = 4 * qc + 4
            thunks = []
            for pi in range(nkt // 2):
                def th(pi=pi):
                    kts = (2 * pi, 2 * pi + 1)
                    ps_t = ps.tile([128, 1024], f32, tag="ps", name="ps_t")
                    pt_t = ptp.tile([128, 1024], sdt, tag="pt", name="pt_t")
                    for li, kt in enumerate(kts):
                        dj = kt - 4 * qc
                        qo = 128 * dj if dj >= 0 else 0
                        lo = li * 512
                        nc.tensor.matmul(
                            ps_t[:, lo + qo:lo + 512],
                            _mm_ap(KVt[0:64, kt * 128:(kt + 1) * 128], mmdt),
                            _mm_ap(QT[:, hf + q0 + qo:hf + q0 + 512], mmdt),
                            start=True, stop=True)
                    if 2 * pi + 1 < 4 * qc or (qc >= 1
                                               and 2 * pi == 4 * qc):
                        # first diag pair (dj 0,1) of chunks >= 1: exp the
                        # full mega in one op; the extra columns hold stale
                        # scores (bounded, and never read by the series-
                        # major PV), and one wide op beats two narrow ones
                        # on the exp-saturated ACT engine
                        nc.scalar.activation(
                            pt_t[:], ps_t[:],
                            mybir.ActivationFunctionType.Exp, scale=SCALE)
                    else:
                        for li, kt in enumerate(kts):
                            dj = kt - 4 * qc
                            qo = 128 * dj if dj >= 0 else 0
                            lo = li * 512
                            nc.scalar.activation(
                                pt_t[:, lo + qo:lo + 512],
                                ps_t[:, lo + qo:lo + 512],
                                mybir.ActivationFunctionType.Exp,
                                scale=SCALE)
                    for li, kt in enumerate(kts):
                        dj = kt - 4 * qc
                        qo = 128 * dj if dj >= 0 else 0
                        lo = li * 512
                        if dj >= 0:
                            eng = nc.gpsimd if MASK_POOL else nc.vector
                            eng.tensor_mul(
                                pt_t[:, lo + qo:lo + qo + 128],
                                pt_t[:, lo + qo:lo + qo + 128], utri_sb[:])
                        tiles.append((kt, qo, lo, pt_t))
                thunks.append(th)
            return thunks

        def pv_thunks(qc, h, tiles):
            """q-major PV accumulation + normalization + O transposes.

            For each incoming P^T tile (k-tile kt), run the N=65 matmuls for
            each live 128-q subtile j: out[q,hd|den] += P^T[:,j*128:].T@Vp.
            After the last k-tile: reciprocal of the denominators, normalize
            psum->O_sb (bf16), PE-transpose to O^T, 2x DVE copy into OT.
            """
            q0 = qc * 512
            hp = (h % 2) * 64
            of = (h // 2) * S
            nkt0 = 4 * qc + 4
            state = {}

            def pv_series(j):
                """All matmuls of q-subtile j back-to-back: start=True
                clears the whole bank's has_written bits, so accumulation
                series sharing a psum bank must not interleave."""
                if "pv" not in state:
                    state["pv"] = pjo.tile([128, 512], f32, tag="pjo",
                                           name="pvt")
                pv_t = state["pv"]
                for kt in range(0, 4 * qc + j + 1):
                    _, qo, lo, pt_t = tiles[kt]
                    nc.tensor.matmul(
                        pv_t[:, j * 128:j * 128 + HD1],
                        _mm_ap(pt_t[:, lo + j * 128:
                                    lo + (j + 1) * 128], mmdt),
                        _mm_ap(Vp[:, kt * HD1:(kt + 1) * HD1], mmdt),
                        start=(kt == 0), stop=(kt == 4 * qc + j))

            def norm():
                pv_t = state["pv"]
                rc = small.tile([128, 4], f32, tag="rc", name="rc")
                dsb = small.tile([128, 4], f32, tag="dsb", name="dsb")
                o_sb = state["o_sb"] = osb_head()
                pvr = pv_t.rearrange("p (b c) -> p b c", c=128)
                nc.vector.tensor_copy(
                    dsb[:].rearrange("p (b c) -> p b c", c=1),
                    pvr[:, :, HD:HD + 1])
                nc.vector.reciprocal(rc[:], dsb[:])
                for j in range(4):
                    nc.vector.tensor_scalar_mul(
                        o_sb[:, j * HD:(j + 1) * HD],
                        pv_t[:, j * 128:j * 128 + HD],
                        rc[:, j:j + 1])

            def trans(half):
                # transpose via a regular matmul against the identity:
                # out[hd, q] = o_sb[q, hd]^T @ I -- f32 psum output, so
                # nonzero free offsets behave like the score megas
                o_sb = state["o_sb"]
                tp = pw.tile([64, 256], f32, tag="pw", name="tp")
                for st2 in range(2):
                    stg = 2 * half + st2
                    nc.tensor.matmul(
                        tp[:, st2 * 128:(st2 + 1) * 128],
                        _mm_ap(o_sb[:, stg * HD:(stg + 1) * HD], mmdt),
                        _mm_ap(ident[:], mmdt),
                        start=True, stop=True)
                oc_eng = nc.scalar if qc <= 1 else nc.vector
                (oc_eng.copy if qc <= 1 else nc.vector.tensor_copy)(
                    OT[hp:hp + 64, of + q0 + half * 256:
                       of + q0 + half * 256 + 256], tp[:])

            def osb_head():
                return opool.tile([128, 4 * HD], sdt, tag=f"osb{h % 2}",
                                  name="o_sb")

            th = [lambda j=j: pv_series(j) for j in range(4)]
            th.append(norm)
            th += [lambda half=half: trans(half) for half in range(2)]
            return th

        def wo_half(qt, np2, half, obs, pool=None, ptag="pw",
                    copy_eng="mix", split_dma=False):
            """One 512-wide n-chunk of out row-block qt; the final chunk
            fires one [128,2048] fp16 DMA for the whole row-block (fewer
            HWDGE round-trips than per-chunk DMAs). The epilogue splits
            per-np2 ([128,1024]) so the last transfer is shorter."""
            pool = pool or pw
            if (np2, half) == (0, 0):
                obs[qt] = osb.tile([128, 2048], f16, tag="ob", name="ob")
            ob = obs[qt]
            ncn = 2 * np2 + half
            pw_t = pool.tile([128, 512], f32, tag=ptag, name="pw_t")
            for mt in range(2):
                nc.tensor.matmul(
                    pw_t[:],
                    _mm_ap(OT[:, mt * S + qt * 128:
                              mt * S + (qt + 1) * 128], mmdt),
                    _mm_ap(wo_sb[:, mt * DIM + ncn * 512:
                                 mt * DIM + ncn * 512 + 512], mmdt),
                    start=(mt == 0), stop=(mt == 1))
            wo_copy_cnt[0] += 1
            use_act = {"mix": wo_copy_cnt[0] % 3 == 0,
                       "dve": False,
                       "alt": wo_copy_cnt[0] % 2 == 1}[copy_eng]
            if use_act:
                nc.scalar.copy(ob[:, ncn * 512:ncn * 512 + 512], pw_t[:])
            else:
                nc.vector.tensor_copy(
                    ob[:, ncn * 512:ncn * 512 + 512], pw_t[:])
            if split_dma and half == 1:
                nc.sync.dma_start(
                    out[qt * 128:(qt + 1) * 128,
                        np2 * 1024:np2 * 1024 + 1024],
                    ob[:, np2 * 1024:np2 * 1024 + 1024])
                if np2 == 1:
                    del obs[qt]
            elif not split_dma and (np2, half) == (1, 1):
                del obs[qt]
                nc.sync.dma_start(
                    out[qt * 128:(qt + 1) * 128, :], ob[:])
        wo_obs = {}

        # ------------------------------------- merged emission schedule
        def merge(primary, *others):
            """Emit primary thunks; proportionally interleave the others."""
            counters = [0.0] * len(others)
            n = max(1, len(primary))
            for beat in primary:
                for j, lst in enumerate(others):
                    counters[j] += len(lst) / n
                    while counters[j] >= 1.0 and lst:
                        lst.pop(0)()
                        counters[j] -= 1.0
                for th in beat:
                    th()
            for lst in others:
                while lst:
                    lst.pop(0)()

        for th in proj_thunks(0, fused=True):       # prologue
            th()

        # Head processing order: chunk 3's head 0 is pulled forward between
        # (2,2) and (2,3) so part of the causal-triangle-heavy chunk-3 exp
        # load runs while ACT still has slack, instead of piling up at the
        # end where exp rate-limits the whole pipeline.
        ITEMS = [(0, 0), (0, 1), (0, 2), (0, 3),
                 (1, 0), (1, 1), (1, 2), (1, 3),
                 (2, 0), (2, 1), (2, 2), (3, 0), (2, 3),
                 (3, 1), (3, 2), (3, 3)]
        # proj(sc) spread over items [a, b) — must drain before the first
        # (sc, *) item; WO(sc) over [a, b) — may start only after the item
        # containing trans(sc, 3), i.e. one past (sc, 3)'s position.
        PROJ_AT = {1: (0, 4), 2: (4, 8), 3: (8, 11)}
        WO_AT = {0: (5, 8), 1: (9, 13), 2: (14, 16)}
        proj_by_start = {a: s for s, (a, b) in PROJ_AT.items()}
        wo_by_start = {a: s for s, (a, b) in WO_AT.items()}

        prev = None                      # (qc, h, tiles) awaiting PV
        pstream, pend = [], 0
        wopending, woend = [], 0
        for i, (sc, h) in enumerate(ITEMS):
            if i == 4:
                nc.sync.dma_start(wo_sb[:, 0:DIM], wo_s[0:128, :])
                nc.sync.dma_start(wo_sb[:, DIM:2 * DIM], wo_s[128:256, :])
            if i in proj_by_start:
                pstream = proj_thunks(proj_by_start[i])
                pend = PROJ_AT[proj_by_start[i]][1]
            if i in wo_by_start:
                s = wo_by_start[i]
                wopending = [(qt, np2, half)
                             for qt in range(4 * s, 4 * s + 4)
                             for np2 in range(2)
                             for half in range(2)]
                woend = WO_AT[s][1]
            tiles = []
            sth = s_thunks(sc, h, tiles)
            pth = pv_thunks(*prev) if prev is not None else []
            beats = []
            for bi in range(max(len(sth), len(pth))):
                beat = []
                if bi < len(pth):
                    beat.append(pth[bi])
                if bi < len(sth):
                    beat.append(sth[bi])
                beats.append(beat)
            others = []
            if pstream:
                rem = max(1, pend - i)
                ptake = len(pstream) if rem <= 1 else len(pstream) // rem
                others.append(pstream[:ptake])
                pstream = pstream[ptake:]
            if wopending:
                rem = max(1, woend - i)
                wtake = (len(wopending) if rem <= 1
                         else len(wopending) // rem)
                # after proj(3) is done (items >= 11) pjo is mostly idle:
                # route WO psum there and keep its copies off saturated ACT
                wpool, wtag, weng = ((pjo, "pjo", "dve") if i >= 11
                                     else (None, "pw", "mix"))
                others.append(
                    [lambda qt=u[0], np2=u[1], half=u[2]:
                     wo_half(qt, np2, half, wo_obs, pool=wpool,
                             ptag=wtag, copy_eng=weng)
                     for u in wopending[:wtake]])
                wopending = wopending[wtake:]
            merge(beats, *others)
            prev = (sc, h, tiles)
        assert not pstream and not wopending

        # epilogue: PV of the last head, then WO of chunk 3; the score
        # pool's banks are free now, so WO rotates through those too.
        # trans(0) covers q-tiles 12-13, trans(1) covers 14-15: start the
        # WO units for each pair as soon as its OT columns are complete.
        pth = pv_thunks(*prev)
        for th in pth[:-1]:          # pairs + norm + trans(0)
            th()
        epi = [0]
        pools = [(pw, "pw"), (ps, "ps"), (pjo, "pjo")]

        def epi_wo(qt, np2):
            pool, ptag = pools[epi[0] % 3]
            for half in range(2):
                wo_half(qt, np2, half, wo_obs, pool=pool, ptag=ptag,
                        copy_eng="mix", split_dma=True)
            epi[0] += 1

        epi_wo(12, 0)
        pth[-1]()                    # trans(1) overlaps qt-12 copies
        epi_wo(12, 1)
        for qt in range(13, 16):
            for np2 in range(2):
                epi_wo(qt, np2)

    nc.compile()
    return nc


# ------------------------------------------------------------- host side
def _pair_perm64():
    """Column permutation putting the RoPE partner 16 partitions away."""
    return np.array([2 * (16 * (j // 32) + (j % 16)) + ((j % 32) // 16)
                     for j in range(64)])


def _host_prep(x, freqs_cos, freqs_sin, wq, wk, wv, wo):
    _, _, npdt = _dtypes()
    x = np.asarray(x, np.float32)
    fc = np.asarray(freqs_cos, np.float32)
    fs = np.asarray(freqs_sin, np.float32)
    wq = np.asarray(wq, np.float32)
    wk = np.asarray(wk, np.float32)
    wv = np.asarray(wv, np.float32)
    wo = np.asarray(wo, np.float32)

    perm = _pair_perm64()
    xT = np.ascontiguousarray(x[0].T).astype(npdt)

    p = np.arange(64)
    pair = 16 * ((p % 64) // 32) + (p % 16)
    sign = np.where((p % 32) < 16, -1.0, 1.0).astype(np.float32)
    cosE = np.ascontiguousarray(fc[:, pair].T)                  # [64, S]
    sinE = np.ascontiguousarray(fs[:, pair].T) * sign[:, None]  # [64, S]
    utri = np.triu(np.ones((128, 128), np.float32)).astype(npdt)

    in_maps = []
    for c in range(NCORES):
        qcols = np.concatenate(
            [wq[:, (4 * c + i) * 64 + perm] for i in range(HQ)], axis=1)
        kcols = wk[:, c * 64 + perm]
        vcols = wv[:, c * 64:(c + 1) * 64]
        wqkv_c = np.concatenate([qcols, kcols, vcols], axis=1).astype(npdt)
        wo_c = wo[QW * c:QW * (c + 1), :].astype(npdt)
        in_maps.append({
            "xT": xT,
            "wqkv": np.ascontiguousarray(wqkv_c),
            "wo_s": np.ascontiguousarray(wo_c),
            "cosE": cosE.astype(np.float32),
            "sinE": np.ascontiguousarray(sinE).astype(np.float32),
            "utri": np.ascontiguousarray(utri),
        })
    return in_maps


_NC_CACHE = {}


def get_program():
    if MM not in _NC_CACHE:
        _NC_CACHE[MM] = build_program()
    return _NC_CACHE[MM]


def kernel(x, freqs_cos, freqs_sin, wq, wk, wv, wo):
    nc = get_program()
    in_maps = _host_prep(x, freqs_cos, freqs_sin, wq, wk, wv, wo)
    res = run_bass_kernel_spmd(nc, in_maps, core_ids=list(range(NCORES)))
    acc = np.zeros((S, DIM), np.float64)
    for r in res.results:
        acc += r["out"].astype(np.float64)
    return acc.astype(np.float32).reshape(1, S, DIM)


# revision 62
# speedup vs baseline: 1.0339x; 1.0006x over previous
"""Trainium2 Bass kernel for a GQA attention layer (dense transformer).

Reference computation (B=1, S=2048, DIM=2048, 32 q-heads, 8 kv-heads, hd=64):
    xq = x @ wq; xk = x @ wk; xv = x @ wv
    rope(xq, xk); GQA causal attention; out = attn @ wo

Sharding: tensor-parallel over heads across 8 cores. Core c owns q-heads
4c..4c+3 (wq cols), kv-head c (wk/wv cols), and wo rows 256c..256c+255.
Each core computes a full [S, DIM] partial of the output projection; the
host sums the 8 partials (the TP all-reduce, done at gather time).

Kernel layout strategy (everything "transposed", head_dim on partitions):
  - QT/KT/VT = W.T @ x computed with lhsT = weight shard (natural [DIM, m]
    layout), rhs = x.T tiles streamed from DRAM.
  - RoPE pairs are de-interleaved by permuting wq/wk columns on the host so
    the rotation partner sits 16 partitions away (within a 32-partition
    quadrant, so DVE stream_shuffle can swap them).
  - Scores are computed transposed: S^T[k, q] = K^T.T @ Q^T per 128-row
    k-tile; exp on ACT (scale fused); causal mask = upper-tri multiply on
    the single diagonal 128x128 block of each k-tile (on GPSIMD).
  - P@V runs q-major: out[q,hd] = sum_k P^T[k,q].T @ Vp[k,hd|1] per
    (k-tile, 128-q subtile) with N=65 moving columns - 65 PE cycles per
    k-tile instead of 512, full 128x128 array use. The ones column makes
    psum col 64 the softmax denominator.
  - Normalize: one reciprocal per head ([128,4]), then tensor_scalar_mul
    psum->SBUF (per-partition scalar = per-q denom) into O_sb, then PE
    transposes [128q,64] -> [64,128q] (bf16 psum) and 2x-speed DVE copies
    into OT for the output projection.
  - Output projection from O^T with wo shard as rhs; partials DMA'd fp16.
"""

import numpy as np
import ml_dtypes

import concourse.bass as bass
import concourse.mybir as mybir
from concourse import bacc
from concourse.tile import TileContext
from concourse.masks import make_identity
from concourse.bass_utils import run_bass_kernel_spmd

# ---------------------------------------------------------------- constants
S = 2048          # sequence length
DIM = 2048        # model dim
NH = 32           # query heads
NKV = 8           # kv heads
HD = 64           # head dim
NCORES = 8
HQ = NH // NCORES          # query heads per core = 4
QW = HQ * HD               # q width per core = 256
KT_S = S // 128            # 16 seq k-tiles
KT_D = DIM // 128          # 16 dim k-tiles
NSC = S // 512             # 4 s-chunks
SCALE = 1.0 / 8.0          # 1/sqrt(64)
HD1 = HD + 1               # V tile width incl ones column

# matmul dtype knob: 'bf16' | 'f32' | 'f32r'
MM = 'bf16'
MASK_POOL = True           # causal masks on GPSIMD (else DVE)
NWARM = 12                 # p-state warmup transposes
ACT_COPY_EVERY = 4         # every Nth WO psum->sbuf copy goes to ACT

_SHUF_SWAP16 = list(range(16, 32)) + list(range(16))


def _dtypes():
    if MM == 'bf16':
        return mybir.dt.bfloat16, mybir.dt.bfloat16, ml_dtypes.bfloat16
    if MM == 'f32':
        return mybir.dt.float32, mybir.dt.float32, np.float32
    if MM == 'f32r':
        return mybir.dt.float32, mybir.dt.float32r, np.float32
    raise ValueError(MM)


def _mm_ap(ap, mmdt):
    """View an AP in the matmul dtype (bitcast f32 -> f32r when needed)."""
    if ap.dtype != mmdt:
        return ap.bitcast(mmdt)
    return ap


def build_program():
    """Build the per-core Bass program (same program on all 8 cores).

    Emission is a fine-grained software pipeline: attention beats for chunk
    sc (S^T mega-matmul for head h + PV beats of head h-1) are merged with
    the projection matmuls of chunk sc+1 and the WO units of chunk sc-1.

    PSUM (8 banks): pjo 3 (projection passes + PV accumulators, shared tag)
    + ps 4 (two [128,1024] score megas) + pw 1 (WO + transposes).
    """
    sdt, mmdt, _ = _dtypes()
    f32 = mybir.dt.float32
    f16 = mybir.dt.float16

    nc = bacc.Bacc("TRN2", target_bir_lowering=False, debug=False,
                   num_devices=NCORES)

    xT = nc.dram_tensor("xT", [DIM, S], sdt, kind="ExternalInput")
    wqkv = nc.dram_tensor("wqkv", [DIM, QW + 2 * HD], sdt,
                          kind="ExternalInput")
    wo_s = nc.dram_tensor("wo_s", [QW, DIM], sdt, kind="ExternalInput")
    cosE = nc.dram_tensor("cosE", [64, S], f32, kind="ExternalInput")
    sinE = nc.dram_tensor("sinE", [64, S], f32, kind="ExternalInput")
    utri = nc.dram_tensor("utri", [128, 128], sdt, kind="ExternalInput")
    out = nc.dram_tensor("out", [S, DIM], f16, kind="ExternalOutput")

    WQKV = QW + 2 * HD  # 384

    import contextlib
    with TileContext(nc) as tc, contextlib.ExitStack() as ctx:
        const = ctx.enter_context(tc.tile_pool(name="const", bufs=1))
        work = ctx.enter_context(tc.tile_pool(name="work", bufs=2))
        xtp = ctx.enter_context(tc.tile_pool(name="xtp", bufs=10))
        ptp = ctx.enter_context(tc.tile_pool(name="ptp", bufs=20))
        small = ctx.enter_context(tc.tile_pool(name="small", bufs=5))
        osb = ctx.enter_context(tc.tile_pool(name="osb", bufs=4))
        opool = ctx.enter_context(tc.tile_pool(name="opool", bufs=2))

        pjo = ctx.enter_context(tc.tile_pool(name="pjo", bufs=3,
                                             space="PSUM"))
        ps = ctx.enter_context(tc.tile_pool(name="ps", bufs=2, space="PSUM"))
        pw = ctx.enter_context(tc.tile_pool(name="pw", bufs=1, space="PSUM"))

        # ----------------------------------------------- persistent SBUF
        w_sb = const.tile([128, KT_D * WQKV], sdt, tag="w_sb")
        wo_sb = const.tile([128, 2 * DIM], sdt, tag="wo_sb")
        cos_sb = const.tile([128, S], f32, tag="cos_sb")
        sin_sb = const.tile([128, S], f32, tag="sin_sb")
        utri_sb = const.tile([128, 128], sdt, tag="utri_sb")
        ident = const.tile([128, 128], sdt, tag="ident")
        QT = const.tile([64, HQ * S], sdt, tag="QT")
        KVt = const.tile([128, S], sdt, tag="KVt")
        Vp = const.tile([128, KT_S * HD1], sdt, tag="Vp")
        OT = const.tile([128, 2 * S], sdt, tag="OT")

        # p-state warmup: keep PE streaming during the initial DMA wait so
        # the frequency ramp (3us) is over before real matmuls start; the
        # source tile only needs to hold valid bits, so a fast DVE memset
        # unblocks the first transpose ~200ns in
        wsrc = const.tile([128, 128], sdt, tag="wsrc")
        nc.vector.memset(wsrc[:], 0.0)
        warm = pw.tile([128, 128], sdt, tag="pw", name="warm")
        for _ in range(NWARM):
            nc.tensor.transpose(warm[:], wsrc[:], wsrc[:])

        make_identity(nc, ident[:])
        nc.gpsimd.memset(Vp[:], 1.0)  # ones columns for denominator

        wo_copy_cnt = [0]
        xpf = {}  # (sc, batch) -> prefetched x tile

        def prefetch_x(psc, bi, b0, bn):
            xt4 = xtp.tile([128, 4 * 512], sdt, tag="xt", name="xt4")
            nc.sync.dma_start(
                xt4[:, 0:bn * 512].rearrange("r (k c) -> r k c", k=bn),
                xT[b0 * 128:(b0 + bn) * 128,
                   psc * 512:psc * 512 + 512].rearrange(
                       "(k r) c -> r k c", k=bn))
            xpf[(psc, bi)] = xt4

        # ---------------------------------------------- thunk generators
        def proj_thunks(sc, fused=False):
            """Projection of chunk sc: KV pass, K-rope, V transposes, then
            Q passes (one PSUM slot each, sequential). With fused=True
            (prologue) all three matmuls run per k-tile, using 3 slots."""
            s0 = sc * 512
            xts = []
            st = {}

            # small first batches so the first matmul starts early; bigger
            # later ones amortize HWDGE overhead
            batches = [2, 2, 4, 4, 4] if sc == 0 else [4, 4, 4, 4]
            starts = [sum(batches[:i]) for i in range(len(batches))]
            kt_slot = {}
            for bi, (b0, bn) in enumerate(zip(starts, batches)):
                for j in range(bn):
                    kt_slot[b0 + j] = (bi, j, bn, b0)

            def dma_kv(kt):
                bi, j, bn, b0 = kt_slot[kt]
                if j == 0:
                    if sc == 0:
                        nc.sync.dma_start(
                            w_sb[:, b0 * WQKV:(b0 + bn) * WQKV].rearrange(
                                "r (k w) -> r k w", k=bn),
                            wqkv[b0 * 128:(b0 + bn) * 128, :].rearrange(
                                "(k r) w -> r k w", k=bn))
                    if (sc, bi) in xpf:
                        xt4 = xpf.pop((sc, bi))
                    else:
                        xt4 = xtp.tile([128, 4 * 512], sdt, tag="xt",
                                       name="xt4")
                        nc.sync.dma_start(
                            xt4[:, 0:bn * 512].rearrange(
                                "r (k c) -> r k c", k=bn),
                            xT[b0 * 128:(b0 + bn) * 128,
                               s0:s0 + 512].rearrange(
                                   "(k r) c -> r k c", k=bn))
                    xts.append(xt4)
                    if sc == 0 and kt == 12:
                        # chunk 1's first x batch jumps the DMA queue ahead
                        # of the trig loads (k_rope only needs those at
                        # ~15us, while window 1 is starved for x); rows
                        # [64:128] of cos/sin are engine-copied from [0:64]
                        prefetch_x(1, 0, 0, 4)
                        nc.sync.dma_start(cos_sb[0:64, :], cosE[:])
                        nc.sync.dma_start(sin_sb[0:64, :], sinE[:])
                        nc.sync.dma_start(utri_sb[:], utri[:])
                if sc == 0 and kt == 15:
                    prefetch_x(1, 1, 4, 4)
                bi, j, bn, b0 = kt_slot[kt]
                xt = xts[bi][:, j * 512:j * 512 + 512]
                if kt == 0:
                    st["pkv"] = pjo.tile([128, 512], f32, tag="pjo",
                                         name="pkv")
                    if fused:
                        st["fq0"] = pjo.tile([128, 512], f32, tag="pjo",
                                             name="fq0")
                        st["fq1"] = pjo.tile([128, 512], f32, tag="pjo",
                                             name="fq1")
                nc.tensor.matmul(
                    st["pkv"][:],
                    _mm_ap(w_sb[:, kt * WQKV + 256:kt * WQKV + 384], mmdt),
                    _mm_ap(xt, mmdt),
                    start=(kt == 0), stop=(kt == KT_D - 1))
                if fused:
                    for mt in range(2):
                        nc.tensor.matmul(
                            st[f"fq{mt}"][:],
                            _mm_ap(w_sb[:, kt * WQKV + mt * 128:
                                        kt * WQKV + mt * 128 + 128], mmdt),
                            _mm_ap(xt, mmdt),
                            start=(kt == 0), stop=(kt == KT_D - 1))

            def k_rope():
                pkv = st["pkv"]
                shufk = work.tile([64, 512], f32, tag="shufk", name="shufk")
                m1k = work.tile([64, 512], f32, tag="m1k", name="m1k")
                t2k = work.tile([64, 512], f32, tag="t2k", name="t2k")
                nc.vector.stream_shuffle(shufk[:], pkv[0:64, :],
                                         _SHUF_SWAP16)
                nc.vector.tensor_mul(m1k[:], pkv[0:64, :],
                                     cos_sb[0:64, s0:s0 + 512])
                nc.vector.tensor_mul(t2k[:], shufk[:],
                                     sin_sb[0:64, s0:s0 + 512])
                nc.vector.tensor_add(KVt[0:64, s0:s0 + 512], m1k[:], t2k[:])
                nc.scalar.copy(KVt[64:128, s0:s0 + 512],
                               pkv[64:128, :])

            def v_trans(kt):
                pv = pw.tile([128, HD], sdt, tag="pw", name="pv")
                nc.tensor.transpose(
                    pv[:], KVt[64:128, kt * 128:(kt + 1) * 128],
                    ident[64:128, 64:128])
                nc.vector.tensor_copy(
                    Vp[:, kt * HD1:kt * HD1 + HD], pv[:])

            def q_mm(mt, kt):
                if kt == 0:
                    st["pq"] = pjo.tile([128, 512], f32, tag="pjo",
                                        name="pq")
                w0 = kt * WQKV + mt * 128
                bi, j, bn, b0 = kt_slot[kt]
                xt = xts[bi][:, j * 512:j * 512 + 512]
                nc.tensor.matmul(
                    st["pq"][:], _mm_ap(w_sb[:, w0:w0 + 128], mmdt),
                    _mm_ap(xt, mmdt),
                    start=(kt == 0), stop=(kt == KT_D - 1))

            def q_rope(mt):
                pq = st[f"fq{mt}"] if fused else st["pq"]
                shuf = work.tile([128, 512], f32, tag="shuf", name="shuf")
                m1 = work.tile([128, 512], f32, tag="m1", name="m1")
                t2 = work.tile([128, 512], f32, tag="t2", name="t2")
                nc.vector.stream_shuffle(shuf[:], pq[:], _SHUF_SWAP16)
                nc.vector.tensor_mul(m1[:], pq[:], cos_sb[:, s0:s0 + 512])
                nc.vector.tensor_mul(t2[:], shuf[:], sin_sb[:, s0:s0 + 512])
                he = (2 * mt) * S
                ho = (2 * mt + 1) * S
                nc.vector.tensor_add(
                    QT[:, he + s0:he + s0 + 512], m1[0:64, :], t2[0:64, :])
                nc.vector.tensor_add(
                    QT[:, ho + s0:ho + s0 + 512], m1[64:128, :],
                    t2[64:128, :])

            def cs_dup():
                # duplicate cos/sin rows [0:64] into [64:128] for q_rope
                # (cos on idle ACT, sin on idle GPSIMD, in parallel)
                nc.scalar.copy(cos_sb[64:128, :], cos_sb[0:64, :])
                nc.gpsimd.tensor_copy(sin_sb[64:128, :], sin_sb[0:64, :])

            th = [lambda kt=kt: dma_kv(kt) for kt in range(KT_D)]
            th.append(k_rope)
            if sc == 0:
                th.append(cs_dup)
            th += [lambda kt=kt: v_trans(kt)
                   for kt in range(4 * sc, 4 * sc + 4)]
            if fused:
                th += [lambda mt=mt: q_rope(mt) for mt in range(2)]
            else:
                for mt in range(2):
                    th += [lambda mt=mt, kt=kt: q_mm(mt, kt)
                           for kt in range(KT_D)]
                    th.append(lambda mt=mt: q_rope(mt))
            return th

        def s_thunks(qc, h, tiles):
            """S^T mega matmuls + exp + mask for one head; fills `tiles`."""
            q0 = qc * 512
            hf = h * S
            nkt = 4 * qc + 4
            thunks = []
            for pi in range(nkt // 2):
                def th(pi=pi):
                    kts = (2 * pi, 2 * pi + 1)
                    ps_t = ps.tile([128, 1024], f32, tag="ps", name="ps_t")
                    pt_t = ptp.tile([128, 1024], sdt, tag="pt", name="pt_t")
                    for li, kt in enumerate(kts):
                        dj = kt - 4 * qc
                        qo = 128 * dj if dj >= 0 else 0
                        lo = li * 512
                        nc.tensor.matmul(
                            ps_t[:, lo + qo:lo + 512],
                            _mm_ap(KVt[0:64, kt * 128:(kt + 1) * 128], mmdt),
                            _mm_ap(QT[:, hf + q0 + qo:hf + q0 + 512], mmdt),
                            start=True, stop=True)
                    if 2 * pi + 1 < 4 * qc or (qc >= 1
                                               and 2 * pi == 4 * qc):
                        # first diag pair (dj 0,1) of chunks >= 1: exp the
                        # full mega in one op; the extra columns hold stale
                        # scores (bounded, and never read by the series-
                        # major PV), and one wide op beats two narrow ones
                        # on the exp-saturated ACT engine
                        nc.scalar.activation(
                            pt_t[:], ps_t[:],
                            mybir.ActivationFunctionType.Exp, scale=SCALE)
                    else:
                        for li, kt in enumerate(kts):
                            dj = kt - 4 * qc
                            qo = 128 * dj if dj >= 0 else 0
                            lo = li * 512
                            nc.scalar.activation(
                                pt_t[:, lo + qo:lo + 512],
                                ps_t[:, lo + qo:lo + 512],
                                mybir.ActivationFunctionType.Exp,
                                scale=SCALE)
                    for li, kt in enumerate(kts):
                        dj = kt - 4 * qc
                        qo = 128 * dj if dj >= 0 else 0
                        lo = li * 512
                        if dj >= 0:
                            eng = nc.gpsimd if MASK_POOL else nc.vector
                            eng.tensor_mul(
                                pt_t[:, lo + qo:lo + qo + 128],
                                pt_t[:, lo + qo:lo + qo + 128], utri_sb[:])
                        tiles.append((kt, qo, lo, pt_t))
                thunks.append(th)
            return thunks

        def pv_thunks(qc, h, tiles):
            """q-major PV accumulation + normalization + O transposes.

            For each incoming P^T tile (k-tile kt), run the N=65 matmuls for
            each live 128-q subtile j: out[q,hd|den] += P^T[:,j*128:].T@Vp.
            After the last k-tile: reciprocal of the denominators, normalize
            psum->O_sb (bf16), PE-transpose to O^T, 2x DVE copy into OT.
            """
            q0 = qc * 512
            hp = (h % 2) * 64
            of = (h // 2) * S
            nkt0 = 4 * qc + 4
            state = {}

            def pv_series(j):
                """All matmuls of q-subtile j back-to-back: start=True
                clears the whole bank's has_written bits, so accumulation
                series sharing a psum bank must not interleave."""
                if "pv" not in state:
                    state["pv"] = pjo.tile([128, 512], f32, tag="pjo",
                                           name="pvt")
                pv_t = state["pv"]
                for kt in range(0, 4 * qc + j + 1):
                    _, qo, lo, pt_t = tiles[kt]
                    nc.tensor.matmul(
                        pv_t[:, j * 128:j * 128 + HD1],
                        _mm_ap(pt_t[:, lo + j * 128:
                                    lo + (j + 1) * 128], mmdt),
                        _mm_ap(Vp[:, kt * HD1:(kt + 1) * HD1], mmdt),
                        start=(kt == 0), stop=(kt == 4 * qc + j))

            def norm():
                pv_t = state["pv"]
                rc = small.tile([128, 4], f32, tag="rc", name="rc")
                dsb = small.tile([128, 4], f32, tag="dsb", name="dsb")
                o_sb = state["o_sb"] = osb_head()
                pvr = pv_t.rearrange("p (b c) -> p b c", c=128)
                nc.vector.tensor_copy(
                    dsb[:].rearrange("p (b c) -> p b c", c=1),
                    pvr[:, :, HD:HD + 1])
                nc.vector.reciprocal(rc[:], dsb[:])
                for j in range(4):
                    nc.vector.tensor_scalar_mul(
                        o_sb[:, j * HD:(j + 1) * HD],
                        pv_t[:, j * 128:j * 128 + HD],
                        rc[:, j:j + 1])

            def trans(half):
                # transpose via a regular matmul against the identity:
                # out[hd, q] = o_sb[q, hd]^T @ I -- f32 psum output, so
                # nonzero free offsets behave like the score megas
                o_sb = state["o_sb"]
                tp = pw.tile([64, 256], f32, tag="pw", name="tp")
                for st2 in range(2):
                    stg = 2 * half + st2
                    nc.tensor.matmul(
                        tp[:, st2 * 128:(st2 + 1) * 128],
                        _mm_ap(o_sb[:, stg * HD:(stg + 1) * HD], mmdt),
                        _mm_ap(ident[:], mmdt),
                        start=True, stop=True)
                oc_eng = nc.scalar if qc <= 1 else nc.vector
                (oc_eng.copy if qc <= 1 else nc.vector.tensor_copy)(
                    OT[hp:hp + 64, of + q0 + half * 256:
                       of + q0 + half * 256 + 256], tp[:])

            def osb_head():
                return opool.tile([128, 4 * HD], sdt, tag=f"osb{h % 2}",
                                  name="o_sb")

            th = [lambda j=j: pv_series(j) for j in range(4)]
            th.append(norm)
            th += [lambda half=half: trans(half) for half in range(2)]
            return th

        def wo_half(qt, np2, half, obs, pool=None, ptag="pw",
                    copy_eng="mix", split_dma=False):
            """One 512-wide n-chunk of out row-block qt; the final chunk
            fires one [128,2048] fp16 DMA for the whole row-block (fewer
            HWDGE round-trips than per-chunk DMAs). The epilogue splits
            per-np2 ([128,1024]) so the last transfer is shorter."""
            pool = pool or pw
            if (np2, half) == (0, 0):
                obs[qt] = osb.tile([128, 2048], f16, tag="ob", name="ob")
            ob = obs[qt]
            ncn = 2 * np2 + half
            pw_t = pool.tile([128, 512], f32, tag=ptag, name="pw_t")
            for mt in range(2):
                nc.tensor.matmul(
                    pw_t[:],
                    _mm_ap(OT[:, mt * S + qt * 128:
                              mt * S + (qt + 1) * 128], mmdt),
                    _mm_ap(wo_sb[:, mt * DIM + ncn * 512:
                                 mt * DIM + ncn * 512 + 512], mmdt),
                    start=(mt == 0), stop=(mt == 1))
            wo_copy_cnt[0] += 1
            use_act = {"mix": wo_copy_cnt[0] % 3 == 0,
                       "dve": False,
                       "alt": wo_copy_cnt[0] % 2 == 1}[copy_eng]
            if use_act:
                nc.scalar.copy(ob[:, ncn * 512:ncn * 512 + 512], pw_t[:])
            else:
                nc.vector.tensor_copy(
                    ob[:, ncn * 512:ncn * 512 + 512], pw_t[:])
            if split_dma and half == 1:
                nc.sync.dma_start(
                    out[qt * 128:(qt + 1) * 128,
                        np2 * 1024:np2 * 1024 + 1024],
                    ob[:, np2 * 1024:np2 * 1024 + 1024])
                if np2 == 1:
                    del obs[qt]
            elif not split_dma and (np2, half) == (1, 1):
                del obs[qt]
                nc.sync.dma_start(
                    out[qt * 128:(qt + 1) * 128, :], ob[:])
        wo_obs = {}

        # ------------------------------------- merged emission schedule
        def merge(primary, *others):
            """Emit primary thunks; proportionally interleave the others."""
            counters = [0.0] * len(others)
            n = max(1, len(primary))
            for beat in primary:
                for j, lst in enumerate(others):
                    counters[j] += len(lst) / n
                    while counters[j] >= 1.0 and lst:
                        lst.pop(0)()
                        counters[j] -= 1.0
                for th in beat:
                    th()
            for lst in others:
                while lst:
                    lst.pop(0)()

        for th in proj_thunks(0, fused=True):       # prologue
            th()

        # Head processing order: chunk 3's head 0 is pulled forward between
        # (2,2) and (2,3) so part of the causal-triangle-heavy chunk-3 exp
        # load runs while ACT still has slack, instead of piling up at the
        # end where exp rate-limits the whole pipeline.
        ITEMS = [(0, 0), (0, 1), (0, 2), (0, 3),
                 (1, 0), (1, 1), (1, 2), (1, 3),
                 (2, 0), (2, 1), (2, 2), (3, 0), (2, 3),
                 (3, 1), (3, 2), (3, 3)]
        # proj(sc) spread over items [a, b) — must drain before the first
        # (sc, *) item; WO(sc) over [a, b) — may start only after the item
        # containing trans(sc, 3), i.e. one past (sc, 3)'s position.
        PROJ_AT = {1: (0, 4), 2: (4, 8), 3: (8, 11)}
        WO_AT = {0: (5, 8), 1: (9, 13), 2: (14, 16)}
        proj_by_start = {a: s for s, (a, b) in PROJ_AT.items()}
        wo_by_start = {a: s for s, (a, b) in WO_AT.items()}

        prev = None                      # (qc, h, tiles) awaiting PV
        pstream, pend = [], 0
        wopending, woend = [], 0
        for i, (sc, h) in enumerate(ITEMS):
            if i == 4:
                nc.sync.dma_start(wo_sb[:, 0:DIM], wo_s[0:128, :])
                nc.sync.dma_start(wo_sb[:, DIM:2 * DIM], wo_s[128:256, :])
            if i in proj_by_start:
                pstream = proj_thunks(proj_by_start[i])
                pend = PROJ_AT[proj_by_start[i]][1]
            if i in wo_by_start:
                s = wo_by_start[i]
                wopending = [(qt, np2, half)
                             for qt in range(4 * s, 4 * s + 4)
                             for np2 in range(2)
                             for half in range(2)]
                woend = WO_AT[s][1]
            tiles = []
            sth = s_thunks(sc, h, tiles)
            pth = pv_thunks(*prev) if prev is not None else []
            beats = []
            for bi in range(max(len(sth), len(pth))):
                beat = []
                if bi < len(pth):
                    beat.append(pth[bi])
                if bi < len(sth):
                    beat.append(sth[bi])
                beats.append(beat)
            others = []
            if pstream:
                rem = max(1, pend - i)
                ptake = len(pstream) if rem <= 1 else len(pstream) // rem
                others.append(pstream[:ptake])
                pstream = pstream[ptake:]
            if wopending:
                rem = max(1, woend - i)
                wtake = (len(wopending) if rem <= 1
                         else len(wopending) // rem)
                # after proj(3) is done (items >= 11) pjo is mostly idle:
                # route WO psum there and keep its copies off saturated ACT
                wpool, wtag, weng = ((pjo, "pjo", "dve") if i >= 11
                                     else (None, "pw", "mix"))
                others.append(
                    [lambda qt=u[0], np2=u[1], half=u[2]:
                     wo_half(qt, np2, half, wo_obs, pool=wpool,
                             ptag=wtag, copy_eng=weng)
                     for u in wopending[:wtake]])
                wopending = wopending[wtake:]
            merge(beats, *others)
            prev = (sc, h, tiles)
        assert not pstream and not wopending

        # epilogue: PV of the last head, then WO of chunk 3; the score
        # pool's banks are free now, so WO rotates through those too.
        # trans(0) covers q-tiles 12-13, trans(1) covers 14-15: start the
        # WO units for each pair as soon as its OT columns are complete.
        pth = pv_thunks(*prev)
        for th in pth[:-1]:          # pairs + norm + trans(0)
            th()
        epi = [0]
        pools = [(pw, "pw"), (ps, "ps"), (pjo, "pjo")]

        def epi_wo(qt, np2):
            pool, ptag = pools[epi[0] % 3]
            for half in range(2):
                wo_half(qt, np2, half, wo_obs, pool=pool, ptag=ptag,
                        copy_eng="mix", split_dma=True)
            epi[0] += 1

        epi_wo(12, 0)
        pth[-1]()                    # trans(1) overlaps qt-12 copies
        epi_wo(12, 1)
        for qt in range(13, 16):
            for np2 in range(2):
                epi_wo(qt, np2)

    nc.compile()
    return nc


# ------------------------------------------------------------- host side
def _pair_perm64():
    """Column permutation putting the RoPE partner 16 partitions away."""
    return np.array([2 * (16 * (j // 32) + (j % 16)) + ((j % 32) // 16)
                     for j in range(64)])


def _host_prep(x, freqs_cos, freqs_sin, wq, wk, wv, wo):
    _, _, npdt = _dtypes()
    x = np.asarray(x, np.float32)
    fc = np.asarray(freqs_cos, np.float32)
    fs = np.asarray(freqs_sin, np.float32)
    wq = np.asarray(wq, np.float32)
    wk = np.asarray(wk, np.float32)
    wv = np.asarray(wv, np.float32)
    wo = np.asarray(wo, np.float32)

    perm = _pair_perm64()
    xT = np.ascontiguousarray(x[0].T).astype(npdt)

    p = np.arange(64)
    pair = 16 * ((p % 64) // 32) + (p % 16)
    sign = np.where((p % 32) < 16, -1.0, 1.0).astype(np.float32)
    cosE = np.ascontiguousarray(fc[:, pair].T)                  # [64, S]
    sinE = np.ascontiguousarray(fs[:, pair].T) * sign[:, None]  # [64, S]
    utri = np.triu(np.ones((128, 128), np.float32)).astype(npdt)

    in_maps = []
    for c in range(NCORES):
        qcols = np.concatenate(
            [wq[:, (4 * c + i) * 64 + perm] for i in range(HQ)], axis=1)
        kcols = wk[:, c * 64 + perm]
        vcols = wv[:, c * 64:(c + 1) * 64]
        wqkv_c = np.concatenate([qcols, kcols, vcols], axis=1).astype(npdt)
        wo_c = wo[QW * c:QW * (c + 1), :].astype(npdt)
        in_maps.append({
            "xT": xT,
            "wqkv": np.ascontiguousarray(wqkv_c),
            "wo_s": np.ascontiguousarray(wo_c),
            "cosE": cosE.astype(np.float32),
            "sinE": np.ascontiguousarray(sinE).astype(np.float32),
            "utri": np.ascontiguousarray(utri),
        })
    return in_maps


_NC_CACHE = {}


def get_program():
    if MM not in _NC_CACHE:
        _NC_CACHE[MM] = build_program()
    return _NC_CACHE[MM]


def kernel(x, freqs_cos, freqs_sin, wq, wk, wv, wo):
    nc = get_program()
    in_maps = _host_prep(x, freqs_cos, freqs_sin, wq, wk, wv, wo)
    res = run_bass_kernel_spmd(nc, in_maps, core_ids=list(range(NCORES)))
    acc = np.zeros((S, DIM), np.float64)
    for r in res.results:
        acc += r["out"].astype(np.float64)
    return acc.astype(np.float32).reshape(1, S, DIM)


# revision 63
# speedup vs baseline: 1.0355x; 1.0016x over previous
"""Trainium2 Bass kernel for a GQA attention layer (dense transformer).

Reference computation (B=1, S=2048, DIM=2048, 32 q-heads, 8 kv-heads, hd=64):
    xq = x @ wq; xk = x @ wk; xv = x @ wv
    rope(xq, xk); GQA causal attention; out = attn @ wo

Sharding: tensor-parallel over heads across 8 cores. Core c owns q-heads
4c..4c+3 (wq cols), kv-head c (wk/wv cols), and wo rows 256c..256c+255.
Each core computes a full [S, DIM] partial of the output projection; the
host sums the 8 partials (the TP all-reduce, done at gather time).

Kernel layout strategy (everything "transposed", head_dim on partitions):
  - QT/KT/VT = W.T @ x computed with lhsT = weight shard (natural [DIM, m]
    layout), rhs = x.T tiles streamed from DRAM.
  - RoPE pairs are de-interleaved by permuting wq/wk columns on the host so
    the rotation partner sits 16 partitions away (within a 32-partition
    quadrant, so DVE stream_shuffle can swap them).
  - Scores are computed transposed: S^T[k, q] = K^T.T @ Q^T per 128-row
    k-tile; exp on ACT (scale fused); causal mask = upper-tri multiply on
    the single diagonal 128x128 block of each k-tile (on GPSIMD).
  - P@V runs q-major: out[q,hd] = sum_k P^T[k,q].T @ Vp[k,hd|1] per
    (k-tile, 128-q subtile) with N=65 moving columns - 65 PE cycles per
    k-tile instead of 512, full 128x128 array use. The ones column makes
    psum col 64 the softmax denominator.
  - Normalize: one reciprocal per head ([128,4]), then tensor_scalar_mul
    psum->SBUF (per-partition scalar = per-q denom) into O_sb, then PE
    transposes [128q,64] -> [64,128q] (bf16 psum) and 2x-speed DVE copies
    into OT for the output projection.
  - Output projection from O^T with wo shard as rhs; partials DMA'd fp16.
"""

import numpy as np
import ml_dtypes

import concourse.bass as bass
import concourse.mybir as mybir
from concourse import bacc
from concourse.tile import TileContext
from concourse.masks import make_identity
from concourse.bass_utils import run_bass_kernel_spmd

# ---------------------------------------------------------------- constants
S = 2048          # sequence length
DIM = 2048        # model dim
NH = 32           # query heads
NKV = 8           # kv heads
HD = 64           # head dim
NCORES = 8
HQ = NH // NCORES          # query heads per core = 4
QW = HQ * HD               # q width per core = 256
KT_S = S // 128            # 16 seq k-tiles
KT_D = DIM // 128          # 16 dim k-tiles
NSC = S // 512             # 4 s-chunks
SCALE = 1.0 / 8.0          # 1/sqrt(64)
HD1 = HD + 1               # V tile width incl ones column

# matmul dtype knob: 'bf16' | 'f32' | 'f32r'
MM = 'bf16'
MASK_POOL = True           # causal masks on GPSIMD (else DVE)
NWARM = 12                 # p-state warmup transposes
ACT_COPY_EVERY = 4         # every Nth WO psum->sbuf copy goes to ACT

_SHUF_SWAP16 = list(range(16, 32)) + list(range(16))


def _dtypes():
    if MM == 'bf16':
        return mybir.dt.bfloat16, mybir.dt.bfloat16, ml_dtypes.bfloat16
    if MM == 'f32':
        return mybir.dt.float32, mybir.dt.float32, np.float32
    if MM == 'f32r':
        return mybir.dt.float32, mybir.dt.float32r, np.float32
    raise ValueError(MM)


def _mm_ap(ap, mmdt):
    """View an AP in the matmul dtype (bitcast f32 -> f32r when needed)."""
    if ap.dtype != mmdt:
        return ap.bitcast(mmdt)
    return ap


def build_program():
    """Build the per-core Bass program (same program on all 8 cores).

    Emission is a fine-grained software pipeline: attention beats for chunk
    sc (S^T mega-matmul for head h + PV beats of head h-1) are merged with
    the projection matmuls of chunk sc+1 and the WO units of chunk sc-1.

    PSUM (8 banks): pjo 3 (projection passes + PV accumulators, shared tag)
    + ps 4 (two [128,1024] score megas) + pw 1 (WO + transposes).
    """
    sdt, mmdt, _ = _dtypes()
    f32 = mybir.dt.float32
    f16 = mybir.dt.float16

    nc = bacc.Bacc("TRN2", target_bir_lowering=False, debug=False,
                   num_devices=NCORES)

    xT = nc.dram_tensor("xT", [DIM, S], sdt, kind="ExternalInput")
    wqkv = nc.dram_tensor("wqkv", [DIM, QW + 2 * HD], sdt,
                          kind="ExternalInput")
    wo_s = nc.dram_tensor("wo_s", [QW, DIM], sdt, kind="ExternalInput")
    cosE = nc.dram_tensor("cosE", [64, S], f32, kind="ExternalInput")
    sinE = nc.dram_tensor("sinE", [64, S], f32, kind="ExternalInput")
    utri = nc.dram_tensor("utri", [128, 128], sdt, kind="ExternalInput")
    out = nc.dram_tensor("out", [S, DIM], f16, kind="ExternalOutput")

    WQKV = QW + 2 * HD  # 384

    import contextlib
    with TileContext(nc) as tc, contextlib.ExitStack() as ctx:
        const = ctx.enter_context(tc.tile_pool(name="const", bufs=1))
        work = ctx.enter_context(tc.tile_pool(name="work", bufs=2))
        xtp = ctx.enter_context(tc.tile_pool(name="xtp", bufs=10))
        ptp = ctx.enter_context(tc.tile_pool(name="ptp", bufs=20))
        small = ctx.enter_context(tc.tile_pool(name="small", bufs=5))
        osb = ctx.enter_context(tc.tile_pool(name="osb", bufs=4))
        opool = ctx.enter_context(tc.tile_pool(name="opool", bufs=2))

        pjo = ctx.enter_context(tc.tile_pool(name="pjo", bufs=3,
                                             space="PSUM"))
        ps = ctx.enter_context(tc.tile_pool(name="ps", bufs=2, space="PSUM"))
        pw = ctx.enter_context(tc.tile_pool(name="pw", bufs=1, space="PSUM"))

        # ----------------------------------------------- persistent SBUF
        w_sb = const.tile([128, KT_D * WQKV], sdt, tag="w_sb")
        wo_sb = const.tile([128, 2 * DIM], sdt, tag="wo_sb")
        cos_sb = const.tile([128, S], f32, tag="cos_sb")
        sin_sb = const.tile([128, S], f32, tag="sin_sb")
        utri_sb = const.tile([128, 128], sdt, tag="utri_sb")
        ident = const.tile([128, 128], sdt, tag="ident")
        QT = const.tile([64, HQ * S], sdt, tag="QT")
        KVt = const.tile([128, S], sdt, tag="KVt")
        Vp = const.tile([128, KT_S * HD1], sdt, tag="Vp")
        OT = const.tile([128, 2 * S], sdt, tag="OT")

        # p-state warmup: keep PE streaming during the initial DMA wait so
        # the frequency ramp (3us) is over before real matmuls start; the
        # source tile only needs to hold valid bits, so a fast DVE memset
        # unblocks the first transpose ~200ns in
        wsrc = const.tile([128, 128], sdt, tag="wsrc")
        nc.vector.memset(wsrc[:], 0.0)
        warm = pw.tile([128, 128], sdt, tag="pw", name="warm")
        for _ in range(NWARM):
            nc.tensor.transpose(warm[:], wsrc[:], wsrc[:])

        make_identity(nc, ident[:])
        nc.gpsimd.memset(Vp[:], 1.0)  # ones columns for denominator

        wo_copy_cnt = [0]
        xpf = {}  # (sc, batch) -> prefetched x tile

        def prefetch_x(psc, bi, b0, bn):
            xt4 = xtp.tile([128, 4 * 512], sdt, tag="xt", name="xt4")
            nc.sync.dma_start(
                xt4[:, 0:bn * 512].rearrange("r (k c) -> r k c", k=bn),
                xT[b0 * 128:(b0 + bn) * 128,
                   psc * 512:psc * 512 + 512].rearrange(
                       "(k r) c -> r k c", k=bn))
            xpf[(psc, bi)] = xt4

        # ---------------------------------------------- thunk generators
        def proj_thunks(sc, fused=False):
            """Projection of chunk sc: KV pass, K-rope, V transposes, then
            Q passes (one PSUM slot each, sequential). With fused=True
            (prologue) all three matmuls run per k-tile, using 3 slots."""
            s0 = sc * 512
            xts = []
            st = {}

            # small first batches so the first matmul starts early; bigger
            # later ones amortize HWDGE overhead
            batches = [2, 2, 4, 4, 4] if sc == 0 else [4, 4, 4, 4]
            starts = [sum(batches[:i]) for i in range(len(batches))]
            kt_slot = {}
            for bi, (b0, bn) in enumerate(zip(starts, batches)):
                for j in range(bn):
                    kt_slot[b0 + j] = (bi, j, bn, b0)

            def dma_kv(kt):
                bi, j, bn, b0 = kt_slot[kt]
                if j == 0:
                    if sc == 0:
                        nc.sync.dma_start(
                            w_sb[:, b0 * WQKV:(b0 + bn) * WQKV].rearrange(
                                "r (k w) -> r k w", k=bn),
                            wqkv[b0 * 128:(b0 + bn) * 128, :].rearrange(
                                "(k r) w -> r k w", k=bn))
                    if (sc, bi) in xpf:
                        xt4 = xpf.pop((sc, bi))
                    else:
                        xt4 = xtp.tile([128, 4 * 512], sdt, tag="xt",
                                       name="xt4")
                        nc.sync.dma_start(
                            xt4[:, 0:bn * 512].rearrange(
                                "r (k c) -> r k c", k=bn),
                            xT[b0 * 128:(b0 + bn) * 128,
                               s0:s0 + 512].rearrange(
                                   "(k r) c -> r k c", k=bn))
                    xts.append(xt4)
                    if sc == 0 and kt == 12:
                        # chunk 1's first x batch jumps the DMA queue ahead
                        # of the trig loads (k_rope only needs those at
                        # ~15us, while window 1 is starved for x); rows
                        # [64:128] of cos/sin are engine-copied from [0:64]
                        prefetch_x(1, 0, 0, 4)
                        nc.sync.dma_start(cos_sb[0:64, :], cosE[:])
                        nc.sync.dma_start(sin_sb[0:64, :], sinE[:])
                        nc.sync.dma_start(utri_sb[:], utri[:])
                if sc == 0 and kt == 15:
                    prefetch_x(1, 1, 4, 4)
                bi, j, bn, b0 = kt_slot[kt]
                xt = xts[bi][:, j * 512:j * 512 + 512]
                if kt == 0:
                    st["pkv"] = pjo.tile([128, 512], f32, tag="pjo",
                                         name="pkv")
                    if fused:
                        st["fq0"] = pjo.tile([128, 512], f32, tag="pjo",
                                             name="fq0")
                        st["fq1"] = pjo.tile([128, 512], f32, tag="pjo",
                                             name="fq1")
                nc.tensor.matmul(
                    st["pkv"][:],
                    _mm_ap(w_sb[:, kt * WQKV + 256:kt * WQKV + 384], mmdt),
                    _mm_ap(xt, mmdt),
                    start=(kt == 0), stop=(kt == KT_D - 1))
                if fused:
                    for mt in range(2):
                        nc.tensor.matmul(
                            st[f"fq{mt}"][:],
                            _mm_ap(w_sb[:, kt * WQKV + mt * 128:
                                        kt * WQKV + mt * 128 + 128], mmdt),
                            _mm_ap(xt, mmdt),
                            start=(kt == 0), stop=(kt == KT_D - 1))

            def k_rope():
                pkv = st["pkv"]
                shufk = work.tile([64, 512], f32, tag="shufk", name="shufk")
                m1k = work.tile([64, 512], f32, tag="m1k", name="m1k")
                t2k = work.tile([64, 512], f32, tag="t2k", name="t2k")
                nc.vector.stream_shuffle(shufk[:], pkv[0:64, :],
                                         _SHUF_SWAP16)
                nc.vector.tensor_mul(m1k[:], pkv[0:64, :],
                                     cos_sb[0:64, s0:s0 + 512])
                nc.vector.tensor_mul(t2k[:], shufk[:],
                                     sin_sb[0:64, s0:s0 + 512])
                nc.vector.tensor_add(KVt[0:64, s0:s0 + 512], m1k[:], t2k[:])
                nc.scalar.copy(KVt[64:128, s0:s0 + 512],
                               pkv[64:128, :])

            def v_trans(kt):
                pv = pw.tile([128, HD], sdt, tag="pw", name="pv")
                nc.tensor.transpose(
                    pv[:], KVt[64:128, kt * 128:(kt + 1) * 128],
                    ident[64:128, 64:128])
                nc.vector.tensor_copy(
                    Vp[:, kt * HD1:kt * HD1 + HD], pv[:])

            def q_mm(mt, kt):
                if kt == 0:
                    st["pq"] = pjo.tile([128, 512], f32, tag="pjo",
                                        name="pq")
                w0 = kt * WQKV + mt * 128
                bi, j, bn, b0 = kt_slot[kt]
                xt = xts[bi][:, j * 512:j * 512 + 512]
                nc.tensor.matmul(
                    st["pq"][:], _mm_ap(w_sb[:, w0:w0 + 128], mmdt),
                    _mm_ap(xt, mmdt),
                    start=(kt == 0), stop=(kt == KT_D - 1))

            def q_rope(mt):
                pq = st[f"fq{mt}"] if fused else st["pq"]
                shuf = work.tile([128, 512], f32, tag="shuf", name="shuf")
                m1 = work.tile([128, 512], f32, tag="m1", name="m1")
                t2 = work.tile([128, 512], f32, tag="t2", name="t2")
                nc.vector.stream_shuffle(shuf[:], pq[:], _SHUF_SWAP16)
                nc.vector.tensor_mul(m1[:], pq[:], cos_sb[:, s0:s0 + 512])
                nc.vector.tensor_mul(t2[:], shuf[:], sin_sb[:, s0:s0 + 512])
                he = (2 * mt) * S
                ho = (2 * mt + 1) * S
                nc.vector.tensor_add(
                    QT[:, he + s0:he + s0 + 512], m1[0:64, :], t2[0:64, :])
                nc.vector.tensor_add(
                    QT[:, ho + s0:ho + s0 + 512], m1[64:128, :],
                    t2[64:128, :])

            def cs_dup():
                # duplicate cos/sin rows [0:64] into [64:128] for q_rope
                # (cos on idle ACT, sin on idle GPSIMD, in parallel)
                nc.scalar.copy(cos_sb[64:128, :], cos_sb[0:64, :])
                nc.gpsimd.tensor_copy(sin_sb[64:128, :], sin_sb[0:64, :])

            th = [lambda kt=kt: dma_kv(kt) for kt in range(KT_D)]
            th.append(k_rope)
            if sc == 0:
                th.append(cs_dup)
            th += [lambda kt=kt: v_trans(kt)
                   for kt in range(4 * sc, 4 * sc + 4)]
            if fused:
                th += [lambda mt=mt: q_rope(mt) for mt in range(2)]
            else:
                for mt in range(2):
                    th += [lambda mt=mt, kt=kt: q_mm(mt, kt)
                           for kt in range(KT_D)]
                    th.append(lambda mt=mt: q_rope(mt))
            return th

        def s_thunks(qc, h, tiles):
            """S^T mega matmuls + exp + mask for one head; fills `tiles`."""
            q0 = qc * 512
            hf = h * S
            nkt = 4 * qc + 4
            thunks = []
            for pi in range(nkt // 2):
                def th(pi=pi):
                    kts = (2 * pi, 2 * pi + 1)
                    ps_t = ps.tile([128, 1024], f32, tag="ps", name="ps_t")
                    pt_t = ptp.tile([128, 1024], sdt, tag="pt", name="pt_t")
                    for li, kt in enumerate(kts):
                        dj = kt - 4 * qc
                        qo = 128 * dj if dj >= 0 else 0
                        lo = li * 512
                        nc.tensor.matmul(
                            ps_t[:, lo + qo:lo + 512],
                            _mm_ap(KVt[0:64, kt * 128:(kt + 1) * 128], mmdt),
                            _mm_ap(QT[:, hf + q0 + qo:hf + q0 + 512], mmdt),
                            start=True, stop=True)
                    if 2 * pi + 1 < 4 * qc or (qc >= 1
                                               and 2 * pi == 4 * qc):
                        # first diag pair (dj 0,1) of chunks >= 1: exp the
                        # full mega in one op; the extra columns hold stale
                        # scores (bounded, and never read by the series-
                        # major PV), and one wide op beats two narrow ones
                        # on the exp-saturated ACT engine
                        nc.scalar.activation(
                            pt_t[:], ps_t[:],
                            mybir.ActivationFunctionType.Exp, scale=SCALE)
                    else:
                        for li, kt in enumerate(kts):
                            dj = kt - 4 * qc
                            qo = 128 * dj if dj >= 0 else 0
                            lo = li * 512
                            nc.scalar.activation(
                                pt_t[:, lo + qo:lo + 512],
                                ps_t[:, lo + qo:lo + 512],
                                mybir.ActivationFunctionType.Exp,
                                scale=SCALE)
                    for li, kt in enumerate(kts):
                        dj = kt - 4 * qc
                        qo = 128 * dj if dj >= 0 else 0
                        lo = li * 512
                        if dj >= 0:
                            eng = nc.gpsimd if MASK_POOL else nc.vector
                            eng.tensor_mul(
                                pt_t[:, lo + qo:lo + qo + 128],
                                pt_t[:, lo + qo:lo + qo + 128], utri_sb[:])
                        tiles.append((kt, qo, lo, pt_t))
                thunks.append(th)
            return thunks

        def pv_thunks(qc, h, tiles):
            """q-major PV accumulation + normalization + O transposes.

            For each incoming P^T tile (k-tile kt), run the N=65 matmuls for
            each live 128-q subtile j: out[q,hd|den] += P^T[:,j*128:].T@Vp.
            After the last k-tile: reciprocal of the denominators, normalize
            psum->O_sb (bf16), PE-transpose to O^T, 2x DVE copy into OT.
            """
            q0 = qc * 512
            hp = (h % 2) * 64
            of = (h // 2) * S
            nkt0 = 4 * qc + 4
            state = {}

            def pv_series(j):
                """All matmuls of q-subtile j back-to-back: start=True
                clears the whole bank's has_written bits, so accumulation
                series sharing a psum bank must not interleave."""
                if "pv" not in state:
                    state["pv"] = pjo.tile([128, 512], f32, tag="pjo",
                                           name="pvt")
                pv_t = state["pv"]
                for kt in range(0, 4 * qc + j + 1):
                    _, qo, lo, pt_t = tiles[kt]
                    nc.tensor.matmul(
                        pv_t[:, j * 128:j * 128 + HD1],
                        _mm_ap(pt_t[:, lo + j * 128:
                                    lo + (j + 1) * 128], mmdt),
                        _mm_ap(Vp[:, kt * HD1:(kt + 1) * HD1], mmdt),
                        start=(kt == 0), stop=(kt == 4 * qc + j))

            def norm():
                pv_t = state["pv"]
                rc = small.tile([128, 4], f32, tag="rc", name="rc")
                dsb = small.tile([128, 4], f32, tag="dsb", name="dsb")
                o_sb = state["o_sb"] = osb_head()
                pvr = pv_t.rearrange("p (b c) -> p b c", c=128)
                nc.vector.tensor_copy(
                    dsb[:].rearrange("p (b c) -> p b c", c=1),
                    pvr[:, :, HD:HD + 1])
                nc.vector.reciprocal(rc[:], dsb[:])
                for j in range(4):
                    nc.vector.tensor_scalar_mul(
                        o_sb[:, j * HD:(j + 1) * HD],
                        pv_t[:, j * 128:j * 128 + HD],
                        rc[:, j:j + 1])

            def trans(half):
                # transpose via a regular matmul against the identity:
                # out[hd, q] = o_sb[q, hd]^T @ I -- f32 psum output, so
                # nonzero free offsets behave like the score megas
                o_sb = state["o_sb"]
                tp = pw.tile([64, 256], f32, tag="pw", name="tp")
                for st2 in range(2):
                    stg = 2 * half + st2
                    nc.tensor.matmul(
                        tp[:, st2 * 128:(st2 + 1) * 128],
                        _mm_ap(o_sb[:, stg * HD:(stg + 1) * HD], mmdt),
                        _mm_ap(ident[:], mmdt),
                        start=True, stop=True)
                oc_eng = nc.scalar if qc <= 1 else nc.vector
                (oc_eng.copy if qc <= 1 else nc.vector.tensor_copy)(
                    OT[hp:hp + 64, of + q0 + half * 256:
                       of + q0 + half * 256 + 256], tp[:])

            def osb_head():
                return opool.tile([128, 4 * HD], sdt, tag=f"osb{h % 2}",
                                  name="o_sb")

            th = [lambda j=j: pv_series(j) for j in range(4)]
            th.append(norm)
            th += [lambda half=half: trans(half) for half in range(2)]
            return th

        def wo_half(qt, np2, half, obs, pool=None, ptag="pw",
                    copy_eng="mix", split_dma=False):
            """One 512-wide n-chunk of out row-block qt; the final chunk
            fires one [128,2048] fp16 DMA for the whole row-block (fewer
            HWDGE round-trips than per-chunk DMAs). The epilogue splits
            per-np2 ([128,1024]) so the last transfer is shorter."""
            pool = pool or pw
            if (np2, half) == (0, 0):
                obs[qt] = osb.tile([128, 2048], f16, tag="ob", name="ob")
            ob = obs[qt]
            ncn = 2 * np2 + half
            pw_t = pool.tile([128, 512], f32, tag=ptag, name="pw_t")
            for mt in range(2):
                nc.tensor.matmul(
                    pw_t[:],
                    _mm_ap(OT[:, mt * S + qt * 128:
                              mt * S + (qt + 1) * 128], mmdt),
                    _mm_ap(wo_sb[:, mt * DIM + ncn * 512:
                                 mt * DIM + ncn * 512 + 512], mmdt),
                    start=(mt == 0), stop=(mt == 1))
            wo_copy_cnt[0] += 1
            use_act = {"mix": wo_copy_cnt[0] % 3 == 0,
                       "dve": False,
                       "alt": wo_copy_cnt[0] % 2 == 1}[copy_eng]
            if use_act:
                nc.scalar.copy(ob[:, ncn * 512:ncn * 512 + 512], pw_t[:])
            else:
                nc.vector.tensor_copy(
                    ob[:, ncn * 512:ncn * 512 + 512], pw_t[:])
            if split_dma and half == 1:
                nc.sync.dma_start(
                    out[qt * 128:(qt + 1) * 128,
                        np2 * 1024:np2 * 1024 + 1024],
                    ob[:, np2 * 1024:np2 * 1024 + 1024])
                if np2 == 1:
                    del obs[qt]
            elif not split_dma and (np2, half) == (1, 1):
                del obs[qt]
                nc.sync.dma_start(
                    out[qt * 128:(qt + 1) * 128, :], ob[:])
        wo_obs = {}

        # ------------------------------------- merged emission schedule
        def merge(primary, *others):
            """Emit primary thunks; proportionally interleave the others."""
            counters = [0.0] * len(others)
            n = max(1, len(primary))
            for beat in primary:
                for j, lst in enumerate(others):
                    counters[j] += len(lst) / n
                    while counters[j] >= 1.0 and lst:
                        lst.pop(0)()
                        counters[j] -= 1.0
                for th in beat:
                    th()
            for lst in others:
                while lst:
                    lst.pop(0)()

        for th in proj_thunks(0, fused=True):       # prologue
            th()

        # Head processing order: chunk 3's head 0 is pulled forward between
        # (2,2) and (2,3) so part of the causal-triangle-heavy chunk-3 exp
        # load runs while ACT still has slack, instead of piling up at the
        # end where exp rate-limits the whole pipeline.
        ITEMS = [(0, 0), (0, 1), (0, 2), (0, 3),
                 (1, 0), (1, 1), (1, 2), (1, 3),
                 (2, 0), (2, 1), (2, 2), (3, 0), (3, 1), (2, 3),
                 (3, 2), (3, 3)]
        # proj(sc) spread over items [a, b) — must drain before the first
        # (sc, *) item; WO(sc) over [a, b) — may start only after the item
        # containing trans(sc, 3), i.e. one past (sc, 3)'s position.
        PROJ_AT = {1: (0, 4), 2: (4, 8), 3: (8, 11)}
        WO_AT = {0: (5, 8), 1: (9, 14), 2: (15, 16)}
        proj_by_start = {a: s for s, (a, b) in PROJ_AT.items()}
        wo_by_start = {a: s for s, (a, b) in WO_AT.items()}

        prev = None                      # (qc, h, tiles) awaiting PV
        pstream, pend = [], 0
        wopending, woend = [], 0
        for i, (sc, h) in enumerate(ITEMS):
            if i == 4:
                nc.sync.dma_start(wo_sb[:, 0:DIM], wo_s[0:128, :])
                nc.sync.dma_start(wo_sb[:, DIM:2 * DIM], wo_s[128:256, :])
            if i in proj_by_start:
                pstream = proj_thunks(proj_by_start[i])
                pend = PROJ_AT[proj_by_start[i]][1]
            if i in wo_by_start:
                s = wo_by_start[i]
                wopending = [(qt, np2, half)
                             for qt in range(4 * s, 4 * s + 4)
                             for np2 in range(2)
                             for half in range(2)]
                woend = WO_AT[s][1]
            tiles = []
            sth = s_thunks(sc, h, tiles)
            pth = pv_thunks(*prev) if prev is not None else []
            beats = []
            for bi in range(max(len(sth), len(pth))):
                beat = []
                if bi < len(pth):
                    beat.append(pth[bi])
                if bi < len(sth):
                    beat.append(sth[bi])
                beats.append(beat)
            others = []
            if pstream:
                rem = max(1, pend - i)
                ptake = len(pstream) if rem <= 1 else len(pstream) // rem
                others.append(pstream[:ptake])
                pstream = pstream[ptake:]
            if wopending:
                rem = max(1, woend - i)
                wtake = (len(wopending) if rem <= 1
                         else len(wopending) // rem)
                # after proj(3) is done (items >= 11) pjo is mostly idle:
                # route WO psum there and keep its copies off saturated ACT
                wpool, wtag, weng = ((pjo, "pjo", "dve") if i >= 11
                                     else (None, "pw", "mix"))
                others.append(
                    [lambda qt=u[0], np2=u[1], half=u[2]:
                     wo_half(qt, np2, half, wo_obs, pool=wpool,
                             ptag=wtag, copy_eng=weng)
                     for u in wopending[:wtake]])
                wopending = wopending[wtake:]
            merge(beats, *others)
            prev = (sc, h, tiles)
        assert not pstream and not wopending

        # epilogue: PV of the last head, then WO of chunk 3; the score
        # pool's banks are free now, so WO rotates through those too.
        # trans(0) covers q-tiles 12-13, trans(1) covers 14-15: start the
        # WO units for each pair as soon as its OT columns are complete.
        pth = pv_thunks(*prev)
        for th in pth[:-1]:          # pairs + norm + trans(0)
            th()
        epi = [0]
        pools = [(pw, "pw"), (ps, "ps"), (pjo, "pjo")]

        def epi_wo(qt, np2):
            pool, ptag = pools[epi[0] % 3]
            for half in range(2):
                wo_half(qt, np2, half, wo_obs, pool=pool, ptag=ptag,
                        copy_eng="mix", split_dma=True)
            epi[0] += 1

        epi_wo(12, 0)
        pth[-1]()                    # trans(1) overlaps qt-12 copies
        epi_wo(12, 1)
        for qt in range(13, 16):
            for np2 in range(2):
                epi_wo(qt, np2)

    nc.compile()
    return nc


# ------------------------------------------------------------- host side
def _pair_perm64():
    """Column permutation putting the RoPE partner 16 partitions away."""
    return np.array([2 * (16 * (j // 32) + (j % 16)) + ((j % 32) // 16)
                     for j in range(64)])


def _host_prep(x, freqs_cos, freqs_sin, wq, wk, wv, wo):
    _, _, npdt = _dtypes()
    x = np.asarray(x, np.float32)
    fc = np.asarray(freqs_cos, np.float32)
    fs = np.asarray(freqs_sin, np.float32)
    wq = np.asarray(wq, np.float32)
    wk = np.asarray(wk, np.float32)
    wv = np.asarray(wv, np.float32)
    wo = np.asarray(wo, np.float32)

    perm = _pair_perm64()
    xT = np.ascontiguousarray(x[0].T).astype(npdt)

    p = np.arange(64)
    pair = 16 * ((p % 64) // 32) + (p % 16)
    sign = np.where((p % 32) < 16, -1.0, 1.0).astype(np.float32)
    cosE = np.ascontiguousarray(fc[:, pair].T)                  # [64, S]
    sinE = np.ascontiguousarray(fs[:, pair].T) * sign[:, None]  # [64, S]
    utri = np.triu(np.ones((128, 128), np.float32)).astype(npdt)

    in_maps = []
    for c in range(NCORES):
        qcols = np.concatenate(
            [wq[:, (4 * c + i) * 64 + perm] for i in range(HQ)], axis=1)
        kcols = wk[:, c * 64 + perm]
        vcols = wv[:, c * 64:(c + 1) * 64]
        wqkv_c = np.concatenate([qcols, kcols, vcols], axis=1).astype(npdt)
        wo_c = wo[QW * c:QW * (c + 1), :].astype(npdt)
        in_maps.append({
            "xT": xT,
            "wqkv": np.ascontiguousarray(wqkv_c),
            "wo_s": np.ascontiguousarray(wo_c),
            "cosE": cosE.astype(np.float32),
            "sinE": np.ascontiguousarray(sinE).astype(np.float32),
            "utri": np.ascontiguousarray(utri),
        })
    return in_maps


_NC_CACHE = {}


def get_program():
    if MM not in _NC_CACHE:
        _NC_CACHE[MM] = build_program()
    return _NC_CACHE[MM]


def kernel(x, freqs_cos, freqs_sin, wq, wk, wv, wo):
    nc = get_program()
    in_maps = _host_prep(x, freqs_cos, freqs_sin, wq, wk, wv, wo)
    res = run_bass_kernel_spmd(nc, in_maps, core_ids=list(range(NCORES)))
    acc = np.zeros((S, DIM), np.float64)
    for r in res.results:
        acc += r["out"].astype(np.float64)
    return acc.astype(np.float32).reshape(1, S, DIM)


# revision 70
# speedup vs baseline: 1.0468x; 1.0109x over previous
"""Trainium2 Bass kernel for a GQA attention layer (dense transformer).

Reference computation (B=1, S=2048, DIM=2048, 32 q-heads, 8 kv-heads, hd=64):
    xq = x @ wq; xk = x @ wk; xv = x @ wv
    rope(xq, xk); GQA causal attention; out = attn @ wo

Sharding: tensor-parallel over heads across 8 cores. Core c owns q-heads
4c..4c+3 (wq cols), kv-head c (wk/wv cols), and wo rows 256c..256c+255.
Each core computes a full [S, DIM] partial of the output projection; the
host sums the 8 partials (the TP all-reduce, done at gather time).

Kernel layout strategy (everything "transposed", head_dim on partitions):
  - QT/KT/VT = W.T @ x computed with lhsT = weight shard (natural [DIM, m]
    layout), rhs = x.T tiles streamed from DRAM.
  - RoPE pairs are de-interleaved by permuting wq/wk columns on the host so
    the rotation partner sits 16 partitions away (within a 32-partition
    quadrant, so DVE stream_shuffle can swap them).
  - Scores are computed transposed: S^T[k, q] = K^T.T @ Q^T per 128-row
    k-tile; exp on ACT (scale fused); causal mask = upper-tri multiply on
    the single diagonal 128x128 block of each k-tile (on GPSIMD).
  - P@V runs q-major: out[q,hd] = sum_k P^T[k,q].T @ Vp[k,hd|1] per
    (k-tile, 128-q subtile) with N=65 moving columns - 65 PE cycles per
    k-tile instead of 512, full 128x128 array use. The ones column makes
    psum col 64 the softmax denominator.
  - Normalize: one reciprocal per head ([128,4]), then tensor_scalar_mul
    psum->SBUF (per-partition scalar = per-q denom) into O_sb, then PE
    transposes [128q,64] -> [64,128q] (bf16 psum) and 2x-speed DVE copies
    into OT for the output projection.
  - Output projection from O^T with wo shard as rhs; partials DMA'd fp16.
"""

import numpy as np
import ml_dtypes

import concourse.bass as bass
import concourse.mybir as mybir
from concourse import bacc
from concourse.tile import TileContext
from concourse.masks import make_identity
from concourse.bass_utils import run_bass_kernel_spmd

# ---------------------------------------------------------------- constants
S = 2048          # sequence length
DIM = 2048        # model dim
NH = 32           # query heads
NKV = 8           # kv heads
HD = 64           # head dim
NCORES = 8
HQ = NH // NCORES          # query heads per core = 4
QW = HQ * HD               # q width per core = 256
KT_S = S // 128            # 16 seq k-tiles
KT_D = DIM // 128          # 16 dim k-tiles
NSC = S // 512             # 4 s-chunks
SCALE = 1.0 / 8.0          # 1/sqrt(64)
HD1 = HD + 1               # V tile width incl ones column

# matmul dtype knob: 'bf16' | 'f32' | 'f32r'
MM = 'bf16'
MASK_POOL = True           # causal masks on GPSIMD (else DVE)
NWARM = 12                 # p-state warmup transposes
ACT_COPY_EVERY = 4         # every Nth WO psum->sbuf copy goes to ACT

_SHUF_SWAP16 = list(range(16, 32)) + list(range(16))


def _dtypes():
    if MM == 'bf16':
        return mybir.dt.bfloat16, mybir.dt.bfloat16, ml_dtypes.bfloat16
    if MM == 'f32':
        return mybir.dt.float32, mybir.dt.float32, np.float32
    if MM == 'f32r':
        return mybir.dt.float32, mybir.dt.float32r, np.float32
    raise ValueError(MM)


def _mm_ap(ap, mmdt):
    """View an AP in the matmul dtype (bitcast f32 -> f32r when needed)."""
    if ap.dtype != mmdt:
        return ap.bitcast(mmdt)
    return ap


def build_program():
    """Build the per-core Bass program (same program on all 8 cores).

    Emission is a fine-grained software pipeline: attention beats for chunk
    sc (S^T mega-matmul for head h + PV beats of head h-1) are merged with
    the projection matmuls of chunk sc+1 and the WO units of chunk sc-1.

    PSUM (8 banks): pjo 3 (projection passes + PV accumulators, shared tag)
    + ps 4 (two [128,1024] score megas) + pw 1 (WO + transposes).
    """
    sdt, mmdt, _ = _dtypes()
    f32 = mybir.dt.float32
    f16 = mybir.dt.float16

    nc = bacc.Bacc("TRN2", target_bir_lowering=False, debug=False,
                   num_devices=NCORES)

    xT = nc.dram_tensor("xT", [DIM, S], sdt, kind="ExternalInput")
    wqkv = nc.dram_tensor("wqkv", [DIM, QW + 2 * HD], sdt,
                          kind="ExternalInput")
    wo_s = nc.dram_tensor("wo_s", [QW, DIM], sdt, kind="ExternalInput")
    cosE = nc.dram_tensor("cosE", [64, S], f32, kind="ExternalInput")
    sinE = nc.dram_tensor("sinE", [64, S], f32, kind="ExternalInput")
    utri = nc.dram_tensor("utri", [128, 128], sdt, kind="ExternalInput")
    out = nc.dram_tensor("out", [S, DIM], f16, kind="ExternalOutput")

    WQKV = QW + 2 * HD  # 384

    import contextlib
    with TileContext(nc) as tc, contextlib.ExitStack() as ctx:
        const = ctx.enter_context(tc.tile_pool(name="const", bufs=1))
        work = ctx.enter_context(tc.tile_pool(name="work", bufs=2))
        xtp = ctx.enter_context(tc.tile_pool(name="xtp", bufs=10))
        ptp = ctx.enter_context(tc.tile_pool(name="ptp", bufs=20))
        small = ctx.enter_context(tc.tile_pool(name="small", bufs=5))
        osb = ctx.enter_context(tc.tile_pool(name="osb", bufs=4))
        opool = ctx.enter_context(tc.tile_pool(name="opool", bufs=2))

        pjo = ctx.enter_context(tc.tile_pool(name="pjo", bufs=3,
                                             space="PSUM"))
        ps = ctx.enter_context(tc.tile_pool(name="ps", bufs=2, space="PSUM"))
        pw = ctx.enter_context(tc.tile_pool(name="pw", bufs=1, space="PSUM"))

        # ----------------------------------------------- persistent SBUF
        w_sb = const.tile([128, KT_D * WQKV], sdt, tag="w_sb")
        wo_sb = const.tile([128, 2 * DIM], sdt, tag="wo_sb")
        cos_sb = const.tile([128, S], f32, tag="cos_sb")
        sin_sb = const.tile([128, S], f32, tag="sin_sb")
        utri_sb = const.tile([128, 128], sdt, tag="utri_sb")
        ident = const.tile([128, 128], sdt, tag="ident")
        QT = const.tile([64, HQ * S], sdt, tag="QT")
        KVt = const.tile([128, S], sdt, tag="KVt")
        Vp = const.tile([128, KT_S * HD1], sdt, tag="Vp")
        OT = const.tile([128, 2 * S], sdt, tag="OT")

        # p-state warmup: keep PE streaming during the initial DMA wait so
        # the frequency ramp (3us) is over before real matmuls start; the
        # source tile only needs to hold valid bits, so a fast DVE memset
        # unblocks the first transpose ~200ns in
        wsrc = const.tile([128, 128], sdt, tag="wsrc")
        nc.vector.memset(wsrc[:], 0.0)
        warm = pw.tile([128, 128], sdt, tag="pw", name="warm")
        for _ in range(NWARM):
            nc.tensor.transpose(warm[:], wsrc[:], wsrc[:])

        make_identity(nc, ident[:])
        nc.gpsimd.memset(Vp[:], 1.0)  # ones columns for denominator

        wo_copy_cnt = [0]
        xpf = {}  # (sc, batch) -> prefetched x tile

        def prefetch_x(psc, bi, b0, bn):
            xt4 = xtp.tile([128, 4 * 512], sdt, tag="xt", name="xt4")
            nc.sync.dma_start(
                xt4[:, 0:bn * 512].rearrange("r (k c) -> r k c", k=bn),
                xT[b0 * 128:(b0 + bn) * 128,
                   psc * 512:psc * 512 + 512].rearrange(
                       "(k r) c -> r k c", k=bn))
            xpf[(psc, bi)] = xt4

        # ---------------------------------------------- thunk generators
        def proj_thunks(sc, fused=False):
            """Projection of chunk sc: KV pass, K-rope, V transposes, then
            Q passes (one PSUM slot each, sequential). With fused=True
            (prologue) all three matmuls run per k-tile, using 3 slots."""
            s0 = sc * 512
            xts = []
            st = {}

            # small first batches so the first matmul starts early; bigger
            # later ones amortize HWDGE overhead
            batches = [2, 2, 4, 4, 4] if sc == 0 else [4, 4, 4, 4]
            starts = [sum(batches[:i]) for i in range(len(batches))]
            kt_slot = {}
            for bi, (b0, bn) in enumerate(zip(starts, batches)):
                for j in range(bn):
                    kt_slot[b0 + j] = (bi, j, bn, b0)

            def dma_kv(kt):
                bi, j, bn, b0 = kt_slot[kt]
                if j == 0:
                    if sc == 0:
                        nc.sync.dma_start(
                            w_sb[:, b0 * WQKV:(b0 + bn) * WQKV].rearrange(
                                "r (k w) -> r k w", k=bn),
                            wqkv[b0 * 128:(b0 + bn) * 128, :].rearrange(
                                "(k r) w -> r k w", k=bn))
                    if (sc, bi) in xpf:
                        xt4 = xpf.pop((sc, bi))
                    else:
                        xt4 = xtp.tile([128, 4 * 512], sdt, tag="xt",
                                       name="xt4")
                        nc.sync.dma_start(
                            xt4[:, 0:bn * 512].rearrange(
                                "r (k c) -> r k c", k=bn),
                            xT[b0 * 128:(b0 + bn) * 128,
                               s0:s0 + 512].rearrange(
                                   "(k r) c -> r k c", k=bn))
                    xts.append(xt4)
                    if sc == 0 and kt == 12:
                        # chunk 1's first x batch jumps the DMA queue ahead
                        # of the trig loads (k_rope only needs those at
                        # ~15us, while window 1 is starved for x); rows
                        # [64:128] of cos/sin are engine-copied from [0:64]
                        prefetch_x(1, 0, 0, 4)
                        nc.sync.dma_start(cos_sb[0:64, :], cosE[:])
                        nc.sync.dma_start(sin_sb[0:64, :], sinE[:])
                        nc.sync.dma_start(utri_sb[:], utri[:])
                if sc == 0 and kt == 15:
                    prefetch_x(1, 1, 4, 4)
                bi, j, bn, b0 = kt_slot[kt]
                xt = xts[bi][:, j * 512:j * 512 + 512]
                if kt == 0:
                    st["pkv"] = pjo.tile([128, 512], f32, tag="pjo",
                                         name="pkv")
                    if fused:
                        st["fq0"] = pjo.tile([128, 512], f32, tag="pjo",
                                             name="fq0")
                        st["fq1"] = pjo.tile([128, 512], f32, tag="pjo",
                                             name="fq1")
                nc.tensor.matmul(
                    st["pkv"][:],
                    _mm_ap(w_sb[:, kt * WQKV + 256:kt * WQKV + 384], mmdt),
                    _mm_ap(xt, mmdt),
                    start=(kt == 0), stop=(kt == KT_D - 1))
                if fused:
                    for mt in range(2):
                        nc.tensor.matmul(
                            st[f"fq{mt}"][:],
                            _mm_ap(w_sb[:, kt * WQKV + mt * 128:
                                        kt * WQKV + mt * 128 + 128], mmdt),
                            _mm_ap(xt, mmdt),
                            start=(kt == 0), stop=(kt == KT_D - 1))

            def k_rope():
                pkv = st["pkv"]
                shufk = work.tile([64, 512], f32, tag="shufk", name="shufk")
                m1k = work.tile([64, 512], f32, tag="m1k", name="m1k")
                t2k = work.tile([64, 512], f32, tag="t2k", name="t2k")
                nc.vector.stream_shuffle(shufk[:], pkv[0:64, :],
                                         _SHUF_SWAP16)
                nc.vector.tensor_mul(m1k[:], pkv[0:64, :],
                                     cos_sb[0:64, s0:s0 + 512])
                nc.vector.tensor_mul(t2k[:], shufk[:],
                                     sin_sb[0:64, s0:s0 + 512])
                nc.vector.tensor_add(KVt[0:64, s0:s0 + 512], m1k[:], t2k[:])
                nc.scalar.copy(KVt[64:128, s0:s0 + 512],
                               pkv[64:128, :])

            def v_trans(kt):
                pv = pw.tile([128, HD], sdt, tag="pw", name="pv")
                nc.tensor.transpose(
                    pv[:], KVt[64:128, kt * 128:(kt + 1) * 128],
                    ident[64:128, 64:128])
                nc.vector.tensor_copy(
                    Vp[:, kt * HD1:kt * HD1 + HD], pv[:])

            def q_mm(mt, kt):
                if kt == 0:
                    st["pq"] = pjo.tile([128, 512], f32, tag="pjo",
                                        name="pq")
                w0 = kt * WQKV + mt * 128
                bi, j, bn, b0 = kt_slot[kt]
                xt = xts[bi][:, j * 512:j * 512 + 512]
                nc.tensor.matmul(
                    st["pq"][:], _mm_ap(w_sb[:, w0:w0 + 128], mmdt),
                    _mm_ap(xt, mmdt),
                    start=(kt == 0), stop=(kt == KT_D - 1))

            def q_rope(mt):
                pq = st[f"fq{mt}"] if fused else st["pq"]
                shuf = work.tile([128, 512], f32, tag="shuf", name="shuf")
                m1 = work.tile([128, 512], f32, tag="m1", name="m1")
                t2 = work.tile([128, 512], f32, tag="t2", name="t2")
                nc.vector.stream_shuffle(shuf[:], pq[:], _SHUF_SWAP16)
                nc.vector.tensor_mul(m1[:], pq[:], cos_sb[:, s0:s0 + 512])
                nc.vector.tensor_mul(t2[:], shuf[:], sin_sb[:, s0:s0 + 512])
                he = (2 * mt) * S
                ho = (2 * mt + 1) * S
                nc.vector.tensor_add(
                    QT[:, he + s0:he + s0 + 512], m1[0:64, :], t2[0:64, :])
                nc.vector.tensor_add(
                    QT[:, ho + s0:ho + s0 + 512], m1[64:128, :],
                    t2[64:128, :])

            def cs_dup():
                # duplicate cos/sin rows [0:64] into [64:128] for q_rope
                # (cos on idle ACT, sin on idle GPSIMD, in parallel)
                nc.scalar.copy(cos_sb[64:128, :], cos_sb[0:64, :])
                nc.gpsimd.tensor_copy(sin_sb[64:128, :], sin_sb[0:64, :])

            th = [lambda kt=kt: dma_kv(kt) for kt in range(KT_D)]
            th.append(k_rope)
            if sc == 0:
                th.append(cs_dup)
            th += [lambda kt=kt: v_trans(kt)
                   for kt in range(4 * sc, 4 * sc + 4)]
            if fused:
                th += [lambda mt=mt: q_rope(mt) for mt in range(2)]
            else:
                for mt in range(2):
                    th += [lambda mt=mt, kt=kt: q_mm(mt, kt)
                           for kt in range(KT_D)]
                    th.append(lambda mt=mt: q_rope(mt))
            return th

        def s_thunks(qc, h, tiles):
            """S^T mega matmuls + exp + mask for one head; fills `tiles`."""
            q0 = qc * 512
            hf = h * S
            nkt = 4 * qc + 4
            thunks = []
            for pi in range(nkt // 2):
                def th(pi=pi):
                    kts = (2 * pi, 2 * pi + 1)
                    ps_t = ps.tile([128, 1024], f32, tag="ps", name="ps_t")
                    pt_t = ptp.tile([128, 1024], sdt, tag="pt", name="pt_t")
                    for li, kt in enumerate(kts):
                        dj = kt - 4 * qc
                        qo = 128 * dj if dj >= 0 else 0
                        lo = li * 512
                        nc.tensor.matmul(
                            ps_t[:, lo + qo:lo + 512],
                            _mm_ap(KVt[0:64, kt * 128:(kt + 1) * 128], mmdt),
                            _mm_ap(QT[:, hf + q0 + qo:hf + q0 + 512], mmdt),
                            start=True, stop=True)
                    if 2 * pi + 1 < 4 * qc or (qc >= 1
                                               and 2 * pi == 4 * qc):
                        # first diag pair (dj 0,1) of chunks >= 1: exp the
                        # full mega in one op; the extra columns hold stale
                        # scores (bounded, and never read by the series-
                        # major PV), and one wide op beats two narrow ones
                        # on the exp-saturated ACT engine
                        nc.scalar.activation(
                            pt_t[:], ps_t[:],
                            mybir.ActivationFunctionType.Exp, scale=SCALE)
                    else:
                        for li, kt in enumerate(kts):
                            dj = kt - 4 * qc
                            qo = 128 * dj if dj >= 0 else 0
                            lo = li * 512
                            nc.scalar.activation(
                                pt_t[:, lo + qo:lo + 512],
                                ps_t[:, lo + qo:lo + 512],
                                mybir.ActivationFunctionType.Exp,
                                scale=SCALE)
                    for li, kt in enumerate(kts):
                        dj = kt - 4 * qc
                        qo = 128 * dj if dj >= 0 else 0
                        lo = li * 512
                        if dj >= 0:
                            eng = nc.gpsimd if MASK_POOL else nc.vector
                            eng.tensor_mul(
                                pt_t[:, lo + qo:lo + qo + 128],
                                pt_t[:, lo + qo:lo + qo + 128], utri_sb[:])
                        tiles.append((kt, qo, lo, pt_t))
                thunks.append(th)
            return thunks

        def pv_thunks(qc, h, tiles):
            """q-major PV accumulation + normalization + O transposes.

            For each incoming P^T tile (k-tile kt), run the N=65 matmuls for
            each live 128-q subtile j: out[q,hd|den] += P^T[:,j*128:].T@Vp.
            After the last k-tile: reciprocal of the denominators, normalize
            psum->O_sb (bf16), PE-transpose to O^T, 2x DVE copy into OT.
            """
            q0 = qc * 512
            hp = (h % 2) * 64
            of = (h // 2) * S
            nkt0 = 4 * qc + 4
            state = {}

            def pv_series(j):
                """All matmuls of q-subtile j back-to-back: start=True
                clears the whole bank's has_written bits, so accumulation
                series sharing a psum bank must not interleave."""
                if "pv" not in state:
                    state["pv"] = pjo.tile([128, 512], f32, tag="pjo",
                                           name="pvt")
                pv_t = state["pv"]
                for kt in range(0, 4 * qc + j + 1):
                    _, qo, lo, pt_t = tiles[kt]
                    nc.tensor.matmul(
                        pv_t[:, j * 128:j * 128 + HD1],
                        _mm_ap(pt_t[:, lo + j * 128:
                                    lo + (j + 1) * 128], mmdt),
                        _mm_ap(Vp[:, kt * HD1:(kt + 1) * HD1], mmdt),
                        start=(kt == 0), stop=(kt == 4 * qc + j))

            def norm():
                pv_t = state["pv"]
                rc = small.tile([128, 4], f32, tag="rc", name="rc")
                dsb = small.tile([128, 4], f32, tag="dsb", name="dsb")
                o_sb = state["o_sb"] = osb_head()
                pvr = pv_t.rearrange("p (b c) -> p b c", c=128)
                nc.vector.tensor_copy(
                    dsb[:].rearrange("p (b c) -> p b c", c=1),
                    pvr[:, :, HD:HD + 1])
                nc.vector.reciprocal(rc[:], dsb[:])
                for j in range(4):
                    nc.vector.tensor_scalar_mul(
                        o_sb[:, j * HD:(j + 1) * HD],
                        pv_t[:, j * 128:j * 128 + HD],
                        rc[:, j:j + 1])

            def trans(half):
                # transpose via a regular matmul against the identity:
                # out[hd, q] = o_sb[q, hd]^T @ I -- f32 psum output, so
                # nonzero free offsets behave like the score megas
                o_sb = state["o_sb"]
                tp = pw.tile([64, 256], f32, tag="pw", name="tp")
                for st2 in range(2):
                    stg = 2 * half + st2
                    nc.tensor.matmul(
                        tp[:, st2 * 128:(st2 + 1) * 128],
                        _mm_ap(o_sb[:, stg * HD:(stg + 1) * HD], mmdt),
                        _mm_ap(ident[:], mmdt),
                        start=True, stop=True)
                oc_eng = nc.scalar if qc <= 1 else nc.vector
                (oc_eng.copy if qc <= 1 else nc.vector.tensor_copy)(
                    OT[hp:hp + 64, of + q0 + half * 256:
                       of + q0 + half * 256 + 256], tp[:])

            def osb_head():
                return opool.tile([128, 4 * HD], sdt, tag=f"osb{h % 2}",
                                  name="o_sb")

            th = [lambda j=j: pv_series(j) for j in range(4)]
            th.append(norm)
            th += [lambda half=half: trans(half) for half in range(2)]
            return th

        def wo_half(qt, np2, half, obs, pool=None, ptag="pw",
                    copy_eng="mix", split_dma=False):
            """One 512-wide n-chunk of out row-block qt; the final chunk
            fires one [128,2048] fp16 DMA for the whole row-block (fewer
            HWDGE round-trips than per-chunk DMAs). The epilogue splits
            per-np2 ([128,1024]) so the last transfer is shorter."""
            pool = pool or pw
            if (np2, half) == (0, 0):
                obs[qt] = osb.tile([128, 2048], f16, tag="ob", name="ob")
            ob = obs[qt]
            ncn = 2 * np2 + half
            pw_t = pool.tile([128, 512], f32, tag=ptag, name="pw_t")
            for mt in range(2):
                nc.tensor.matmul(
                    pw_t[:],
                    _mm_ap(OT[:, mt * S + qt * 128:
                              mt * S + (qt + 1) * 128], mmdt),
                    _mm_ap(wo_sb[:, mt * DIM + ncn * 512:
                                 mt * DIM + ncn * 512 + 512], mmdt),
                    start=(mt == 0), stop=(mt == 1))
            wo_copy_cnt[0] += 1
            use_act = {"mix": wo_copy_cnt[0] % 3 == 0,
                       "dve": False,
                       "alt": wo_copy_cnt[0] % 2 == 1}[copy_eng]
            if use_act:
                nc.scalar.copy(ob[:, ncn * 512:ncn * 512 + 512], pw_t[:])
            else:
                nc.vector.tensor_copy(
                    ob[:, ncn * 512:ncn * 512 + 512], pw_t[:])
            if split_dma and half == 1:
                nc.sync.dma_start(
                    out[qt * 128:(qt + 1) * 128,
                        np2 * 1024:np2 * 1024 + 1024],
                    ob[:, np2 * 1024:np2 * 1024 + 1024])
                if np2 == 1:
                    del obs[qt]
            elif not split_dma and (np2, half) == (1, 1):
                del obs[qt]
                nc.sync.dma_start(
                    out[qt * 128:(qt + 1) * 128, :], ob[:])
        wo_obs = {}

        # ------------------------------------- merged emission schedule
        def merge(primary, *others):
            """Emit primary thunks; proportionally interleave the others."""
            counters = [0.0] * len(others)
            n = max(1, len(primary))
            for beat in primary:
                for j, lst in enumerate(others):
                    counters[j] += len(lst) / n
                    while counters[j] >= 1.0 and lst:
                        lst.pop(0)()
                        counters[j] -= 1.0
                for th in beat:
                    th()
            for lst in others:
                while lst:
                    lst.pop(0)()

        for th in proj_thunks(0, fused=True):       # prologue
            th()

        # Head processing order: chunk 3's head 0 is pulled forward between
        # (2,2) and (2,3) so part of the causal-triangle-heavy chunk-3 exp
        # load runs while ACT still has slack, instead of piling up at the
        # end where exp rate-limits the whole pipeline.
        ITEMS = [(0, 0), (0, 1), (0, 2), (0, 3),
                 (1, 0), (1, 1), (1, 2), (1, 3),
                 (2, 0), (2, 1), (2, 2), (3, 0), (3, 1), (2, 3),
                 (3, 2), (3, 3)]
        # proj(sc) spread over items [a, b) — must drain before the first
        # (sc, *) item; WO(sc) over [a, b) — may start only after the item
        # containing trans(sc, 3), i.e. one past (sc, 3)'s position.
        PROJ_AT = {1: (0, 4), 2: (4, 8), 3: (8, 11)}
        WO_AT = {0: (6, 8), 1: (12, 14), 2: (15, 16)}
        proj_by_start = {a: s for s, (a, b) in PROJ_AT.items()}
        wo_by_start = {a: s for s, (a, b) in WO_AT.items()}

        prev = None                      # (qc, h, tiles) awaiting PV
        pstream, pend = [], 0
        wopending, woend = [], 0
        for i, (sc, h) in enumerate(ITEMS):
            if i == 4:
                nc.sync.dma_start(wo_sb[:, 0:DIM], wo_s[0:128, :])
                nc.sync.dma_start(wo_sb[:, DIM:2 * DIM], wo_s[128:256, :])
            if i in proj_by_start:
                pstream = proj_thunks(proj_by_start[i])
                pend = PROJ_AT[proj_by_start[i]][1]
            if i in wo_by_start:
                s = wo_by_start[i]
                wopending = [(qt, np2, half)
                             for qt in range(4 * s, 4 * s + 4)
                             for np2 in range(2)
                             for half in range(2)]
                woend = WO_AT[s][1]
            tiles = []
            sth = s_thunks(sc, h, tiles)
            pth = pv_thunks(*prev) if prev is not None else []
            beats = []
            for bi in range(max(len(sth), len(pth))):
                beat = []
                if bi < len(pth):
                    beat.append(pth[bi])
                if bi < len(sth):
                    beat.append(sth[bi])
                beats.append(beat)
            others = []
            if pstream:
                rem = max(1, pend - i)
                ptake = len(pstream) if rem <= 1 else len(pstream) // rem
                others.append(pstream[:ptake])
                pstream = pstream[ptake:]
            if wopending:
                rem = max(1, woend - i)
                wtake = (len(wopending) if rem <= 1
                         else len(wopending) // rem)
                # after proj(3) is done (items >= 11) pjo is mostly idle:
                # route WO psum there and keep its copies off saturated ACT
                wpool, wtag, weng = ((pjo, "pjo", "dve") if i >= 11
                                     else (None, "pw", "mix"))
                others.append(
                    [lambda qt=u[0], np2=u[1], half=u[2]:
                     wo_half(qt, np2, half, wo_obs, pool=wpool,
                             ptag=wtag, copy_eng=weng)
                     for u in wopending[:wtake]])
                wopending = wopending[wtake:]
            merge(beats, *others)
            prev = (sc, h, tiles)
        assert not pstream and not wopending

        # epilogue: PV of the last head, then WO of chunk 3; the score
        # pool's banks are free now, so WO rotates through those too.
        # trans(0) covers q-tiles 12-13, trans(1) covers 14-15: start the
        # WO units for each pair as soon as its OT columns are complete.
        pth = pv_thunks(*prev)
        for th in pth[:-1]:          # pairs + norm + trans(0)
            th()
        epi = [0]
        pools = [(pw, "pw"), (ps, "ps"), (pjo, "pjo")]

        def epi_wo(qt, np2):
            pool, ptag = pools[epi[0] % 3]
            for half in range(2):
                wo_half(qt, np2, half, wo_obs, pool=pool, ptag=ptag,
                        copy_eng="mix", split_dma=True)
            epi[0] += 1

        epi_wo(12, 0)
        pth[-1]()                    # trans(1) overlaps qt-12 copies
        epi_wo(12, 1)
        for qt in range(13, 16):
            for np2 in range(2):
                epi_wo(qt, np2)

    nc.compile()
    return nc


# ------------------------------------------------------------- host side
def _pair_perm64():
    """Column permutation putting the RoPE partner 16 partitions away."""
    return np.array([2 * (16 * (j // 32) + (j % 16)) + ((j % 32) // 16)
                     for j in range(64)])


def _host_prep(x, freqs_cos, freqs_sin, wq, wk, wv, wo):
    _, _, npdt = _dtypes()
    x = np.asarray(x, np.float32)
    fc = np.asarray(freqs_cos, np.float32)
    fs = np.asarray(freqs_sin, np.float32)
    wq = np.asarray(wq, np.float32)
    wk = np.asarray(wk, np.float32)
    wv = np.asarray(wv, np.float32)
    wo = np.asarray(wo, np.float32)

    perm = _pair_perm64()
    xT = np.ascontiguousarray(x[0].T).astype(npdt)

    p = np.arange(64)
    pair = 16 * ((p % 64) // 32) + (p % 16)
    sign = np.where((p % 32) < 16, -1.0, 1.0).astype(np.float32)
    cosE = np.ascontiguousarray(fc[:, pair].T)                  # [64, S]
    sinE = np.ascontiguousarray(fs[:, pair].T) * sign[:, None]  # [64, S]
    utri = np.triu(np.ones((128, 128), np.float32)).astype(npdt)

    in_maps = []
    for c in range(NCORES):
        qcols = np.concatenate(
            [wq[:, (4 * c + i) * 64 + perm] for i in range(HQ)], axis=1)
        kcols = wk[:, c * 64 + perm]
        vcols = wv[:, c * 64:(c + 1) * 64]
        wqkv_c = np.concatenate([qcols, kcols, vcols], axis=1).astype(npdt)
        wo_c = wo[QW * c:QW * (c + 1), :].astype(npdt)
        in_maps.append({
            "xT": xT,
            "wqkv": np.ascontiguousarray(wqkv_c),
            "wo_s": np.ascontiguousarray(wo_c),
            "cosE": cosE.astype(np.float32),
            "sinE": np.ascontiguousarray(sinE).astype(np.float32),
            "utri": np.ascontiguousarray(utri),
        })
    return in_maps


_NC_CACHE = {}


def get_program():
    if MM not in _NC_CACHE:
        _NC_CACHE[MM] = build_program()
    return _NC_CACHE[MM]


def kernel(x, freqs_cos, freqs_sin, wq, wk, wv, wo):
    nc = get_program()
    in_maps = _host_prep(x, freqs_cos, freqs_sin, wq, wk, wv, wo)
    res = run_bass_kernel_spmd(nc, in_maps, core_ids=list(range(NCORES)))
    acc = np.zeros((S, DIM), np.float64)
    for r in res.results:
        acc += r["out"].astype(np.float64)
    return acc.astype(np.float32).reshape(1, S, DIM)


# revision 82
# speedup vs baseline: 1.0502x; 1.0033x over previous
"""Trainium2 Bass kernel for a GQA attention layer (dense transformer).

Reference computation (B=1, S=2048, DIM=2048, 32 q-heads, 8 kv-heads, hd=64):
    xq = x @ wq; xk = x @ wk; xv = x @ wv
    rope(xq, xk); GQA causal attention; out = attn @ wo

Sharding: tensor-parallel over heads across 8 cores. Core c owns q-heads
4c..4c+3 (wq cols), kv-head c (wk/wv cols), and wo rows 256c..256c+255.
Each core computes a full [S, DIM] partial of the output projection; the
host sums the 8 partials (the TP all-reduce, done at gather time).

Kernel layout strategy (everything "transposed", head_dim on partitions):
  - QT/KT/VT = W.T @ x computed with lhsT = weight shard (natural [DIM, m]
    layout), rhs = x.T tiles streamed from DRAM.
  - RoPE pairs are de-interleaved by permuting wq/wk columns on the host so
    the rotation partner sits 16 partitions away (within a 32-partition
    quadrant, so DVE stream_shuffle can swap them).
  - Scores are computed transposed: S^T[k, q] = K^T.T @ Q^T per 128-row
    k-tile; exp on ACT (scale fused); causal mask = upper-tri multiply on
    the single diagonal 128x128 block of each k-tile (on GPSIMD).
  - P@V runs q-major: out[q,hd] = sum_k P^T[k,q].T @ Vp[k,hd|1] per
    (k-tile, 128-q subtile) with N=65 moving columns - 65 PE cycles per
    k-tile instead of 512, full 128x128 array use. The ones column makes
    psum col 64 the softmax denominator.
  - Normalize: one reciprocal per head ([128,4]), then tensor_scalar_mul
    psum->SBUF (per-partition scalar = per-q denom) into O_sb, then PE
    transposes [128q,64] -> [64,128q] (bf16 psum) and 2x-speed DVE copies
    into OT for the output projection.
  - Output projection from O^T with wo shard as rhs; partials DMA'd fp16.
"""

import numpy as np
import ml_dtypes

import concourse.bass as bass
import concourse.mybir as mybir
from concourse import bacc
from concourse.tile import TileContext
from concourse.masks import make_identity
from concourse.bass_utils import run_bass_kernel_spmd

# ---------------------------------------------------------------- constants
S = 2048          # sequence length
DIM = 2048        # model dim
NH = 32           # query heads
NKV = 8           # kv heads
HD = 64           # head dim
NCORES = 8
HQ = NH // NCORES          # query heads per core = 4
QW = HQ * HD               # q width per core = 256
KT_S = S // 128            # 16 seq k-tiles
KT_D = DIM // 128          # 16 dim k-tiles
NSC = S // 512             # 4 s-chunks
SCALE = 1.0 / 8.0          # 1/sqrt(64)
HD1 = HD + 1               # V tile width incl ones column

# matmul dtype knob: 'bf16' | 'f32' | 'f32r'
MM = 'bf16'
MASK_POOL = True           # causal masks on GPSIMD (else DVE)
NWARM = 12                 # p-state warmup transposes
ACT_COPY_EVERY = 4         # every Nth WO psum->sbuf copy goes to ACT

_SHUF_SWAP16 = list(range(16, 32)) + list(range(16))


def _dtypes():
    if MM == 'bf16':
        return mybir.dt.bfloat16, mybir.dt.bfloat16, ml_dtypes.bfloat16
    if MM == 'f32':
        return mybir.dt.float32, mybir.dt.float32, np.float32
    if MM == 'f32r':
        return mybir.dt.float32, mybir.dt.float32r, np.float32
    raise ValueError(MM)


def _mm_ap(ap, mmdt):
    """View an AP in the matmul dtype (bitcast f32 -> f32r when needed)."""
    if ap.dtype != mmdt:
        return ap.bitcast(mmdt)
    return ap


def build_program():
    """Build the per-core Bass program (same program on all 8 cores).

    Emission is a fine-grained software pipeline: attention beats for chunk
    sc (S^T mega-matmul for head h + PV beats of head h-1) are merged with
    the projection matmuls of chunk sc+1 and the WO units of chunk sc-1.

    PSUM (8 banks): pjo 3 (projection passes + PV accumulators, shared tag)
    + ps 4 (two [128,1024] score megas) + pw 1 (WO + transposes).
    """
    sdt, mmdt, _ = _dtypes()
    f32 = mybir.dt.float32
    f16 = mybir.dt.float16

    nc = bacc.Bacc("TRN2", target_bir_lowering=False, debug=False,
                   num_devices=NCORES)

    xT = nc.dram_tensor("xT", [DIM, S], sdt, kind="ExternalInput")
    wqkv = nc.dram_tensor("wqkv", [DIM, QW + 2 * HD], sdt,
                          kind="ExternalInput")
    wo_s = nc.dram_tensor("wo_s", [QW, DIM], sdt, kind="ExternalInput")
    cosE = nc.dram_tensor("cosE", [64, S], f32, kind="ExternalInput")
    sinE = nc.dram_tensor("sinE", [64, S], f32, kind="ExternalInput")
    utri = nc.dram_tensor("utri", [128, 128], sdt, kind="ExternalInput")
    out = nc.dram_tensor("out", [S, DIM], f16, kind="ExternalOutput")

    WQKV = QW + 2 * HD  # 384

    import contextlib
    with TileContext(nc) as tc, contextlib.ExitStack() as ctx:
        const = ctx.enter_context(tc.tile_pool(name="const", bufs=1))
        work = ctx.enter_context(tc.tile_pool(name="work", bufs=2))
        xtp = ctx.enter_context(tc.tile_pool(name="xtp", bufs=10))
        ptp = ctx.enter_context(tc.tile_pool(name="ptp", bufs=20))
        small = ctx.enter_context(tc.tile_pool(name="small", bufs=5))
        osb = ctx.enter_context(tc.tile_pool(name="osb", bufs=4))
        opool = ctx.enter_context(tc.tile_pool(name="opool", bufs=2))

        pjo = ctx.enter_context(tc.tile_pool(name="pjo", bufs=3,
                                             space="PSUM"))
        ps = ctx.enter_context(tc.tile_pool(name="ps", bufs=2, space="PSUM"))
        pw = ctx.enter_context(tc.tile_pool(name="pw", bufs=1, space="PSUM"))

        # ----------------------------------------------- persistent SBUF
        w_sb = const.tile([128, KT_D * WQKV], sdt, tag="w_sb")
        wo_sb = const.tile([128, 2 * DIM], sdt, tag="wo_sb")
        cos_sb = const.tile([128, S], f32, tag="cos_sb")
        sin_sb = const.tile([128, S], f32, tag="sin_sb")
        utri_sb = const.tile([128, 128], sdt, tag="utri_sb")
        ident = const.tile([128, 128], sdt, tag="ident")
        QT = const.tile([64, HQ * S], sdt, tag="QT")
        KVt = const.tile([128, S], sdt, tag="KVt")
        Vp = const.tile([128, KT_S * HD1], sdt, tag="Vp")
        OT = const.tile([128, 2 * S], sdt, tag="OT")

        # p-state warmup: keep PE streaming during the initial DMA wait so
        # the frequency ramp (3us) is over before real matmuls start; the
        # source tile only needs to hold valid bits, so a fast DVE memset
        # unblocks the first transpose ~200ns in
        wsrc = const.tile([128, 128], sdt, tag="wsrc")
        nc.vector.memset(wsrc[:], 0.0)
        warm = pw.tile([128, 128], sdt, tag="pw", name="warm")
        for _ in range(NWARM):
            nc.tensor.transpose(warm[:], wsrc[:], wsrc[:])

        make_identity(nc, ident[:])
        nc.gpsimd.memset(Vp[:], 1.0)  # ones columns for denominator

        wo_copy_cnt = [0]
        xpf = {}  # (sc, batch) -> prefetched x tile

        def prefetch_x(psc, bi, b0, bn):
            xt4 = xtp.tile([128, 4 * 512], sdt, tag="xt", name="xt4")
            nc.sync.dma_start(
                xt4[:, 0:bn * 512].rearrange("r (k c) -> r k c", k=bn),
                xT[b0 * 128:(b0 + bn) * 128,
                   psc * 512:psc * 512 + 512].rearrange(
                       "(k r) c -> r k c", k=bn))
            xpf[(psc, bi)] = xt4

        # ---------------------------------------------- thunk generators
        def proj_thunks(sc, fused=False):
            """Projection of chunk sc: KV pass, K-rope, V transposes, then
            Q passes (one PSUM slot each, sequential). With fused=True
            (prologue) all three matmuls run per k-tile, using 3 slots."""
            s0 = sc * 512
            xts = []
            st = {}

            # small first batches so the first matmul starts early; bigger
            # later ones amortize HWDGE overhead
            batches = [2, 2, 4, 4, 4] if sc == 0 else [4, 4, 4, 4]
            starts = [sum(batches[:i]) for i in range(len(batches))]
            kt_slot = {}
            for bi, (b0, bn) in enumerate(zip(starts, batches)):
                for j in range(bn):
                    kt_slot[b0 + j] = (bi, j, bn, b0)

            def dma_kv(kt):
                bi, j, bn, b0 = kt_slot[kt]
                if j == 0:
                    if sc == 0:
                        nc.sync.dma_start(
                            w_sb[:, b0 * WQKV:(b0 + bn) * WQKV].rearrange(
                                "r (k w) -> r k w", k=bn),
                            wqkv[b0 * 128:(b0 + bn) * 128, :].rearrange(
                                "(k r) w -> r k w", k=bn))
                    if (sc, bi) in xpf:
                        xt4 = xpf.pop((sc, bi))
                    else:
                        xt4 = xtp.tile([128, 4 * 512], sdt, tag="xt",
                                       name="xt4")
                        nc.sync.dma_start(
                            xt4[:, 0:bn * 512].rearrange(
                                "r (k c) -> r k c", k=bn),
                            xT[b0 * 128:(b0 + bn) * 128,
                               s0:s0 + 512].rearrange(
                                   "(k r) c -> r k c", k=bn))
                    xts.append(xt4)
                    if sc == 0 and kt == 12:
                        # chunk 1's first x batch jumps the DMA queue ahead
                        # of the trig loads (k_rope only needs those at
                        # ~15us, while window 1 is starved for x); rows
                        # [64:128] of cos/sin are engine-copied from [0:64]
                        prefetch_x(1, 0, 0, 4)
                        nc.sync.dma_start(cos_sb[0:64, :], cosE[:])
                        nc.sync.dma_start(sin_sb[0:64, :], sinE[:])
                        nc.sync.dma_start(utri_sb[:], utri[:])
                if sc == 0 and kt == 15:
                    prefetch_x(1, 1, 4, 4)
                bi, j, bn, b0 = kt_slot[kt]
                xt = xts[bi][:, j * 512:j * 512 + 512]
                if kt == 0:
                    st["pkv"] = pjo.tile([128, 512], f32, tag="pjo",
                                         name="pkv")
                    if fused:
                        st["fq0"] = pjo.tile([128, 512], f32, tag="pjo",
                                             name="fq0")
                        st["fq1"] = pjo.tile([128, 512], f32, tag="pjo",
                                             name="fq1")
                nc.tensor.matmul(
                    st["pkv"][:],
                    _mm_ap(w_sb[:, kt * WQKV + 256:kt * WQKV + 384], mmdt),
                    _mm_ap(xt, mmdt),
                    start=(kt == 0), stop=(kt == KT_D - 1))
                if fused:
                    for mt in range(2):
                        nc.tensor.matmul(
                            st[f"fq{mt}"][:],
                            _mm_ap(w_sb[:, kt * WQKV + mt * 128:
                                        kt * WQKV + mt * 128 + 128], mmdt),
                            _mm_ap(xt, mmdt),
                            start=(kt == 0), stop=(kt == KT_D - 1))

            def k_rope():
                pkv = st["pkv"]
                shufk = work.tile([64, 512], f32, tag="shufk", name="shufk")
                m1k = work.tile([64, 512], f32, tag="m1k", name="m1k")
                t2k = work.tile([64, 512], f32, tag="t2k", name="t2k")
                nc.vector.stream_shuffle(shufk[:], pkv[0:64, :],
                                         _SHUF_SWAP16)
                nc.vector.tensor_mul(m1k[:], pkv[0:64, :],
                                     cos_sb[0:64, s0:s0 + 512])
                nc.vector.tensor_mul(t2k[:], shufk[:],
                                     sin_sb[0:64, s0:s0 + 512])
                nc.vector.tensor_add(KVt[0:64, s0:s0 + 512], m1k[:], t2k[:])
                nc.scalar.copy(KVt[64:128, s0:s0 + 512],
                               pkv[64:128, :])

            def v_trans(kt):
                pv = pw.tile([128, HD], sdt, tag="pw", name="pv")
                nc.tensor.transpose(
                    pv[:], KVt[64:128, kt * 128:(kt + 1) * 128],
                    ident[64:128, 64:128])
                nc.vector.tensor_copy(
                    Vp[:, kt * HD1:kt * HD1 + HD], pv[:])

            def q_mm(mt, kt):
                if kt == 0:
                    st["pq"] = pjo.tile([128, 512], f32, tag="pjo",
                                        name="pq")
                w0 = kt * WQKV + mt * 128
                bi, j, bn, b0 = kt_slot[kt]
                xt = xts[bi][:, j * 512:j * 512 + 512]
                nc.tensor.matmul(
                    st["pq"][:], _mm_ap(w_sb[:, w0:w0 + 128], mmdt),
                    _mm_ap(xt, mmdt),
                    start=(kt == 0), stop=(kt == KT_D - 1))

            def q_rope(mt):
                pq = st[f"fq{mt}"] if fused else st["pq"]
                shuf = work.tile([128, 512], f32, tag="shuf", name="shuf")
                m1 = work.tile([128, 512], f32, tag="m1", name="m1")
                t2 = work.tile([128, 512], f32, tag="t2", name="t2")
                nc.vector.stream_shuffle(shuf[:], pq[:], _SHUF_SWAP16)
                nc.vector.tensor_mul(m1[:], pq[:], cos_sb[:, s0:s0 + 512])
                nc.vector.tensor_mul(t2[:], shuf[:], sin_sb[:, s0:s0 + 512])
                he = (2 * mt) * S
                ho = (2 * mt + 1) * S
                nc.vector.tensor_add(
                    QT[:, he + s0:he + s0 + 512], m1[0:64, :], t2[0:64, :])
                nc.vector.tensor_add(
                    QT[:, ho + s0:ho + s0 + 512], m1[64:128, :],
                    t2[64:128, :])

            def cs_dup():
                # duplicate cos/sin rows [0:64] into [64:128] for q_rope
                # (cos on idle ACT, sin on idle GPSIMD, in parallel)
                nc.scalar.copy(cos_sb[64:128, :], cos_sb[0:64, :])
                nc.gpsimd.tensor_copy(sin_sb[64:128, :], sin_sb[0:64, :])

            th = [lambda kt=kt: dma_kv(kt) for kt in range(KT_D)]
            th.append(k_rope)
            if sc == 0:
                th.append(cs_dup)
            th += [lambda kt=kt: v_trans(kt)
                   for kt in range(4 * sc, 4 * sc + 4)]
            if fused:
                th += [lambda mt=mt: q_rope(mt) for mt in range(2)]
            else:
                for mt in range(2):
                    th += [lambda mt=mt, kt=kt: q_mm(mt, kt)
                           for kt in range(KT_D)]
                    th.append(lambda mt=mt: q_rope(mt))
            return th

        def s_thunks(qc, h, tiles):
            """S^T mega matmuls + exp + mask for one head; fills `tiles`."""
            q0 = qc * 512
            hf = h * S
            nkt = 4 * qc + 4
            thunks = []
            for pi in range(nkt // 2):
                def th(pi=pi):
                    kts = (2 * pi, 2 * pi + 1)
                    ps_t = ps.tile([128, 1024], f32, tag="ps", name="ps_t")
                    pt_t = ptp.tile([128, 1024], sdt, tag="pt", name="pt_t")
                    for li, kt in enumerate(kts):
                        dj = kt - 4 * qc
                        qo = 128 * dj if dj >= 0 else 0
                        lo = li * 512
                        nc.tensor.matmul(
                            ps_t[:, lo + qo:lo + 512],
                            _mm_ap(KVt[0:64, kt * 128:(kt + 1) * 128], mmdt),
                            _mm_ap(QT[:, hf + q0 + qo:hf + q0 + 512], mmdt),
                            start=True, stop=True)
                    if 2 * pi + 1 < 4 * qc or (qc >= 1
                                               and 2 * pi == 4 * qc):
                        # first diag pair (dj 0,1) of chunks >= 1: exp the
                        # full mega in one op; the extra columns hold stale
                        # scores (bounded, and never read by the series-
                        # major PV), and one wide op beats two narrow ones
                        # on the exp-saturated ACT engine
                        nc.scalar.activation(
                            pt_t[:], ps_t[:],
                            mybir.ActivationFunctionType.Exp, scale=SCALE)
                    else:
                        for li, kt in enumerate(kts):
                            dj = kt - 4 * qc
                            qo = 128 * dj if dj >= 0 else 0
                            lo = li * 512
                            nc.scalar.activation(
                                pt_t[:, lo + qo:lo + 512],
                                ps_t[:, lo + qo:lo + 512],
                                mybir.ActivationFunctionType.Exp,
                                scale=SCALE)
                    for li, kt in enumerate(kts):
                        dj = kt - 4 * qc
                        qo = 128 * dj if dj >= 0 else 0
                        lo = li * 512
                        if dj >= 0:
                            eng = nc.gpsimd if MASK_POOL else nc.vector
                            eng.tensor_mul(
                                pt_t[:, lo + qo:lo + qo + 128],
                                pt_t[:, lo + qo:lo + qo + 128], utri_sb[:])
                        tiles.append((kt, qo, lo, pt_t))
                thunks.append(th)
            return thunks

        def pv_thunks(qc, h, tiles):
            """q-major PV accumulation + normalization + O transposes.

            For each incoming P^T tile (k-tile kt), run the N=65 matmuls for
            each live 128-q subtile j: out[q,hd|den] += P^T[:,j*128:].T@Vp.
            After the last k-tile: reciprocal of the denominators, normalize
            psum->O_sb (bf16), PE-transpose to O^T, 2x DVE copy into OT.
            """
            q0 = qc * 512
            hp = (h % 2) * 64
            of = (h // 2) * S
            nkt0 = 4 * qc + 4
            state = {}

            def pv_series(j):
                """All matmuls of q-subtile j back-to-back: start=True
                clears the whole bank's has_written bits, so accumulation
                series sharing a psum bank must not interleave."""
                if "pv" not in state:
                    state["pv"] = pjo.tile([128, 512], f32, tag="pjo",
                                           name="pvt")
                pv_t = state["pv"]
                for kt in range(0, 4 * qc + j + 1):
                    _, qo, lo, pt_t = tiles[kt]
                    nc.tensor.matmul(
                        pv_t[:, j * 128:j * 128 + HD1],
                        _mm_ap(pt_t[:, lo + j * 128:
                                    lo + (j + 1) * 128], mmdt),
                        _mm_ap(Vp[:, kt * HD1:(kt + 1) * HD1], mmdt),
                        start=(kt == 0), stop=(kt == 4 * qc + j))

            def norm():
                pv_t = state["pv"]
                rc = small.tile([128, 4], f32, tag="rc", name="rc")
                dsb = small.tile([128, 4], f32, tag="dsb", name="dsb")
                o_sb = state["o_sb"] = osb_head()
                pvr = pv_t.rearrange("p (b c) -> p b c", c=128)
                nc.vector.tensor_copy(
                    dsb[:].rearrange("p (b c) -> p b c", c=1),
                    pvr[:, :, HD:HD + 1])
                nc.vector.reciprocal(rc[:], dsb[:])
                for j in range(4):
                    nc.vector.tensor_scalar_mul(
                        o_sb[:, j * HD:(j + 1) * HD],
                        pv_t[:, j * 128:j * 128 + HD],
                        rc[:, j:j + 1])

            def trans(half):
                # transpose via a regular matmul against the identity:
                # out[hd, q] = o_sb[q, hd]^T @ I -- f32 psum output, so
                # nonzero free offsets behave like the score megas
                o_sb = state["o_sb"]
                tp = pw.tile([64, 256], f32, tag="pw", name="tp")
                for st2 in range(2):
                    stg = 2 * half + st2
                    nc.tensor.matmul(
                        tp[:, st2 * 128:(st2 + 1) * 128],
                        _mm_ap(o_sb[:, stg * HD:(stg + 1) * HD], mmdt),
                        _mm_ap(ident[:], mmdt),
                        start=True, stop=True)
                oc_eng = nc.scalar if qc <= 1 else nc.vector
                (oc_eng.copy if qc <= 1 else nc.vector.tensor_copy)(
                    OT[hp:hp + 64, of + q0 + half * 256:
                       of + q0 + half * 256 + 256], tp[:])

            def osb_head():
                return opool.tile([128, 4 * HD], sdt, tag=f"osb{h % 2}",
                                  name="o_sb")

            th = [lambda j=j: pv_series(j) for j in range(4)]
            th.append(norm)
            th += [lambda half=half: trans(half) for half in range(2)]
            return th

        def wo_half(qt, np2, half, obs, pool=None, ptag="pw",
                    copy_eng="mix", split_dma=False):
            """One 512-wide n-chunk of out row-block qt; the final chunk
            fires one [128,2048] fp16 DMA for the whole row-block (fewer
            HWDGE round-trips than per-chunk DMAs). The epilogue splits
            per-np2 ([128,1024]) so the last transfer is shorter."""
            pool = pool or pw
            if (np2, half) == (0, 0):
                obs[qt] = osb.tile([128, 2048], f16, tag="ob", name="ob")
            ob = obs[qt]
            ncn = 2 * np2 + half
            pw_t = pool.tile([128, 512], f32, tag=ptag, name="pw_t")
            for mt in range(2):
                nc.tensor.matmul(
                    pw_t[:],
                    _mm_ap(OT[:, mt * S + qt * 128:
                              mt * S + (qt + 1) * 128], mmdt),
                    _mm_ap(wo_sb[:, mt * DIM + ncn * 512:
                                 mt * DIM + ncn * 512 + 512], mmdt),
                    start=(mt == 0), stop=(mt == 1))
            wo_copy_cnt[0] += 1
            use_act = {"mix": wo_copy_cnt[0] % 3 == 0,
                       "dve": False,
                       "alt": wo_copy_cnt[0] % 2 == 1}[copy_eng]
            if use_act:
                nc.scalar.copy(ob[:, ncn * 512:ncn * 512 + 512], pw_t[:])
            else:
                nc.vector.tensor_copy(
                    ob[:, ncn * 512:ncn * 512 + 512], pw_t[:])
            if split_dma and half == 1:
                nc.sync.dma_start(
                    out[qt * 128:(qt + 1) * 128,
                        np2 * 1024:np2 * 1024 + 1024],
                    ob[:, np2 * 1024:np2 * 1024 + 1024])
                if np2 == 1:
                    del obs[qt]
            elif not split_dma and (np2, half) == (1, 1):
                del obs[qt]
                nc.sync.dma_start(
                    out[qt * 128:(qt + 1) * 128, :], ob[:])
        wo_obs = {}

        # ------------------------------------- merged emission schedule
        def merge(primary, *others):
            """Emit primary thunks; proportionally interleave the others."""
            counters = [0.0] * len(others)
            n = max(1, len(primary))
            for beat in primary:
                for j, lst in enumerate(others):
                    counters[j] += len(lst) / n
                    while counters[j] >= 1.0 and lst:
                        lst.pop(0)()
                        counters[j] -= 1.0
                for th in beat:
                    th()
            for lst in others:
                while lst:
                    lst.pop(0)()

        for th in proj_thunks(0, fused=True):       # prologue
            th()

        # Head processing order: chunk 3's head 0 is pulled forward between
        # (2,2) and (2,3) so part of the causal-triangle-heavy chunk-3 exp
        # load runs while ACT still has slack, instead of piling up at the
        # end where exp rate-limits the whole pipeline.
        ITEMS = [(0, 0), (0, 1), (0, 2), (0, 3),
                 (1, 0), (1, 1), (1, 2), (1, 3),
                 (2, 0), (2, 1), (2, 2), (3, 0), (3, 1), (2, 3),
                 (3, 2), (3, 3)]
        # proj(sc) spread over items [a, b) — must drain before the first
        # (sc, *) item; WO(sc) over [a, b) — may start only after the item
        # containing trans(sc, 3), i.e. one past (sc, 3)'s position.
        PROJ_AT = {1: (0, 4), 2: (4, 8), 3: (9, 11)}
        WO_AT = {0: (6, 8), 1: (12, 14), 2: (15, 16)}
        proj_by_start = {a: s for s, (a, b) in PROJ_AT.items()}
        wo_by_start = {a: s for s, (a, b) in WO_AT.items()}

        prev = None                      # (qc, h, tiles) awaiting PV
        pstream, pend = [], 0
        wopending, woend = [], 0
        for i, (sc, h) in enumerate(ITEMS):
            if i == 4:
                nc.sync.dma_start(wo_sb[:, 0:DIM], wo_s[0:128, :])
                nc.sync.dma_start(wo_sb[:, DIM:2 * DIM], wo_s[128:256, :])
            if i in proj_by_start:
                pstream = proj_thunks(proj_by_start[i])
                pend = PROJ_AT[proj_by_start[i]][1]
            if i in wo_by_start:
                s = wo_by_start[i]
                wopending = [(qt, np2, half)
                             for qt in range(4 * s, 4 * s + 4)
                             for np2 in range(2)
                             for half in range(2)]
                woend = WO_AT[s][1]
            tiles = []
            sth = s_thunks(sc, h, tiles)
            pth = pv_thunks(*prev) if prev is not None else []
            beats = []
            for bi in range(max(len(sth), len(pth))):
                beat = []
                if bi < len(pth):
                    beat.append(pth[bi])
                if bi < len(sth):
                    beat.append(sth[bi])
                beats.append(beat)
            others = []
            if pstream:
                rem = max(1, pend - i)
                ptake = len(pstream) if rem <= 1 else len(pstream) // rem
                others.append(pstream[:ptake])
                pstream = pstream[ptake:]
            if wopending:
                rem = max(1, woend - i)
                wtake = (len(wopending) if rem <= 1
                         else len(wopending) // rem)
                # after proj(3) is done (items >= 11) pjo is mostly idle:
                # route WO psum there and keep its copies off saturated ACT
                wpool, wtag, weng = ((pjo, "pjo", "dve") if i >= 11
                                     else (None, "pw", "mix"))
                others.append(
                    [lambda qt=u[0], np2=u[1], half=u[2]:
                     wo_half(qt, np2, half, wo_obs, pool=wpool,
                             ptag=wtag, copy_eng=weng)
                     for u in wopending[:wtake]])
                wopending = wopending[wtake:]
            merge(beats, *others)
            prev = (sc, h, tiles)
        assert not pstream and not wopending

        # epilogue: PV of the last head, then WO of chunk 3; the score
        # pool's banks are free now, so WO rotates through those too.
        # trans(0) covers q-tiles 12-13, trans(1) covers 14-15: start the
        # WO units for each pair as soon as its OT columns are complete.
        pth = pv_thunks(*prev)
        for th in pth[:-1]:          # pairs + norm + trans(0)
            th()
        epi = [0]
        pools = [(pw, "pw"), (ps, "ps"), (pjo, "pjo")]

        def epi_wo(qt, np2):
            pool, ptag = pools[epi[0] % 3]
            for half in range(2):
                wo_half(qt, np2, half, wo_obs, pool=pool, ptag=ptag,
                        copy_eng="mix", split_dma=True)
            epi[0] += 1

        epi_wo(12, 0)
        pth[-1]()                    # trans(1) overlaps qt-12 copies
        epi_wo(12, 1)
        for qt in range(13, 16):
            for np2 in range(2):
                epi_wo(qt, np2)

    nc.compile()
    return nc


# ------------------------------------------------------------- host side
def _pair_perm64():
    """Column permutation putting the RoPE partner 16 partitions away."""
    return np.array([2 * (16 * (j // 32) + (j % 16)) + ((j % 32) // 16)
                     for j in range(64)])


def _host_prep(x, freqs_cos, freqs_sin, wq, wk, wv, wo):
    _, _, npdt = _dtypes()
    x = np.asarray(x, np.float32)
    fc = np.asarray(freqs_cos, np.float32)
    fs = np.asarray(freqs_sin, np.float32)
    wq = np.asarray(wq, np.float32)
    wk = np.asarray(wk, np.float32)
    wv = np.asarray(wv, np.float32)
    wo = np.asarray(wo, np.float32)

    perm = _pair_perm64()
    xT = np.ascontiguousarray(x[0].T).astype(npdt)

    p = np.arange(64)
    pair = 16 * ((p % 64) // 32) + (p % 16)
    sign = np.where((p % 32) < 16, -1.0, 1.0).astype(np.float32)
    cosE = np.ascontiguousarray(fc[:, pair].T)                  # [64, S]
    sinE = np.ascontiguousarray(fs[:, pair].T) * sign[:, None]  # [64, S]
    utri = np.triu(np.ones((128, 128), np.float32)).astype(npdt)

    in_maps = []
    for c in range(NCORES):
        qcols = np.concatenate(
            [wq[:, (4 * c + i) * 64 + perm] for i in range(HQ)], axis=1)
        kcols = wk[:, c * 64 + perm]
        vcols = wv[:, c * 64:(c + 1) * 64]
        wqkv_c = np.concatenate([qcols, kcols, vcols], axis=1).astype(npdt)
        wo_c = wo[QW * c:QW * (c + 1), :].astype(npdt)
        in_maps.append({
            "xT": xT,
            "wqkv": np.ascontiguousarray(wqkv_c),
            "wo_s": np.ascontiguousarray(wo_c),
            "cosE": cosE.astype(np.float32),
            "sinE": np.ascontiguousarray(sinE).astype(np.float32),
            "utri": np.ascontiguousarray(utri),
        })
    return in_maps


_NC_CACHE = {}


def get_program():
    if MM not in _NC_CACHE:
        _NC_CACHE[MM] = build_program()
    return _NC_CACHE[MM]


def kernel(x, freqs_cos, freqs_sin, wq, wk, wv, wo):
    nc = get_program()
    in_maps = _host_prep(x, freqs_cos, freqs_sin, wq, wk, wv, wo)
    res = run_bass_kernel_spmd(nc, in_maps, core_ids=list(range(NCORES)))
    acc = np.zeros((S, DIM), np.float64)
    for r in res.results:
        acc += r["out"].astype(np.float64)
    return acc.astype(np.float32).reshape(1, S, DIM)
